# revision 16
# baseline (speedup 1.0000x reference)
"""Trainium2 Bass kernel for nn_DecoderLayer_33758442946809.

Sharding (8 cores = 2 batches x 4-core groups):
- Self-attention is HEAD-sharded: core (b, j) computes heads 4j..4j+3 for
  all T=2048 rows of batch b; causal skipping is SPMD-uniform (only
  lower-triangle key blocks are scored/exp'd).
- W1 is row-parallel over the head-sharded cat features; partials are
  summed with a chunked ReduceScatter (bf16) over each 4-core group.
  After the RS, core (b, j) owns the strided row set
  {512*rc + 128*j + i : rc<4, i<128}; cross-attention, W2 and the FFN
  are data-parallel over those rows.
- tile(attn2, H) @ W2 == attn2 @ sum_h W2[h] (host precomputes the sum).

v2 schedule (vs the phase-serial v1):
- Row chunks processed big-first [3,2,1,0] so the last ReduceScatter has
  the shortest dependency tail.
- Score PSUM is [128,1024] tiles (1 key block, both heads) with bufs=2,
  so ACT exp streams without stalling on PSUM WAR.
- Softmax normalization: denominator row is reciprocal'd at [1,2,512]
  (cheap) then partition-broadcast; the numerator is multiplied straight
  out of PSUM -- no [65,512] evictions, no 6.5us [64,1024] reciprocals.
- QKV projection matmuls (q tcols 2..0, all of v) drain as pending PE
  work under the first row-chunk's exp stream.
- RS outputs land via the sync queue (the gpsimd queue used to block
  ~90us on the RS-done semaphore).
- LN affine ops are skipped when gamma==1/beta==0 (host-checked program
  variant); the attention 1/sqrt(dk) scale is folded into LN1's rstd.
- Transposes run in bf16 (pre-cast) -- 1 PE cycle/row instead of 2.
- xT / x_tm / Wf1 / W2sum are prefetched a phase early.
- FFN1 relu+bias runs on ACT (idle there) instead of DVE.
- FFN2's last 4 weight blocks run row-major so row block 0 finishes
  ~6us early and the final LN3+store overlaps the remaining matmuls.
"""
import math
import sys

import numpy as np

sys.path.insert(0, "/opt/trn_rl_repo")

import ml_dtypes  # noqa: E402

import concourse.bass as bass  # noqa: E402
import concourse.tile as tile  # noqa: E402
from concourse import bacc, mybir  # noqa: E402
from concourse.bass_utils import run_bass_kernel_spmd  # noqa: E402
from concourse.masks import make_identity  # noqa: E402

B, S, D, H, DF = 2, 2048, 1024, 16, 4096
DK = D // H                      # 64
P = 128
T = S                            # rows/keys per batch
R = 512                          # own rows per core (after RS)
KC = D // P                      # 8 contraction chunks of D
TB = T // P                      # 16 key blocks
RB = R // P                      # 4 row blocks
FB = DF // P                     # 32 ffn blocks
NCORES = 8
HL = 4                           # local heads per core
SCALE = 1.0 / math.sqrt(DK)
RG = [[0, 1, 2, 3], [4, 5, 6, 7]]

F32 = mybir.dt.float32
BF16 = mybir.dt.bfloat16
AF = mybir.ActivationFunctionType
ALU = mybir.AluOpType

_cached = {}


def build_nc(f):
    nc = bacc.Bacc("TRN2", target_bir_lowering=False, debug=False,
                   num_devices=NCORES)

    dram = {}

    def din(name, shape, dt):
        dram[name] = nc.dram_tensor(name, shape, dt, kind="ExternalInput").ap()

    din("yT", [D, T], BF16)          # y[b].T
    din("wq", [P, KC * HL * DK], BF16)   # pre-chunked [p][kc][f] layout
    din("wk", [P, KC * HL * DK], BF16)
    din("wv", [P, KC * HL * DK], BF16)
    din("bq_s", [HL * DK], F32)      # bq * SCALE, local heads
    din("bk_f", [HL * DK], F32)
    din("bv_f", [HL * DK], F32)
    din("mask4", [P, 4, R], BF16)    # diagonal-block masks (key, i, row)
    din("w1loc", [P, 2 * D], BF16)   # W1 rows owned by this core, pre-chunked
    din("b1", [D], F32)
    din("ln1_g", [D], F32)
    din("ln1_b", [D], F32)
    din("y_rows", [R, D], F32)       # this core's (strided) y rows
    din("xT", [D, T], BF16)
    din("x_tm", [T, D], BF16)
    din("W2sum", [D, D], BF16)
    din("b2", [D], F32)
    din("ln2_g", [D], F32)
    din("ln2_b", [D], F32)
    din("Wf1", [P, KC * DF], BF16)   # pre-chunked [p][kc][f] layout
    din("bf1", [DF], F32)
    din("Wf2", [DF, D], BF16)
    din("bf2", [D], F32)
    din("ln3_g", [D], F32)
    din("ln3_b", [D], F32)
    out_d = nc.dram_tensor("out", [R, D], F32, kind="ExternalOutput").ap()

    with tile.TileContext(nc) as tc:
        _build(nc, tc, dram, out_d, f)
    nc.compile()
    return nc


def _build(nc, tc, d, out_d, f):
    pool_cms = {}

    def open_pool(*args, **kw):
        cm = tc.tile_pool(*args, **kw)
        p = cm.__enter__()
        pool_cms[id(p)] = cm
        return p

    def close_pool(p):
        pool_cms.pop(id(p)).__exit__(None, None, None)

    const = open_pool(name="const", bufs=1, side="left")
    ident = const.tile([P, P], BF16, name="ident", tag="ident")
    make_identity(nc, ident[:])
    ones_col = const.tile([P, 1], BF16, name="ones_col", tag="ones_col")
    nc.vector.memset(ones_col[:], 1.0)
    ones_row = const.tile([1, P], BF16, name="ones_row", tag="ones_row")
    nc.vector.memset(ones_row[:], 1.0)
    eps_sb = const.tile([P, 1], F32, name="eps", tag="eps")
    nc.vector.memset(eps_sb[:], 1e-5)
    # eps/SCALE^2: sqrt(var/SCALE^2 + eps/SCALE^2) = sqrt(var+eps)/SCALE,
    # so LN1's rstd comes out pre-multiplied by the attention scale.
    eps_s2 = const.tile([P, 1], F32, name="eps_s2", tag="eps_s2")
    nc.vector.memset(eps_s2[:], 1e-5 / (SCALE * SCALE))

    def bias_chunks(pool, name, n):
        t = pool.tile([P, n], F32, name=f"bc_{name}", tag=f"bc_{name}")
        nc.sync.dma_start(out=t[:], in_=d[name].rearrange("(n p) -> p n", p=P))
        return t

    def bcast_row(pool, name):
        src = d[name]
        t = pool.tile([P, D], F32, name=f"br_{name}", tag=f"br_{name}")
        bc = bass.AP(tensor=src.tensor, offset=src.offset,
                     ap=[[0, P]] + list(src.ap))
        nc.sync.dma_start(out=t[:], in_=bc)
        return t

    def ln_slim(pool, x_ap, out_ap, eps_ap, var_scale=1.0, g_b=None,
                be_b=None):
        """LayerNorm along the free axis (D) of a token-major [128, D]
        f32 tile into out_ap. var_scale folds a constant into rstd."""
        x3 = x_ap.rearrange("p (n f) -> p n f", f=512)
        stats = pool.tile([P, 2, 6], F32, name="ln_stats", tag="ln_stats",
                          bufs=4)
        for sg in range(2):
            nc.vector.bn_stats(out=stats[:, sg, :], in_=x3[:, sg, :])
        mv = pool.tile([P, 2], F32, name="ln_mv", tag="ln_mv", bufs=4)
        nc.vector.bn_aggr(out=mv[:], in_=stats[:])
        std = pool.tile([P, 1], F32, name="ln_std", tag="ln_std", bufs=4)
        nc.scalar.activation(out=std[:], in_=mv[:, 1:2], func=AF.Sqrt,
                             bias=eps_ap[:], scale=var_scale)
        rstd = pool.tile([P, 1], F32, name="ln_rstd", tag="ln_rstd", bufs=4)
        nc.vector.reciprocal(out=rstd[:], in_=std[:])
        nc.vector.tensor_scalar(out=out_ap, in0=x_ap, scalar1=mv[:, 0:1],
                                scalar2=rstd[:], op0=ALU.subtract,
                                op1=ALU.mult)
        if g_b is not None:
            nc.vector.tensor_mul(out=out_ap, in0=out_ap, in1=g_b)
        if be_b is not None:
            nc.vector.tensor_add(out=out_ap, in0=out_ap, in1=be_b)

    # ======== pools whose tiles live into ph4 (right-side bottom) =======
    xpre = open_pool(name="xpre", bufs=1, side="right")
    # only half of xT is prefetched (SBUF is tight during rc=3);
    # xT[4..7] load at ph4 open and are the last kcs of each score group
    xT = [xpre.tile([P, T], BF16, name=f"xT{i}", tag=f"xT{i}")
          if i < 4 else None for i in range(KC)]
    a1pl = open_pool(name="a1pl", bufs=1, side="right")
    a1T = [a1pl.tile([P, R], BF16, name=f"a1T{i}", tag=f"a1T{i}")
           for i in range(KC)]

    # ===================== input DMAs (spread across queues) ============
    attn = open_pool(name="attn", bufs=1, side="right")  # live through ph2
    qTp = [attn.tile([P, T], BF16, name=f"qTp{i}", tag=f"qTp{i}")
           for i in range(2)]
    kTp = [attn.tile([P, T], BF16, name=f"kTp{i}", tag=f"kTp{i}")
           for i in range(2)]
    v_sb = [attn.tile([P, HL, DK + 1], BF16, name=f"v{i}", tag=f"v{i}")
            for i in range(TB)]
    mask4 = attn.tile([P, 4, R], BF16, name="mask4", tag="mask4")

    ph2w = open_pool(name="ph2w", bufs=1, side="left")   # ph2 working set
    ph1 = open_pool(name="ph1", bufs=1, side="left")     # closed after rc=3
    yT = [ph1.tile([P, T], BF16, name=f"yT{i}", tag=f"yT{i}")
          for i in range(KC)]
    wq_sb = ph1.tile([P, KC, 2 * P], BF16, name="wq", tag="wq")
    wk_sb = ph1.tile([P, KC, 2 * P], BF16, name="wk", tag="wk")
    wv_sb = ph1.tile([P, KC, 2 * P], BF16, name="wv", tag="wv")
    for kc in range(KC):
        nc.sync.dma_start(out=yT[kc][:], in_=d["yT"][kc * P:(kc + 1) * P, :])
    nc.scalar.dma_start(out=wk_sb[:],
                        in_=d["wk"].rearrange("p (c f) -> p c f", c=KC))
    nc.scalar.dma_start(out=wq_sb[:],
                        in_=d["wq"].rearrange("p (c f) -> p c f", c=KC))
    nc.scalar.dma_start(out=wv_sb[:],
                        in_=d["wv"].rearrange("p (c f) -> p c f", c=KC))
    nc.gpsimd.dma_start(out=mask4[:], in_=d["mask4"][:])
    bq_sb = bias_chunks(ph1, "bq_s", 2)
    bk_sb = bias_chunks(ph1, "bk_f", 2)
    bv_b = ph1.tile([P, 2 * P], F32, name="bv_b", tag="bv_b")
    bv_src = d["bv_f"]
    nc.sync.dma_start(out=bv_b[:], in_=bass.AP(
        tensor=bv_src.tensor, offset=bv_src.offset,
        ap=[[0, P]] + list(bv_src.ap)))
    # cross-attention keys prefetched behind yT on the sync queue
    for kc in range(4):
        nc.sync.dma_start(out=xT[kc][:], in_=d["xT"][kc * P:(kc + 1) * P, :])

    # ============ ph1 QKV emission (k full, q tcol 3; rest pending) =====
    # psX: shared 1-bank [P,512] ring for q/v projections, W1 partials and
    # LN1 transposes (keeps psS at bufs=2 within the 8-bank budget)
    psX = open_pool(name="psX", bufs=1, space="PSUM", side="left")
    pending = []
    drained = [0]

    def drain(k):
        for _ in range(min(k, len(pending))):
            pending.pop(0)()
            drained[0] += 1

    def px_tile():
        return psX.tile([P, 512], F32, name="px", tag="px", bufs=2)

    def qk_group(dst, w_sb, b_sb, p, tcol):
        ps = px_tile()
        for kc in range(KC):
            nc.tensor.matmul(ps[:], lhsT=w_sb[:, kc, p * P:(p + 1) * P],
                             rhs=yT[kc][:, tcol * 512:(tcol + 1) * 512],
                             start=(kc == 0), stop=(kc == KC - 1))
        nc.vector.tensor_scalar(out=dst[p][:, tcol * 512:(tcol + 1) * 512],
                                in0=ps[:], scalar1=b_sb[:, p:p + 1],
                                scalar2=None, op0=ALU.add)

    # k: all tcols (every score block needs all keys); q: tcol 3 first
    for p in range(2):
        for tcol in range(4):
            qk_group(kTp, wk_sb, bk_sb, p, tcol)
    for p in range(2):
        qk_group(qTp, wq_sb, bq_sb, p, 3)

    def v_work(tb):
        work = []
        box = [None]

        def v_start():
            nc.vector.memset(v_sb[tb][:, :, DK:DK + 1], 1.0)
            box[0] = px_tile()

        def v_mm(kc):
            nc.tensor.matmul(box[0][:, 0:2 * P],
                             lhsT=yT[kc][:, tb * P:(tb + 1) * P],
                             rhs=wv_sb[:, kc, :],
                             start=(kc == 0), stop=(kc == KC - 1))

        def v_evict():
            nc.vector.tensor_add(
                out=v_sb[tb][:, :, 0:DK],
                in0=box[0][:, 0:2 * P].rearrange("p (h k) -> p h k", h=HL),
                in1=bv_b[:].rearrange("p (h k) -> p h k", h=HL))

        work.append(v_start)
        work.extend(lambda kc=kc: v_mm(kc) for kc in range(KC))
        work.append(v_evict)
        return work

    def q_work(p, tcol):
        work = []
        box = [None]

        def q_start():
            box[0] = px_tile()

        def q_mm(kc):
            nc.tensor.matmul(box[0][:],
                             lhsT=wq_sb[:, kc, p * P:(p + 1) * P],
                             rhs=yT[kc][:, tcol * 512:(tcol + 1) * 512],
                             start=(kc == 0), stop=(kc == KC - 1))

        def q_evict():
            nc.vector.tensor_scalar(
                out=qTp[p][:, tcol * 512:(tcol + 1) * 512],
                in0=box[0][:], scalar1=bq_sb[:, p:p + 1],
                scalar2=None, op0=ALU.add)

        work.append(q_start)
        work.extend(lambda kc=kc: q_mm(kc) for kc in range(KC))
        work.append(q_evict)
        return work

    # v must be fully projected before the first attnV drains; emit v
    # first, then the remaining q columns.
    for tb in range(TB):
        pending.extend(v_work(tb))
    for tcol in (2, 1, 0):
        for p in range(2):
            pending.extend(q_work(p, tcol))
    ph1_work_n = len(pending)

    # ============ ph2: causal attention + W1 + ReduceScatter ============
    cat = open_pool(name="cat", bufs=1, side="right")     # catT, ph2-long
    catT = [cat.tile([P, T], BF16, name=f"catT{i}", tag=f"catT{i}")
            for i in range(2)]
    ph3 = open_pool(name="ph3", bufs=1, side="right")     # W1/LN1 working
    w1_sb = ph3.tile([P, 2, D], BF16, name="w1", tag="w1")
    nc.gpsimd.dma_start(out=w1_sb[:],
                        in_=d["w1loc"].rearrange("p (c n) -> p c n", c=2))
    a1pre_box = {}
    if not f["b1_zero"]:
        f["b1_b"] = bcast_row(ph3, "b1")
    if not f["ln1_unit_g"]:
        f["g1_b"] = bcast_row(ph3, "ln1_g")
    if not f["ln1_zero_b"]:
        f["be1_b"] = bcast_row(ph3, "ln1_b")

    dramp = open_pool(name="dramp", bufs=1, space="DRAM", side="left")
    rs_in = [dramp.tile([4 * P, D], BF16, name=f"rsi{i}", tag=f"rsi{i}")
             for i in range(RB)]
    rs_out = [dramp.tile([P, D], BF16, name=f"rso{i}", tag=f"rso{i}")
              for i in range(RB)]

    psS = open_pool(name="psS", bufs=1, space="PSUM", side="left")
    psV = open_pool(name="psV", bufs=1, space="PSUM", side="left")

    def make_attn_work(rc, p, expP):
        nkb = 4 * rc + 4
        work = []
        pa_t = [None, None]

        def start_head(hh):
            pa_t[hh] = psV.tile([DK + 1, 512], F32, name="pa", tag="pa",
                                bufs=2)

        def mm_head(hh, kb):
            hl = 2 * p + hh
            nc.tensor.matmul(pa_t[hh][:], lhsT=v_sb[kb][:, hl, :],
                             rhs=expP[:, kb, hh, :],
                             start=(kb == 0), stop=(kb == nkb - 1))

        den_row = [None]

        def evict_den(hh):
            if hh == 0:
                den_row[0] = ph2w.tile([1, 2, 512], BF16, name="den_row",
                                       tag="den_row", bufs=2)
            nc.vector.tensor_copy(out=den_row[0][:, hh, :],
                                  in_=pa_t[hh][DK:DK + 1, :])

        recB = [None]

        def recip_bcast():
            rec_row = ph2w.tile([1, 2, 512], BF16, name="rec_row",
                                tag="rec_row", bufs=2)
            with nc.allow_low_precision(reason="softmax denom bf16 ok"):
                nc.vector.reciprocal(out=rec_row[:], in_=den_row[0][:])
            recB[0] = ph2w.tile([DK, 2, 512], BF16, name="recB", tag="recB",
                                bufs=2)
            nc.gpsimd.partition_broadcast(recB[0][:], rec_row[:])

        def mul_head(hh):
            nc.vector.tensor_mul(
                out=catT[p][hh * DK:(hh + 1) * DK,
                            rc * 512:(rc + 1) * 512],
                in0=pa_t[hh][0:DK, :], in1=recB[0][:, hh, :])

        for hh in range(2):
            work.append(lambda hh=hh: start_head(hh))
            for kb in range(nkb):
                work.append(lambda hh=hh, kb=kb: mm_head(hh, kb))
            work.append(lambda hh=hh: evict_den(hh))
        work.append(recip_bcast)
        work.append(lambda: mul_head(0))
        work.append(lambda: mul_head(1))
        return work

    y_box = {}

    def make_w1_work(rc):
        work = []

        def w1_block(rb, nt, box):
            c0 = rc * 512 + rb * P
            if nt == 0:
                box[0] = ph2w.tile([P, D], BF16, name="a1p", tag="a1p",
                                   bufs=2)
            psw = px_tile()
            for kc2 in range(2):
                nc.tensor.matmul(psw[:],
                                 lhsT=catT[kc2][:, c0:c0 + P],
                                 rhs=w1_sb[:, kc2, nt * 512:(nt + 1) * 512],
                                 start=(kc2 == 0), stop=(kc2 == 1))
            nc.vector.tensor_copy(out=box[0][:, nt * 512:(nt + 1) * 512],
                                  in_=psw[:])
            if nt == 1:
                nc.gpsimd.dma_start(out=rs_in[rc][rb * P:(rb + 1) * P, :],
                                    in_=box[0][:])

        for rb in range(4):
            box = [None]
            for nt in range(2):
                work.append(lambda rb=rb, nt=nt, box=box: w1_block(rb, nt, box))

        def do_rs():
            nc.gpsimd.collective_compute(
                "ReduceScatter", ALU.add, replica_groups=RG,
                ins=[rs_in[rc][:].opt()], outs=[rs_out[rc][:].opt()])
            # y residual rows for this chunk (sync queue, no waits)
            yb = ph3.tile([P, D], F32, name="y_sb", tag="y_sb", bufs=1)
            nc.sync.dma_start(out=yb[:],
                              in_=d["y_rows"][rc * P:(rc + 1) * P, :])
            y_box[rc] = yb
            # RS result lands via sync queue (gpsimd must stay unblocked)
            ap = ph3.tile([P, D], BF16, name="a1pre", tag="a1pre", bufs=2)
            nc.sync.dma_start(out=ap[:], in_=rs_out[rc][:])
            a1pre_box[rc] = ap
        work.append(do_rs)
        return work

    def make_ln1_work(rc):
        work = []
        a1 = [None]
        fold = f["ln1_unit_g"] and f["ln1_zero_b"]

        def residual():
            a1[0] = ph3.tile([P, D], F32, name="a1", tag="a1", bufs=1)
            nc.vector.tensor_add(out=a1[0][:], in0=a1pre_box[rc][:],
                                 in1=y_box[rc][:])
            if not f["b1_zero"]:
                nc.vector.tensor_add(out=a1[0][:], in0=a1[0][:],
                                     in1=f["b1_b"][:])

        a1b = [None]

        def ln():
            a1b[0] = ph3.tile([P, D], BF16, name="a1b", tag="a1b", bufs=1)
            if fold:
                # rstd folds SCALE -> a1T comes out pre-scaled
                ln_slim(ph3, a1[0][:], a1b[0][:], eps_s2,
                        var_scale=1.0 / (SCALE * SCALE))
            else:
                ln_slim(ph3, a1[0][:], a1b[0][:], eps_sb,
                        g_b=None if f["ln1_unit_g"] else f["g1_b"][:],
                        be_b=None if f["ln1_zero_b"] else f["be1_b"][:])

        def tr(kc):
            pt = px_tile()[:].bitcast(BF16)[:, 0:P]   # bf16 view of psum
            nc.tensor.transpose(pt, a1b[0][:, kc * P:(kc + 1) * P],
                                ident[:])
            if fold:
                nc.vector.tensor_copy(out=a1T[kc][:, rc * P:(rc + 1) * P],
                                      in_=pt)
            else:
                nc.vector.tensor_scalar(
                    out=a1T[kc][:, rc * P:(rc + 1) * P], in0=pt,
                    scalar1=float(SCALE), scalar2=None, op0=ALU.mult)

        work.append(residual)
        work.append(ln)
        work.extend(lambda kc=kc: tr(kc) for kc in range(KC))
        return work

    ph1_closed = False
    for rc in (3, 2, 1, 0):
        nkb = 4 * rc + 4
        dn = 7 if rc == 3 else 4
        for p in range(2):
            # expP[kb][i] holds exp(scores) for head 2p+i, keys block kb
            expP = ph2w.tile([P, TB, 2, 512], BF16, name="expP", tag="expP",
                             bufs=2)
            for kb in range(nkb):
                ps = psS.tile([P, 1024], F32, name="ps_sc", tag="ps_sc",
                              bufs=2)
                nc.tensor.matmul(ps[:, 0:512],
                                 lhsT=kTp[p][0:DK, kb * P:(kb + 1) * P],
                                 rhs=qTp[p][0:DK, rc * 512:(rc + 1) * 512],
                                 start=True, stop=True,
                                 tile_position=(0, 0))
                nc.tensor.matmul(ps[:, 512:1024],
                                 lhsT=kTp[p][DK:P, kb * P:(kb + 1) * P],
                                 rhs=qTp[p][DK:P, rc * 512:(rc + 1) * 512],
                                 start=True, stop=True,
                                 tile_position=(64, 0))
                nc.scalar.activation(
                    out=expP[:, kb, :, :],
                    in_=ps[:].rearrange("p (h r) -> p h r", h=2),
                    func=AF.Exp)
                if kb >= 4 * rc:       # diagonal block: apply causal mask
                    i = kb - 4 * rc
                    for hh in range(2):
                        nc.vector.tensor_mul(out=expP[:, kb, hh, :],
                                             in0=expP[:, kb, hh, :],
                                             in1=mask4[:, i, :])
                drain(dn)
            pending.extend(make_attn_work(rc, p, expP))
        pending.extend(make_w1_work(rc))
        pending.extend(make_ln1_work(rc))
        if not ph1_closed:
            # all q/v pending work must be emitted before ph1 frees
            ph1_closed = True
            drain(max(0, ph1_work_n - drained[0]))
            close_pool(ph1)
    drain(len(pending))
    close_pool(psV)
    close_pool(psS)
    close_pool(ph3)
    close_pool(cat)
    close_pool(ph2w)
    close_pool(psX)
    close_pool(attn)

    # ================= Phase 4: cross-attention =========================
    fw = open_pool(name="fw", bufs=1, side="left")        # Wf1, lives to FFN1
    wf1_all = fw.tile([P, KC, DF], BF16, name="wf1", tag="wf1")
    nc.gpsimd.dma_start(out=wf1_all[:],
                        in_=d["Wf1"].rearrange("p (c f) -> p c f", c=KC))
    at2p = open_pool(name="at2p", bufs=1, side="left")    # at2T, into ph5
    at2T = [at2p.tile([P, R], BF16, name=f"at2T{i}", tag=f"at2T{i}")
            for i in range(KC)]
    w2p = open_pool(name="w2p", bufs=1, side="left")      # W2sum, into ph5
    w2 = [w2p.tile([P, D], BF16, name=f"w2_{i}", tag=f"w2_{i}")
          for i in range(KC)]
    for kc in range(KC):
        nc.gpsimd.dma_start(out=w2[kc][:],
                            in_=d["W2sum"][kc * P:(kc + 1) * P, :])
    ph4 = open_pool(name="ph4", bufs=1, side="left")
    pp4 = open_pool(name="pp4", bufs=4, space="PSUM", side="left")
    pd4 = open_pool(name="pd4", bufs=1, space="PSUM", side="left")
    for kc in range(4, KC):
        xT[kc] = ph4.tile([P, T], BF16, name=f"xT{kc}", tag=f"xT{kc}")
        nc.sync.dma_start(out=xT[kc][:],
                          in_=d["xT"][kc * P:(kc + 1) * P, :])
    # x_tm: single strided DMA (one descriptor on the scalar queue)
    x_tm = ph4.tile([P, TB, D], BF16, name="xtm", tag="xtm")
    nc.scalar.dma_start(out=x_tm[:],
                        in_=d["x_tm"].rearrange("(t p) d -> p t d", p=P))

    p2T = [ph4.tile([P, R], BF16, name=f"p2T{i}", tag=f"p2T{i}")
           for i in range(TB)]
    for tb in range(TB):
        ps = pp4.tile([P, 512], F32, name="ps4", tag="ps4")
        for kc in range(KC):
            nc.tensor.matmul(ps[:], lhsT=xT[kc][:, tb * P:(tb + 1) * P],
                             rhs=a1T[kc][:, :],
                             start=(kc == 0), stop=(kc == KC - 1))
        nc.scalar.activation(out=p2T[tb][:], in_=ps[:], func=AF.Exp)
    # denominator: 4 col-tiled ones-matmul accumulators run concurrently
    pd = pd4.tile([P, R], F32, name="ps_d2", tag="ps_d2")
    for g in range(4):
        for u in range(4):
            tb = 4 * g + u
            nc.tensor.matmul(pd[32 * g:32 * g + 1, :], lhsT=ones_col[:],
                             rhs=p2T[tb][:], start=(u == 0), stop=(u == 3),
                             tile_position=(0, 32 * g))
    den4 = ph4.tile([1, 4, R], F32, name="den4", tag="den4")
    for g in range(4):
        nc.vector.tensor_copy(out=den4[:, g, :], in_=pd[32 * g:32 * g + 1, :])
    den2a = ph4.tile([1, R], F32, name="den2a", tag="den2a")
    den2b = ph4.tile([1, R], F32, name="den2b", tag="den2b")
    den2 = ph4.tile([1, R], F32, name="den2", tag="den2")
    nc.vector.tensor_add(out=den2a[:], in0=den4[:, 0, :], in1=den4[:, 1, :])
    nc.vector.tensor_add(out=den2b[:], in0=den4[:, 2, :], in1=den4[:, 3, :])
    nc.vector.tensor_add(out=den2[:], in0=den2a[:], in1=den2b[:])
    recip2 = ph4.tile([1, R], BF16, name="recip2", tag="recip2")
    with nc.allow_low_precision(reason="softmax denom bf16 ok"):
        nc.vector.reciprocal(out=recip2[:], in_=den2[:])
    psb2 = pd4.tile([P, R], F32, name="psb2", tag="psb2")
    nc.tensor.matmul(psb2[:], lhsT=ones_row[:], rhs=recip2[:],
                     start=True, stop=True)
    recip2b = ph4.tile([P, R], F32, name="recip2b", tag="recip2b")
    nc.vector.tensor_copy(out=recip2b[:], in_=psb2[:])
    for db in range(KC):
        ps = pp4.tile([P, 512], F32, name="ps4", tag="ps4")
        for tb in range(TB):
            nc.tensor.matmul(ps[:], lhsT=x_tm[:, tb, db * P:(db + 1) * P],
                             rhs=p2T[tb][:],
                             start=(tb == 0), stop=(tb == TB - 1))
        nc.vector.tensor_mul(out=at2T[db][:], in0=ps[:], in1=recip2b[:])
    close_pool(pd4)
    close_pool(pp4)
    close_pool(ph4)
    close_pool(a1pl)
    close_pool(xpre)

    # ========= Phase 5: W2sum + residual + LN2, produce a2T =============
    a2p = open_pool(name="a2p", bufs=1, side="right")     # a2T into ph6
    a2T = [a2p.tile([P, R], BF16, name=f"a2T{i}", tag=f"a2T{i}")
           for i in range(KC)]
    ph5 = open_pool(name="ph5", bufs=1, side="right")
    pp5 = open_pool(name="pp5", bufs=4, space="PSUM", side="left")
    pt5 = open_pool(name="pt5", bufs=2, space="PSUM", side="left")
    if not f["b2_zero"]:
        f["b2_b"] = bcast_row(ph5, "b2")
    if not f["ln2_unit_g"]:
        f["g2_b"] = bcast_row(ph5, "ln2_g")
    if not f["ln2_zero_b"]:
        f["be2_b"] = bcast_row(ph5, "ln2_b")
    for rb in range(RB):
        y5 = ph5.tile([P, D], F32, name="y5", tag="y5", bufs=2)
        nc.sync.dma_start(out=y5[:], in_=d["y_rows"][rb * P:(rb + 1) * P, :])
        a2 = ph5.tile([P, D], F32, name="a2", tag="a2", bufs=2)
        for nt in range(2):
            ps = pp5.tile([P, 512], F32, name="ps_a2", tag="ps_a2")
            for kc in range(KC):
                nc.tensor.matmul(ps[:],
                                 lhsT=at2T[kc][:, rb * P:(rb + 1) * P],
                                 rhs=w2[kc][:, nt * 512:(nt + 1) * 512],
                                 start=(kc == 0), stop=(kc == KC - 1))
            sl = slice(nt * 512, (nt + 1) * 512)
            nc.vector.tensor_add(out=a2[:, sl], in0=ps[:], in1=y5[:, sl])
            if not f["b2_zero"]:
                nc.vector.tensor_add(out=a2[:, sl], in0=a2[:, sl],
                                     in1=f["b2_b"][:, sl])
        a2b = ph5.tile([P, D], BF16, name="a2b", tag="a2b", bufs=2)
        ln_slim(ph5, a2[:], a2b[:], eps_sb,
                g_b=None if f["ln2_unit_g"] else f["g2_b"][:],
                be_b=None if f["ln2_zero_b"] else f["be2_b"][:])
        for kc in range(KC):
            pt = pt5.tile([P, P], BF16, name="pt_a2", tag="pt_a2")
            nc.tensor.transpose(pt[:], a2b[:, kc * P:(kc + 1) * P], ident[:])
            nc.vector.tensor_copy(out=a2T[kc][:, rb * P:(rb + 1) * P],
                                  in_=pt[:])
    close_pool(pt5)
    close_pool(pp5)
    close_pool(ph5)
    close_pool(w2p)
    close_pool(at2p)

    # ========== Phase 6: FFN + residual + LN3 ===========================
    fA = open_pool(name="fA", bufs=1, side="right")
    f1T = [fA.tile([P, R], BF16, name=f"f1T{i}", tag=f"f1T{i}")
           for i in range(FB)]
    bf1_sb = bias_chunks(fA, "bf1", FB)
    pfA = open_pool(name="pfA", bufs=3, space="PSUM", side="left")
    for fb in range(FB):
        ps = pfA.tile([P, 512], F32, name="ps_f1", tag="ps_f1")
        for kc in range(KC):
            nc.tensor.matmul(ps[:], lhsT=wf1_all[:, kc, fb * P:(fb + 1) * P],
                             rhs=a2T[kc][:, :],
                             start=(kc == 0), stop=(kc == KC - 1))
        # relu + bias on ACT (idle during the FFN)
        nc.scalar.activation(out=f1T[fb][:], in_=ps[:], func=AF.Relu,
                             bias=bf1_sb[:, fb:fb + 1], scale=1.0)
    close_pool(pfA)
    close_pool(fw)

    pfB = open_pool(name="pfB", bufs=1, space="PSUM", side="left")
    fB = open_pool(name="fB", bufs=1, side="right")
    ps_rb = [pfB.tile([P, D], F32, name=f"ps_rb{i}", tag=f"ps_rb{i}")
             for i in range(RB)]
    y6 = [fB.tile([P, D], F32, name=f"y6{i}", tag=f"y6{i}")
          for i in range(RB)]
    for rb in range(RB):
        nc.sync.dma_start(out=y6[rb][:],
                          in_=d["y_rows"][rb * P:(rb + 1) * P, :])
    if not f["bf2_zero"]:
        f["bf2_b"] = bcast_row(fB, "bf2")
    if not f["ln3_unit_g"]:
        f["g3_b"] = bcast_row(fB, "ln3_g")
    if not f["ln3_zero_b"]:
        f["be3_b"] = bcast_row(fB, "ln3_b")
    wf2_t = {}
    for fb in range(FB):
        wf2_fb = fB.tile([P, D], BF16, name="wf2s", tag="wf2s", bufs=6)
        nc.sync.dma_start(out=wf2_fb[:], in_=d["Wf2"][fb * P:(fb + 1) * P, :])
        wf2_t[fb] = wf2_fb
        if fb < FB - 4:
            for rb in range(RB):
                for nt in range(2):
                    nc.tensor.matmul(
                        ps_rb[rb][:, nt * 512:(nt + 1) * 512],
                        lhsT=f1T[fb][:, rb * P:(rb + 1) * P],
                        rhs=wf2_fb[:, nt * 512:(nt + 1) * 512],
                        start=(fb == 0), stop=False)

    def tail(rb):
        ff = fB.tile([P, D], F32, name="ff", tag="ff", bufs=2)
        nc.vector.tensor_add(out=ff[:], in0=ps_rb[rb][:], in1=y6[rb][:])
        if not f["bf2_zero"]:
            nc.vector.tensor_add(out=ff[:], in0=ff[:], in1=f["bf2_b"][:])
        o = fB.tile([P, D], F32, name="o", tag="o", bufs=2)
        ln_slim(fB, ff[:], o[:], eps_sb,
                g_b=None if f["ln3_unit_g"] else f["g3_b"][:],
                be_b=None if f["ln3_zero_b"] else f["be3_b"][:])
        nc.sync.dma_start(out=out_d[rb * P:(rb + 1) * P, :], in_=o[:])

    # last 4 fb row-major: each row block finishes early and its LN3+store
    # overlaps the remaining matmuls
    for rb in range(RB):
        for fb in range(FB - 4, FB):
            for nt in range(2):
                nc.tensor.matmul(ps_rb[rb][:, nt * 512:(nt + 1) * 512],
                                 lhsT=f1T[fb][:, rb * P:(rb + 1) * P],
                                 rhs=wf2_t[fb][:, nt * 512:(nt + 1) * 512],
                                 start=False, stop=(fb == FB - 1))
        tail(rb)
    close_pool(fB)
    close_pool(pfB)
    close_pool(fA)
    close_pool(a2p)
    close_pool(dramp)
    close_pool(const)


def _row_idx(j):
    return np.concatenate(
        [np.arange(512 * rc + 128 * j, 512 * rc + 128 * j + 128)
         for rc in range(4)])


def _flags(inputs):
    z = lambda a: bool(np.all(np.asarray(a) == 0.0))
    u = lambda a: bool(np.all(np.asarray(a) == 1.0))
    return {
        "b1_zero": z(inputs["b1"]), "b2_zero": z(inputs["b2"]),
        "bf2_zero": z(inputs["bf2"]),
        "ln1_unit_g": u(inputs["ln1_g"]), "ln1_zero_b": z(inputs["ln1_b"]),
        "ln2_unit_g": u(inputs["ln2_g"]), "ln2_zero_b": z(inputs["ln2_b"]),
        "ln3_unit_g": u(inputs["ln3_g"]), "ln3_zero_b": z(inputs["ln3_b"]),
    }


def _prep_host(inputs):
    f32 = lambda a: np.ascontiguousarray(np.asarray(a, np.float32))
    bf = lambda a: np.ascontiguousarray(
        np.asarray(a, np.float32).astype(ml_dtypes.bfloat16))
    x = f32(inputs["x"])
    y = f32(inputs["y"])
    mask = np.asarray(inputs["y_mask"]).astype(np.float32)
    # diagonal-block masks: mask4[ky, i, r] = mask[r, 128*i + ky]
    m4 = mask[0:512, 0:512].reshape(512, 4, 128).transpose(2, 1, 0)
    Wq = f32(inputs["Wq"])   # [H, D, DK]
    Wk = f32(inputs["Wk"])
    Wv = f32(inputs["Wv"])

    def chunkP(a):
        """[C*P, F] -> [P, C*F] so each partition's data is contiguous."""
        cp, fdim = a.shape
        return np.ascontiguousarray(
            a.reshape(cp // P, P, fdim).transpose(1, 0, 2).reshape(P, -1))

    shared = {
        "mask4": bf(m4),
        "b1": f32(inputs["b1"]),
        "ln1_g": f32(inputs["ln1_g"]), "ln1_b": f32(inputs["ln1_b"]),
        "W2sum": bf(f32(inputs["W2"]).reshape(H, D, D).sum(0)),
        "b2": f32(inputs["b2"]),
        "ln2_g": f32(inputs["ln2_g"]), "ln2_b": f32(inputs["ln2_b"]),
        "Wf1": chunkP(bf(inputs["Wf1"])),
        "bf1": f32(inputs["bf1"]),
        "Wf2": bf(inputs["Wf2"]),
        "bf2": f32(inputs["bf2"]),
        "ln3_g": f32(inputs["ln3_g"]), "ln3_b": f32(inputs["ln3_b"]),
    }
    in_maps = []
    for c in range(NCORES):
        b, j = c // 4, c % 4
        hh = slice(4 * j, 4 * j + 4)
        ridx = _row_idx(j)
        in_maps.append({
            "yT": bf(y[b].T),
            "wq": chunkP(bf(Wq[hh].transpose(1, 0, 2).reshape(D, 256) * SCALE)),
            "wk": chunkP(bf(Wk[hh].transpose(1, 0, 2).reshape(D, 256))),
            "wv": chunkP(bf(Wv[hh].transpose(1, 0, 2).reshape(D, 256))),
            "bq_s": f32(inputs["bq"])[hh].reshape(256) * np.float32(SCALE),
            "bk_f": f32(inputs["bk"])[hh].reshape(256),
            "bv_f": f32(inputs["bv"])[hh].reshape(256),
            "w1loc": chunkP(bf(f32(inputs["W1"])[256 * j:256 * (j + 1), :])),
            "y_rows": np.ascontiguousarray(y[b][ridx]),
            "xT": bf(x[b].T),
            "x_tm": bf(x[b]),
            **shared,
        })
    return in_maps


def kernel(**inputs):
    fl = _flags(inputs)
    key = tuple(sorted(fl.items()))
    if key not in _cached:
        _cached[key] = build_nc(dict(fl))
    nc = _cached[key]
    in_maps = _prep_host(inputs)
    res = run_bass_kernel_spmd(nc, in_maps, core_ids=list(range(NCORES)))
    out = np.zeros((B, S, D), np.float32)
    for c in range(NCORES):
        b, j = c // 4, c % 4
        out[b, _row_idx(j)] = res.results[c]["out"]
    return out


# revision 24
# speedup vs baseline: 1.0151x; 1.0151x over previous
"""Trainium2 Bass kernel for nn_DecoderLayer_33758442946809.

Sharding (8 cores = 2 batches x 4-core groups):
- Self-attention is HEAD-sharded: core (b, j) computes heads 4j..4j+3 for
  all T=2048 rows of batch b; causal skipping is SPMD-uniform (only
  lower-triangle key blocks are scored/exp'd).
- W1 is row-parallel over the head-sharded cat features; partials are
  summed with a chunked ReduceScatter (bf16) over each 4-core group.
  After the RS, core (b, j) owns the strided row set
  {512*rc + 128*j + i : rc<4, i<128}; cross-attention, W2 and the FFN
  are data-parallel over those rows.
- tile(attn2, H) @ W2 == attn2 @ sum_h W2[h] (host precomputes the sum).

v2 schedule (vs the phase-serial v1):
- Row chunks processed big-first [3,2,1,0] so the last ReduceScatter has
  the shortest dependency tail.
- Score PSUM is [128,1024] tiles (1 key block, both heads) with bufs=2,
  so ACT exp streams without stalling on PSUM WAR.
- Softmax normalization: denominator row is reciprocal'd at [1,2,512]
  (cheap) then partition-broadcast; the numerator is multiplied straight
  out of PSUM -- no [65,512] evictions, no 6.5us [64,1024] reciprocals.
- QKV projection matmuls (q tcols 2..0, all of v) drain as pending PE
  work under the first row-chunk's exp stream.
- RS outputs land via the sync queue (the gpsimd queue used to block
  ~90us on the RS-done semaphore).
- LN affine ops are skipped when gamma==1/beta==0 (host-checked program
  variant); the attention 1/sqrt(dk) scale is folded into LN1's rstd.
- Transposes run in bf16 (pre-cast) -- 1 PE cycle/row instead of 2.
- xT / x_tm / Wf1 / W2sum are prefetched a phase early.
- FFN1 relu+bias runs on ACT (idle there) instead of DVE.
- FFN2's last 4 weight blocks run row-major so row block 0 finishes
  ~6us early and the final LN3+store overlaps the remaining matmuls.
"""
import math
import sys

import numpy as np

sys.path.insert(0, "/opt/trn_rl_repo")

import ml_dtypes  # noqa: E402

import concourse.bass as bass  # noqa: E402
import concourse.tile as tile  # noqa: E402
from concourse import bacc, mybir  # noqa: E402
from concourse.bass_utils import run_bass_kernel_spmd  # noqa: E402
from concourse.masks import make_identity  # noqa: E402

B, S, D, H, DF = 2, 2048, 1024, 16, 4096
DK = D // H                      # 64
P = 128
T = S                            # rows/keys per batch
R = 512                          # own rows per core (after RS)
KC = D // P                      # 8 contraction chunks of D
TB = T // P                      # 16 key blocks
RB = R // P                      # 4 row blocks
FB = DF // P                     # 32 ffn blocks
NCORES = 8
HL = 4                           # local heads per core
SCALE = 1.0 / math.sqrt(DK)
RG = [[0, 1, 2, 3], [4, 5, 6, 7]]

F32 = mybir.dt.float32
BF16 = mybir.dt.bfloat16
AF = mybir.ActivationFunctionType
ALU = mybir.AluOpType

_cached = {}


def build_nc(f):
    nc = bacc.Bacc("TRN2", target_bir_lowering=False, debug=False,
                   num_devices=NCORES)

    dram = {}

    def din(name, shape, dt):
        dram[name] = nc.dram_tensor(name, shape, dt, kind="ExternalInput").ap()

    din("yT", [D, T], BF16)          # y[b].T
    din("wq", [P, KC * HL * DK], BF16)   # pre-chunked [p][kc][f] layout
    din("wk", [P, KC * HL * DK], BF16)
    din("wv", [P, KC * HL * DK], BF16)
    din("bq_s", [HL * DK], F32)      # bq * SCALE, local heads
    din("bk_f", [HL * DK], F32)
    din("bv_f", [HL * DK], F32)
    din("mask4", [P, 4, R], BF16)    # diagonal-block masks (key, i, row)
    din("w1loc", [P, 2 * D], BF16)   # W1 rows owned by this core, pre-chunked
    din("b1", [D], F32)
    din("ln1_g", [D], F32)
    din("ln1_b", [D], F32)
    din("y_rows", [R, D], F32)       # this core's (strided) y rows
    din("xT", [D, T], BF16)
    din("x_tm", [T, D], BF16)
    din("W2sum", [D, D], BF16)
    din("b2", [D], F32)
    din("ln2_g", [D], F32)
    din("ln2_b", [D], F32)
    din("Wf1", [P, KC * DF], BF16)   # pre-chunked [p][kc][f] layout
    din("bf1", [DF], F32)
    din("Wf2", [DF, D], BF16)
    din("bf2", [D], F32)
    din("ln3_g", [D], F32)
    din("ln3_b", [D], F32)
    out_d = nc.dram_tensor("out", [R, D], F32, kind="ExternalOutput").ap()

    with tile.TileContext(nc) as tc:
        _build(nc, tc, dram, out_d, f)
    nc.compile()
    return nc


def _build(nc, tc, d, out_d, f):
    pool_cms = {}

    def open_pool(*args, **kw):
        cm = tc.tile_pool(*args, **kw)
        p = cm.__enter__()
        pool_cms[id(p)] = cm
        return p

    def close_pool(p):
        pool_cms.pop(id(p)).__exit__(None, None, None)

    const = open_pool(name="const", bufs=1, side="left")
    ident = const.tile([P, P], BF16, name="ident", tag="ident")
    make_identity(nc, ident[:])
    ones_col = const.tile([P, 1], BF16, name="ones_col", tag="ones_col")
    nc.vector.memset(ones_col[:], 1.0)
    ones_row = const.tile([1, P], BF16, name="ones_row", tag="ones_row")
    nc.vector.memset(ones_row[:], 1.0)
    eps_sb = const.tile([P, 1], F32, name="eps", tag="eps")
    nc.vector.memset(eps_sb[:], 1e-5)
    # eps/SCALE^2: sqrt(var/SCALE^2 + eps/SCALE^2) = sqrt(var+eps)/SCALE,
    # so LN1's rstd comes out pre-multiplied by the attention scale.
    eps_s2 = const.tile([P, 1], F32, name="eps_s2", tag="eps_s2")
    nc.vector.memset(eps_s2[:], 1e-5 / (SCALE * SCALE))

    def bias_chunks(pool, name, n):
        t = pool.tile([P, n], F32, name=f"bc_{name}", tag=f"bc_{name}")
        nc.sync.dma_start(out=t[:], in_=d[name].rearrange("(n p) -> p n", p=P))
        return t

    def bcast_row(pool, name):
        src = d[name]
        t = pool.tile([P, D], F32, name=f"br_{name}", tag=f"br_{name}")
        bc = bass.AP(tensor=src.tensor, offset=src.offset,
                     ap=[[0, P]] + list(src.ap))
        nc.sync.dma_start(out=t[:], in_=bc)
        return t

    def ln_slim(pool, x_ap, out_ap, eps_ap, var_scale=1.0, g_b=None,
                be_b=None):
        """LayerNorm along the free axis (D) of a token-major [128, D]
        f32 tile into out_ap. var_scale folds a constant into rstd."""
        x3 = x_ap.rearrange("p (n f) -> p n f", f=512)
        stats = pool.tile([P, 2, 6], F32, name="ln_stats", tag="ln_stats",
                          bufs=4)
        for sg in range(2):
            nc.vector.bn_stats(out=stats[:, sg, :], in_=x3[:, sg, :])
        mv = pool.tile([P, 2], F32, name="ln_mv", tag="ln_mv", bufs=4)
        nc.vector.bn_aggr(out=mv[:], in_=stats[:])
        std = pool.tile([P, 1], F32, name="ln_std", tag="ln_std", bufs=4)
        nc.scalar.activation(out=std[:], in_=mv[:, 1:2], func=AF.Sqrt,
                             bias=eps_ap[:], scale=var_scale)
        rstd = pool.tile([P, 1], F32, name="ln_rstd", tag="ln_rstd", bufs=4)
        nc.vector.reciprocal(out=rstd[:], in_=std[:])
        nc.vector.tensor_scalar(out=out_ap, in0=x_ap, scalar1=mv[:, 0:1],
                                scalar2=rstd[:], op0=ALU.subtract,
                                op1=ALU.mult)
        if g_b is not None:
            nc.vector.tensor_mul(out=out_ap, in0=out_ap, in1=g_b)
        if be_b is not None:
            nc.vector.tensor_add(out=out_ap, in0=out_ap, in1=be_b)

    # ======== pools whose tiles live into ph4 (right-side bottom) =======
    xpre = open_pool(name="xpre", bufs=1, side="right")
    # only half of xT is prefetched (SBUF is tight during rc=3);
    # xT[4..7] load at ph4 open and are the last kcs of each score group
    xT = [xpre.tile([P, T], BF16, name=f"xT{i}", tag=f"xT{i}")
          if i < 4 else None for i in range(KC)]
    a1pl = open_pool(name="a1pl", bufs=1, side="right")
    a1T = [a1pl.tile([P, R], BF16, name=f"a1T{i}", tag=f"a1T{i}")
           for i in range(KC)]

    # ===================== input DMAs (spread across queues) ============
    attn = open_pool(name="attn", bufs=1, side="right")  # live through ph2
    qTp = [attn.tile([P, T], BF16, name=f"qTp{i}", tag=f"qTp{i}")
           for i in range(2)]
    kTp = [attn.tile([P, T], BF16, name=f"kTp{i}", tag=f"kTp{i}")
           for i in range(2)]
    v_sb = [attn.tile([P, HL, DK + 1], BF16, name=f"v{i}", tag=f"v{i}")
            for i in range(TB)]
    mask4 = attn.tile([P, 4, R], BF16, name="mask4", tag="mask4")

    ph2w = open_pool(name="ph2w", bufs=1, side="left")   # ph2 working set
    ph1 = open_pool(name="ph1", bufs=1, side="left")     # closed after rc=3
    # yT lands as 4 column slabs so the first k-projection group can
    # start after ~1MB instead of the full 4MB
    yT_all = ph1.tile([P, KC, T], BF16, name="yT", tag="yT")
    wq_sb = ph1.tile([P, KC, 2 * P], BF16, name="wq", tag="wq")
    wk_sb = ph1.tile([P, KC, 2 * P], BF16, name="wk", tag="wk")
    wv_sb = ph1.tile([P, KC, 2 * P], BF16, name="wv", tag="wv")
    yTsrc = d["yT"].rearrange("(c p) t -> p c t", p=P)
    for tcol in range(4):
        sl = slice(tcol * 512, (tcol + 1) * 512)
        nc.sync.dma_start(out=yT_all[:, :, sl], in_=yTsrc[:, :, sl])
    nc.scalar.dma_start(out=wk_sb[:],
                        in_=d["wk"].rearrange("p (c f) -> p c f", c=KC))
    nc.scalar.dma_start(out=wq_sb[:],
                        in_=d["wq"].rearrange("p (c f) -> p c f", c=KC))
    nc.scalar.dma_start(out=wv_sb[:],
                        in_=d["wv"].rearrange("p (c f) -> p c f", c=KC))
    nc.gpsimd.dma_start(out=mask4[:], in_=d["mask4"][:])
    bq_sb = bias_chunks(ph1, "bq_s", 2)
    bk_sb = bias_chunks(ph1, "bk_f", 2)
    bv_b = ph1.tile([P, 2 * P], F32, name="bv_b", tag="bv_b")
    bv_src = d["bv_f"]
    nc.sync.dma_start(out=bv_b[:], in_=bass.AP(
        tensor=bv_src.tensor, offset=bv_src.offset,
        ap=[[0, P]] + list(bv_src.ap)))
    # cross-attention keys prefetched behind yT on the sync queue
    for kc in range(4):
        nc.sync.dma_start(out=xT[kc][:], in_=d["xT"][kc * P:(kc + 1) * P, :])

    # ============ ph1 QKV emission (k full, q tcol 3; rest pending) =====
    # psX: shared 1-bank [P,512] ring for q/v projections, W1 partials and
    # LN1 transposes (keeps psS at bufs=2 within the 8-bank budget)
    psX = open_pool(name="psX", bufs=1, space="PSUM", side="left")
    pending = []
    drained = [0]

    def drain(k):
        for _ in range(min(k, len(pending))):
            pending.pop(0)()
            drained[0] += 1

    def px_tile():
        return psX.tile([P, 512], F32, name="px", tag="px", bufs=2)

    def qk_group(dst, w_sb, b_sb, p, tcol):
        ps = px_tile()
        for kc in range(KC):
            nc.tensor.matmul(ps[:], lhsT=w_sb[:, kc, p * P:(p + 1) * P],
                             rhs=yT_all[:, kc, tcol * 512:(tcol + 1) * 512],
                             start=(kc == 0), stop=(kc == KC - 1))
        nc.vector.tensor_scalar(out=dst[p][:, tcol * 512:(tcol + 1) * 512],
                                in0=ps[:], scalar1=b_sb[:, p:p + 1],
                                scalar2=None, op0=ALU.add)

    # k: all tcols (every score block needs all keys); q: tcol 3 first
    for p in range(2):
        for tcol in range(4):
            qk_group(kTp, wk_sb, bk_sb, p, tcol)
    for p in range(2):
        qk_group(qTp, wq_sb, bq_sb, p, 3)

    def v_work(tb):
        work = []
        box = [None]

        def v_start():
            nc.vector.memset(v_sb[tb][:, :, DK:DK + 1], 1.0)
            box[0] = px_tile()

        def v_mm(kc):
            nc.tensor.matmul(box[0][:, 0:2 * P],
                             lhsT=yT_all[:, kc, tb * P:(tb + 1) * P],
                             rhs=wv_sb[:, kc, :],
                             start=(kc == 0), stop=(kc == KC - 1))

        def v_evict():
            nc.vector.tensor_add(
                out=v_sb[tb][:, :, 0:DK],
                in0=box[0][:, 0:2 * P].rearrange("p (h k) -> p h k", h=HL),
                in1=bv_b[:].rearrange("p (h k) -> p h k", h=HL))

        work.append(v_start)
        work.extend(lambda kc=kc: v_mm(kc) for kc in range(KC))
        work.append(v_evict)
        return work

    def q_work(p, tcol):
        work = []
        box = [None]

        def q_start():
            box[0] = px_tile()

        def q_mm(kc):
            nc.tensor.matmul(box[0][:],
                             lhsT=wq_sb[:, kc, p * P:(p + 1) * P],
                             rhs=yT_all[:, kc, tcol * 512:(tcol + 1) * 512],
                             start=(kc == 0), stop=(kc == KC - 1))

        def q_evict():
            nc.vector.tensor_scalar(
                out=qTp[p][:, tcol * 512:(tcol + 1) * 512],
                in0=box[0][:], scalar1=bq_sb[:, p:p + 1],
                scalar2=None, op0=ALU.add)

        work.append(q_start)
        work.extend(lambda kc=kc: q_mm(kc) for kc in range(KC))
        work.append(q_evict)
        return work

    # v must be fully projected before the first attnV drains; emit v
    # first, then the remaining q columns.
    for tb in range(TB):
        pending.extend(v_work(tb))
    for tcol in (2, 1, 0):
        for p in range(2):
            pending.extend(q_work(p, tcol))
    ph1_work_n = len(pending)

    # ============ ph2: causal attention + W1 + ReduceScatter ============
    cat = open_pool(name="cat", bufs=1, side="right")     # catT, ph2-long
    catT = [cat.tile([P, T], BF16, name=f"catT{i}", tag=f"catT{i}")
            for i in range(2)]
    ph3 = open_pool(name="ph3", bufs=1, side="right")     # W1/LN1 working
    w1_sb = ph3.tile([P, 2, D], BF16, name="w1", tag="w1")
    nc.gpsimd.dma_start(out=w1_sb[:],
                        in_=d["w1loc"].rearrange("p (c n) -> p c n", c=2))
    a1pre_box = {}
    if not f["b1_zero"]:
        f["b1_b"] = bcast_row(ph3, "b1")
    if not f["ln1_unit_g"]:
        f["g1_b"] = bcast_row(ph3, "ln1_g")
    if not f["ln1_zero_b"]:
        f["be1_b"] = bcast_row(ph3, "ln1_b")

    dramp = open_pool(name="dramp", bufs=1, space="DRAM", side="left")
    rs_in = [dramp.tile([4 * P, D], BF16, name=f"rsi{i}", tag=f"rsi{i}")
             for i in range(RB)]
    rs_out = [dramp.tile([P, D], BF16, name=f"rso{i}", tag=f"rso{i}")
              for i in range(RB)]

    psS = open_pool(name="psS", bufs=1, space="PSUM", side="left")
    psV = open_pool(name="psV", bufs=1, space="PSUM", side="left")

    def make_attn_work(rc, p, expP):
        nkb = 4 * rc + 4
        work = []
        pa_t = [None, None]

        def start_head(hh):
            pa_t[hh] = psV.tile([DK + 1, 512], F32, name="pa", tag="pa",
                                bufs=2)

        def mm_head(hh, kb):
            hl = 2 * p + hh
            nc.tensor.matmul(pa_t[hh][:], lhsT=v_sb[kb][:, hl, :],
                             rhs=expP[:, kb, hh, :],
                             start=(kb == 0), stop=(kb == nkb - 1))

        den_row = [None]

        def evict_den(hh):
            if hh == 0:
                # flat [1, 1024]: 3D APs hit a ~40x slower RECIPROCAL path
                den_row[0] = ph2w.tile([1, 2 * 512], BF16, name="den_row",
                                       tag="den_row", bufs=2)
            nc.vector.tensor_copy(out=den_row[0][:, hh * 512:(hh + 1) * 512],
                                  in_=pa_t[hh][DK:DK + 1, :])

        recB = [None]

        def recip_bcast():
            rec_row = ph2w.tile([1, 2 * 512], BF16, name="rec_row",
                                tag="rec_row", bufs=2)
            with nc.allow_low_precision(reason="softmax denom bf16 ok"):
                nc.vector.reciprocal(out=rec_row[:], in_=den_row[0][:])
            recB[0] = ph2w.tile([DK, 2 * 512], BF16, name="recB", tag="recB",
                                bufs=2)
            nc.gpsimd.partition_broadcast(recB[0][:], rec_row[:])

        def mul_head(hh):
            nc.vector.tensor_mul(
                out=catT[p][hh * DK:(hh + 1) * DK,
                            rc * 512:(rc + 1) * 512],
                in0=pa_t[hh][0:DK, :],
                in1=recB[0][:, hh * 512:(hh + 1) * 512])

        for hh in range(2):
            work.append(lambda hh=hh: start_head(hh))
            for kb in range(nkb):
                work.append(lambda hh=hh, kb=kb: mm_head(hh, kb))
            work.append(lambda hh=hh: evict_den(hh))
        work.append(recip_bcast)
        work.append(lambda: mul_head(0))
        work.append(lambda: mul_head(1))
        return work

    y_box = {}

    def make_w1_work(rc):
        work = []

        def w1_block(rb, nt, box):
            c0 = rc * 512 + rb * P
            if nt == 0:
                box[0] = ph2w.tile([P, D], BF16, name="a1p", tag="a1p",
                                   bufs=2)
            psw = px_tile()
            for kc2 in range(2):
                nc.tensor.matmul(psw[:],
                                 lhsT=catT[kc2][:, c0:c0 + P],
                                 rhs=w1_sb[:, kc2, nt * 512:(nt + 1) * 512],
                                 start=(kc2 == 0), stop=(kc2 == 1))
            nc.vector.tensor_copy(out=box[0][:, nt * 512:(nt + 1) * 512],
                                  in_=psw[:])
            if nt == 1:
                nc.gpsimd.dma_start(out=rs_in[rc][rb * P:(rb + 1) * P, :],
                                    in_=box[0][:])

        for rb in range(4):
            box = [None]
            for nt in range(2):
                work.append(lambda rb=rb, nt=nt, box=box: w1_block(rb, nt, box))

        def do_rs():
            nc.gpsimd.collective_compute(
                "ReduceScatter", ALU.add, replica_groups=RG,
                ins=[rs_in[rc][:].opt()], outs=[rs_out[rc][:].opt()])
            # y residual rows for this chunk (sync queue, no waits)
            yb = ph3.tile([P, D], F32, name="y_sb", tag="y_sb", bufs=1)
            nc.sync.dma_start(out=yb[:],
                              in_=d["y_rows"][rc * P:(rc + 1) * P, :])
            y_box[rc] = yb
            # RS result lands via sync queue (gpsimd must stay unblocked)
            ap = ph3.tile([P, D], BF16, name="a1pre", tag="a1pre", bufs=2)
            nc.sync.dma_start(out=ap[:], in_=rs_out[rc][:])
            a1pre_box[rc] = ap
        work.append(do_rs)
        return work

    def make_ln1_work(rc):
        work = []
        a1 = [None]
        fold = f["ln1_unit_g"] and f["ln1_zero_b"]

        def residual():
            a1[0] = ph3.tile([P, D], F32, name="a1", tag="a1", bufs=1)
            nc.vector.tensor_add(out=a1[0][:], in0=a1pre_box[rc][:],
                                 in1=y_box[rc][:])
            if not f["b1_zero"]:
                nc.vector.tensor_add(out=a1[0][:], in0=a1[0][:],
                                     in1=f["b1_b"][:])

        a1b = [None]

        def ln():
            a1b[0] = ph3.tile([P, D], BF16, name="a1b", tag="a1b", bufs=1)
            if fold:
                # rstd folds SCALE -> a1T comes out pre-scaled
                ln_slim(ph3, a1[0][:], a1b[0][:], eps_s2,
                        var_scale=1.0 / (SCALE * SCALE))
            else:
                ln_slim(ph3, a1[0][:], a1b[0][:], eps_sb,
                        g_b=None if f["ln1_unit_g"] else f["g1_b"][:],
                        be_b=None if f["ln1_zero_b"] else f["be1_b"][:])

        def tr(kc):
            pt = px_tile()[:].bitcast(BF16)[:, 0:P]   # bf16 view of psum
            nc.tensor.transpose(pt, a1b[0][:, kc * P:(kc + 1) * P],
                                ident[:])
            if fold:
                nc.vector.tensor_copy(out=a1T[kc][:, rc * P:(rc + 1) * P],
                                      in_=pt)
            else:
                nc.vector.tensor_scalar(
                    out=a1T[kc][:, rc * P:(rc + 1) * P], in0=pt,
                    scalar1=float(SCALE), scalar2=None, op0=ALU.mult)

        work.append(residual)
        work.append(ln)
        work.extend(lambda kc=kc: tr(kc) for kc in range(KC))
        return work

    ph1_closed = False
    prev_ln1 = None
    for rc in (3, 2, 1, 0):
        nkb = 4 * rc + 4
        dn = 7 if rc == 3 else 4
        for p in range(2):
            # expP[kb][i] holds exp(scores) for head 2p+i, keys block kb
            expP = ph2w.tile([P, TB, 2, 512], BF16, name="expP", tag="expP",
                             bufs=2)
            for kb in range(nkb):
                ps = psS.tile([P, 1024], F32, name="ps_sc", tag="ps_sc",
                              bufs=2)
                nc.tensor.matmul(ps[:, 0:512],
                                 lhsT=kTp[p][0:DK, kb * P:(kb + 1) * P],
                                 rhs=qTp[p][0:DK, rc * 512:(rc + 1) * 512],
                                 start=True, stop=True,
                                 tile_position=(0, 0))
                nc.tensor.matmul(ps[:, 512:1024],
                                 lhsT=kTp[p][DK:P, kb * P:(kb + 1) * P],
                                 rhs=qTp[p][DK:P, rc * 512:(rc + 1) * 512],
                                 start=True, stop=True,
                                 tile_position=(64, 0))
                nc.scalar.activation(
                    out=expP[:, kb, :, :],
                    in_=ps[:].rearrange("p (h r) -> p h r", h=2),
                    func=AF.Exp)
                if kb >= 4 * rc:       # diagonal block: apply causal mask
                    i = kb - 4 * rc
                    for hh in range(2):
                        nc.vector.tensor_mul(out=expP[:, kb, hh, :],
                                             in0=expP[:, kb, hh, :],
                                             in1=mask4[:, i, :])
                drain(dn)
            pending.extend(make_attn_work(rc, p, expP))
        # previous chunk's LN1 is emitted only now: its first DVE op waits
        # on that chunk's RS, and emitting it too early stalls the whole
        # in-order DVE queue behind the collective
        if prev_ln1 is not None:
            pending.extend(prev_ln1)
        pending.extend(make_w1_work(rc))
        prev_ln1 = make_ln1_work(rc)
        if not ph1_closed:
            # all q/v pending work must be emitted before ph1 frees
            ph1_closed = True
            drain(max(0, ph1_work_n - drained[0]))
            close_pool(ph1)
    pending.extend(prev_ln1)     # rc=0's LN1
    drain(len(pending))
    close_pool(psV)
    close_pool(psS)
    close_pool(ph3)
    close_pool(cat)
    close_pool(ph2w)
    close_pool(psX)
    close_pool(attn)

    # ================= Phase 4: cross-attention =========================
    fw = open_pool(name="fw", bufs=1, side="left")        # Wf1, lives to FFN1
    wf1_all = fw.tile([P, KC, DF], BF16, name="wf1", tag="wf1")
    nc.gpsimd.dma_start(out=wf1_all[:],
                        in_=d["Wf1"].rearrange("p (c f) -> p c f", c=KC))
    at2p = open_pool(name="at2p", bufs=1, side="left")    # at2T, into ph5
    at2T = [at2p.tile([P, R], BF16, name=f"at2T{i}", tag=f"at2T{i}")
            for i in range(KC)]
    w2p = open_pool(name="w2p", bufs=1, side="left")      # W2sum, into ph5
    w2 = [w2p.tile([P, D], BF16, name=f"w2_{i}", tag=f"w2_{i}")
          for i in range(KC)]
    for kc in range(KC):
        nc.gpsimd.dma_start(out=w2[kc][:],
                            in_=d["W2sum"][kc * P:(kc + 1) * P, :])
    ph4 = open_pool(name="ph4", bufs=1, side="left")
    pp4 = open_pool(name="pp4", bufs=4, space="PSUM", side="left")
    pd4 = open_pool(name="pd4", bufs=1, space="PSUM", side="left")
    for kc in range(4, KC):
        xT[kc] = ph4.tile([P, T], BF16, name=f"xT{kc}", tag=f"xT{kc}")
        nc.sync.dma_start(out=xT[kc][:],
                          in_=d["xT"][kc * P:(kc + 1) * P, :])
    # x_tm: single strided DMA (one descriptor on the scalar queue)
    x_tm = ph4.tile([P, TB, D], BF16, name="xtm", tag="xtm")
    nc.scalar.dma_start(out=x_tm[:],
                        in_=d["x_tm"].rearrange("(t p) d -> p t d", p=P))

    p2T = [ph4.tile([P, R], BF16, name=f"p2T{i}", tag=f"p2T{i}")
           for i in range(TB)]
    for tb in range(TB):
        ps = pp4.tile([P, 512], F32, name="ps4", tag="ps4")
        for kc in range(KC):
            nc.tensor.matmul(ps[:], lhsT=xT[kc][:, tb * P:(tb + 1) * P],
                             rhs=a1T[kc][:, :],
                             start=(kc == 0), stop=(kc == KC - 1))
        nc.scalar.activation(out=p2T[tb][:], in_=ps[:], func=AF.Exp)
    # denominator: 4 col-tiled ones-matmul accumulators run concurrently
    pd = pd4.tile([P, R], F32, name="ps_d2", tag="ps_d2")
    for g in range(4):
        for u in range(4):
            tb = 4 * g + u
            nc.tensor.matmul(pd[32 * g:32 * g + 1, :], lhsT=ones_col[:],
                             rhs=p2T[tb][:], start=(u == 0), stop=(u == 3),
                             tile_position=(0, 32 * g))
    den4 = ph4.tile([1, 4, R], F32, name="den4", tag="den4")
    for g in range(4):
        nc.vector.tensor_copy(out=den4[:, g, :], in_=pd[32 * g:32 * g + 1, :])
    den2a = ph4.tile([1, R], F32, name="den2a", tag="den2a")
    den2b = ph4.tile([1, R], F32, name="den2b", tag="den2b")
    den2 = ph4.tile([1, R], F32, name="den2", tag="den2")
    nc.vector.tensor_add(out=den2a[:], in0=den4[:, 0, :], in1=den4[:, 1, :])
    nc.vector.tensor_add(out=den2b[:], in0=den4[:, 2, :], in1=den4[:, 3, :])
    nc.vector.tensor_add(out=den2[:], in0=den2a[:], in1=den2b[:])
    recip2 = ph4.tile([1, R], BF16, name="recip2", tag="recip2")
    with nc.allow_low_precision(reason="softmax denom bf16 ok"):
        nc.vector.reciprocal(out=recip2[:], in_=den2[:])
    psb2 = pd4.tile([P, R], F32, name="psb2", tag="psb2")
    nc.tensor.matmul(psb2[:], lhsT=ones_row[:], rhs=recip2[:],
                     start=True, stop=True)
    recip2b = ph4.tile([P, R], F32, name="recip2b", tag="recip2b")
    nc.vector.tensor_copy(out=recip2b[:], in_=psb2[:])
    for db in range(KC):
        ps = pp4.tile([P, 512], F32, name="ps4", tag="ps4")
        for tb in range(TB):
            nc.tensor.matmul(ps[:], lhsT=x_tm[:, tb, db * P:(db + 1) * P],
                             rhs=p2T[tb][:],
                             start=(tb == 0), stop=(tb == TB - 1))
        nc.vector.tensor_mul(out=at2T[db][:], in0=ps[:], in1=recip2b[:])
    close_pool(pd4)
    close_pool(pp4)
    close_pool(ph4)
    close_pool(a1pl)
    close_pool(xpre)

    # ========= Phase 5: W2sum + residual + LN2, produce a2T =============
    a2p = open_pool(name="a2p", bufs=1, side="right")     # a2T into ph6
    a2T = [a2p.tile([P, R], BF16, name=f"a2T{i}", tag=f"a2T{i}")
           for i in range(KC)]
    ph5 = open_pool(name="ph5", bufs=1, side="right")
    pp5 = open_pool(name="pp5", bufs=4, space="PSUM", side="left")
    pt5 = open_pool(name="pt5", bufs=2, space="PSUM", side="left")
    if not f["b2_zero"]:
        f["b2_b"] = bcast_row(ph5, "b2")
    if not f["ln2_unit_g"]:
        f["g2_b"] = bcast_row(ph5, "ln2_g")
    if not f["ln2_zero_b"]:
        f["be2_b"] = bcast_row(ph5, "ln2_b")
    for rb in range(RB):
        y5 = ph5.tile([P, D], F32, name="y5", tag="y5", bufs=2)
        nc.sync.dma_start(out=y5[:], in_=d["y_rows"][rb * P:(rb + 1) * P, :])
        a2 = ph5.tile([P, D], F32, name="a2", tag="a2", bufs=2)
        for nt in range(2):
            ps = pp5.tile([P, 512], F32, name="ps_a2", tag="ps_a2")
            for kc in range(KC):
                nc.tensor.matmul(ps[:],
                                 lhsT=at2T[kc][:, rb * P:(rb + 1) * P],
                                 rhs=w2[kc][:, nt * 512:(nt + 1) * 512],
                                 start=(kc == 0), stop=(kc == KC - 1))
            sl = slice(nt * 512, (nt + 1) * 512)
            nc.vector.tensor_add(out=a2[:, sl], in0=ps[:], in1=y5[:, sl])
            if not f["b2_zero"]:
                nc.vector.tensor_add(out=a2[:, sl], in0=a2[:, sl],
                                     in1=f["b2_b"][:, sl])
        a2b = ph5.tile([P, D], BF16, name="a2b", tag="a2b", bufs=2)
        ln_slim(ph5, a2[:], a2b[:], eps_sb,
                g_b=None if f["ln2_unit_g"] else f["g2_b"][:],
                be_b=None if f["ln2_zero_b"] else f["be2_b"][:])
        for kc in range(KC):
            pt = pt5.tile([P, P], BF16, name="pt_a2", tag="pt_a2")
            nc.tensor.transpose(pt[:], a2b[:, kc * P:(kc + 1) * P], ident[:])
            nc.vector.tensor_copy(out=a2T[kc][:, rb * P:(rb + 1) * P],
                                  in_=pt[:])
    close_pool(pt5)
    close_pool(pp5)
    close_pool(ph5)
    close_pool(w2p)
    close_pool(at2p)

    # ========== Phase 6: FFN + residual + LN3 ===========================
    fA = open_pool(name="fA", bufs=1, side="right")
    f1T = [fA.tile([P, R], BF16, name=f"f1T{i}", tag=f"f1T{i}")
           for i in range(FB)]
    bf1_sb = bias_chunks(fA, "bf1", FB)
    pfA = open_pool(name="pfA", bufs=3, space="PSUM", side="left")
    for fb in range(FB):
        ps = pfA.tile([P, 512], F32, name="ps_f1", tag="ps_f1")
        for kc in range(KC):
            nc.tensor.matmul(ps[:], lhsT=wf1_all[:, kc, fb * P:(fb + 1) * P],
                             rhs=a2T[kc][:, :],
                             start=(kc == 0), stop=(kc == KC - 1))
        # relu + bias on ACT (idle during the FFN)
        nc.scalar.activation(out=f1T[fb][:], in_=ps[:], func=AF.Relu,
                             bias=bf1_sb[:, fb:fb + 1], scale=1.0)
    close_pool(pfA)
    close_pool(fw)

    pfB = open_pool(name="pfB", bufs=1, space="PSUM", side="left")
    fB = open_pool(name="fB", bufs=1, side="right")
    ps_rb = [pfB.tile([P, D], F32, name=f"ps_rb{i}", tag=f"ps_rb{i}")
             for i in range(RB)]
    y6 = [fB.tile([P, D], F32, name=f"y6{i}", tag=f"y6{i}")
          for i in range(RB)]
    for rb in range(RB):
        nc.sync.dma_start(out=y6[rb][:],
                          in_=d["y_rows"][rb * P:(rb + 1) * P, :])
    if not f["bf2_zero"]:
        f["bf2_b"] = bcast_row(fB, "bf2")
    if not f["ln3_unit_g"]:
        f["g3_b"] = bcast_row(fB, "ln3_g")
    if not f["ln3_zero_b"]:
        f["be3_b"] = bcast_row(fB, "ln3_b")
    wf2_t = {}
    for fb in range(FB):
        wf2_fb = fB.tile([P, D], BF16, name="wf2s", tag="wf2s", bufs=6)
        nc.sync.dma_start(out=wf2_fb[:], in_=d["Wf2"][fb * P:(fb + 1) * P, :])
        wf2_t[fb] = wf2_fb
        if fb < FB - 4:
            for rb in range(RB):
                for nt in range(2):
                    nc.tensor.matmul(
                        ps_rb[rb][:, nt * 512:(nt + 1) * 512],
                        lhsT=f1T[fb][:, rb * P:(rb + 1) * P],
                        rhs=wf2_fb[:, nt * 512:(nt + 1) * 512],
                        start=(fb == 0), stop=False)

    def tail(rb):
        ff = fB.tile([P, D], F32, name="ff", tag="ff", bufs=2)
        nc.vector.tensor_add(out=ff[:], in0=ps_rb[rb][:], in1=y6[rb][:])
        if not f["bf2_zero"]:
            nc.vector.tensor_add(out=ff[:], in0=ff[:], in1=f["bf2_b"][:])
        o = fB.tile([P, D], F32, name="o", tag="o", bufs=2)
        ln_slim(fB, ff[:], o[:], eps_sb,
                g_b=None if f["ln3_unit_g"] else f["g3_b"][:],
                be_b=None if f["ln3_zero_b"] else f["be3_b"][:])
        nc.sync.dma_start(out=out_d[rb * P:(rb + 1) * P, :], in_=o[:])

    # last 4 fb row-major: each row block finishes early and its LN3+store
    # overlaps the remaining matmuls
    for rb in range(RB):
        for fb in range(FB - 4, FB):
            for nt in range(2):
                nc.tensor.matmul(ps_rb[rb][:, nt * 512:(nt + 1) * 512],
                                 lhsT=f1T[fb][:, rb * P:(rb + 1) * P],
                                 rhs=wf2_t[fb][:, nt * 512:(nt + 1) * 512],
                                 start=False, stop=(fb == FB - 1))
        tail(rb)
    close_pool(fB)
    close_pool(pfB)
    close_pool(fA)
    close_pool(a2p)
    close_pool(dramp)
    close_pool(const)


def _row_idx(j):
    return np.concatenate(
        [np.arange(512 * rc + 128 * j, 512 * rc + 128 * j + 128)
         for rc in range(4)])


def _flags(inputs):
    z = lambda a: bool(np.all(np.asarray(a) == 0.0))
    u = lambda a: bool(np.all(np.asarray(a) == 1.0))
    return {
        "b1_zero": z(inputs["b1"]), "b2_zero": z(inputs["b2"]),
        "bf2_zero": z(inputs["bf2"]),
        "ln1_unit_g": u(inputs["ln1_g"]), "ln1_zero_b": z(inputs["ln1_b"]),
        "ln2_unit_g": u(inputs["ln2_g"]), "ln2_zero_b": z(inputs["ln2_b"]),
        "ln3_unit_g": u(inputs["ln3_g"]), "ln3_zero_b": z(inputs["ln3_b"]),
    }


def _prep_host(inputs):
    f32 = lambda a: np.ascontiguousarray(np.asarray(a, np.float32))
    bf = lambda a: np.ascontiguousarray(
        np.asarray(a, np.float32).astype(ml_dtypes.bfloat16))
    x = f32(inputs["x"])
    y = f32(inputs["y"])
    mask = np.asarray(inputs["y_mask"]).astype(np.float32)
    # diagonal-block masks: mask4[ky, i, r] = mask[r, 128*i + ky]
    m4 = mask[0:512, 0:512].reshape(512, 4, 128).transpose(2, 1, 0)
    Wq = f32(inputs["Wq"])   # [H, D, DK]
    Wk = f32(inputs["Wk"])
    Wv = f32(inputs["Wv"])

    def chunkP(a):
        """[C*P, F] -> [P, C*F] so each partition's data is contiguous."""
        cp, fdim = a.shape
        return np.ascontiguousarray(
            a.reshape(cp // P, P, fdim).transpose(1, 0, 2).reshape(P, -1))

    shared = {
        "mask4": bf(m4),
        "b1": f32(inputs["b1"]),
        "ln1_g": f32(inputs["ln1_g"]), "ln1_b": f32(inputs["ln1_b"]),
        "W2sum": bf(f32(inputs["W2"]).reshape(H, D, D).sum(0)),
        "b2": f32(inputs["b2"]),
        "ln2_g": f32(inputs["ln2_g"]), "ln2_b": f32(inputs["ln2_b"]),
        "Wf1": chunkP(bf(inputs["Wf1"])),
        "bf1": f32(inputs["bf1"]),
        "Wf2": bf(inputs["Wf2"]),
        "bf2": f32(inputs["bf2"]),
        "ln3_g": f32(inputs["ln3_g"]), "ln3_b": f32(inputs["ln3_b"]),
    }
    in_maps = []
    for c in range(NCORES):
        b, j = c // 4, c % 4
        hh = slice(4 * j, 4 * j + 4)
        ridx = _row_idx(j)
        in_maps.append({
            "yT": bf(y[b].T),
            "wq": chunkP(bf(Wq[hh].transpose(1, 0, 2).reshape(D, 256) * SCALE)),
            "wk": chunkP(bf(Wk[hh].transpose(1, 0, 2).reshape(D, 256))),
            "wv": chunkP(bf(Wv[hh].transpose(1, 0, 2).reshape(D, 256))),
            "bq_s": f32(inputs["bq"])[hh].reshape(256) * np.float32(SCALE),
            "bk_f": f32(inputs["bk"])[hh].reshape(256),
            "bv_f": f32(inputs["bv"])[hh].reshape(256),
            "w1loc": chunkP(bf(f32(inputs["W1"])[256 * j:256 * (j + 1), :])),
            "y_rows": np.ascontiguousarray(y[b][ridx]),
            "xT": bf(x[b].T),
            "x_tm": bf(x[b]),
            **shared,
        })
    return in_maps


def kernel(**inputs):
    fl = _flags(inputs)
    key = tuple(sorted(fl.items()))
    if key not in _cached:
        _cached[key] = build_nc(dict(fl))
    nc = _cached[key]
    in_maps = _prep_host(inputs)
    res = run_bass_kernel_spmd(nc, in_maps, core_ids=list(range(NCORES)))
    out = np.zeros((B, S, D), np.float32)
    for c in range(NCORES):
        b, j = c // 4, c % 4
        out[b, _row_idx(j)] = res.results[c]["out"]
    return out


# revision 28
# speedup vs baseline: 1.0269x; 1.0116x over previous
"""Trainium2 Bass kernel for nn_DecoderLayer_33758442946809.

Sharding (8 cores = 2 batches x 4-core groups):
- Self-attention is HEAD-sharded: core (b, j) computes heads 4j..4j+3 for
  all T=2048 rows of batch b; causal skipping is SPMD-uniform (only
  lower-triangle key blocks are scored/exp'd).
- W1 is row-parallel over the head-sharded cat features; partials are
  summed with a chunked ReduceScatter (bf16) over each 4-core group.
  After the RS, core (b, j) owns the strided row set
  {512*rc + 128*j + i : rc<4, i<128}; cross-attention, W2 and the FFN
  are data-parallel over those rows.
- tile(attn2, H) @ W2 == attn2 @ sum_h W2[h] (host precomputes the sum).

v2 schedule (vs the phase-serial v1):
- Row chunks processed big-first [3,2,1,0] so the last ReduceScatter has
  the shortest dependency tail.
- Score PSUM is [128,1024] tiles (1 key block, both heads) with bufs=2,
  so ACT exp streams without stalling on PSUM WAR.
- Softmax normalization: denominator row is reciprocal'd at [1,2,512]
  (cheap) then partition-broadcast; the numerator is multiplied straight
  out of PSUM -- no [65,512] evictions, no 6.5us [64,1024] reciprocals.
- QKV projection matmuls (q tcols 2..0, all of v) drain as pending PE
  work under the first row-chunk's exp stream.
- RS outputs land via the sync queue (the gpsimd queue used to block
  ~90us on the RS-done semaphore).
- LN affine ops are skipped when gamma==1/beta==0 (host-checked program
  variant); the attention 1/sqrt(dk) scale is folded into LN1's rstd.
- Transposes run in bf16 (pre-cast) -- 1 PE cycle/row instead of 2.
- xT / x_tm / Wf1 / W2sum are prefetched a phase early.
- FFN1 relu+bias runs on ACT (idle there) instead of DVE.
- FFN2's last 4 weight blocks run row-major so row block 0 finishes
  ~6us early and the final LN3+store overlaps the remaining matmuls.
"""
import math
import sys

import numpy as np

sys.path.insert(0, "/opt/trn_rl_repo")

import ml_dtypes  # noqa: E402

import concourse.bass as bass  # noqa: E402
import concourse.tile as tile  # noqa: E402
from concourse import bacc, mybir  # noqa: E402
from concourse.bass_utils import run_bass_kernel_spmd  # noqa: E402
from concourse.masks import make_identity  # noqa: E402

B, S, D, H, DF = 2, 2048, 1024, 16, 4096
DK = D // H                      # 64
P = 128
T = S                            # rows/keys per batch
R = 512                          # own rows per core (after RS)
KC = D // P                      # 8 contraction chunks of D
TB = T // P                      # 16 key blocks
RB = R // P                      # 4 row blocks
FB = DF // P                     # 32 ffn blocks
NCORES = 8
HL = 4                           # local heads per core
SCALE = 1.0 / math.sqrt(DK)
RG = [[0, 1, 2, 3], [4, 5, 6, 7]]

F32 = mybir.dt.float32
BF16 = mybir.dt.bfloat16
AF = mybir.ActivationFunctionType
ALU = mybir.AluOpType

_cached = {}


def build_nc(f):
    nc = bacc.Bacc("TRN2", target_bir_lowering=False, debug=False,
                   num_devices=NCORES)

    dram = {}

    def din(name, shape, dt):
        dram[name] = nc.dram_tensor(name, shape, dt, kind="ExternalInput").ap()

    din("yT", [D, T], BF16)          # y[b].T
    din("wq", [P, KC * HL * DK], BF16)   # pre-chunked [p][kc][f] layout
    din("wk", [P, KC * HL * DK], BF16)
    din("wv", [P, KC * HL * DK], BF16)
    din("bq_s", [HL * DK], F32)      # bq * SCALE, local heads
    din("bk_f", [HL * DK], F32)
    din("bv_f", [HL * DK], F32)
    din("mask4", [P, 4, R], BF16)    # diagonal-block masks (key, i, row)
    din("w1loc", [P, 2 * D], BF16)   # W1 rows owned by this core, pre-chunked
    din("b1", [D], F32)
    din("ln1_g", [D], F32)
    din("ln1_b", [D], F32)
    din("y_rows", [R, D], F32)       # this core's (strided) y rows
    din("xT", [D, T], BF16)
    din("x_tm", [T, D], BF16)
    din("W2sum", [D, D], BF16)
    din("b2", [D], F32)
    din("ln2_g", [D], F32)
    din("ln2_b", [D], F32)
    din("Wf1", [P, KC * DF], BF16)   # pre-chunked [p][kc][f] layout
    din("bf1", [DF], F32)
    din("Wf2", [DF, D], BF16)
    din("bf2", [D], F32)
    din("ln3_g", [D], F32)
    din("ln3_b", [D], F32)
    out_d = nc.dram_tensor("out", [R, D], F32, kind="ExternalOutput").ap()

    with tile.TileContext(nc) as tc:
        _build(nc, tc, dram, out_d, f)
    nc.compile()
    return nc


def _build(nc, tc, d, out_d, f):
    pool_cms = {}

    def open_pool(*args, **kw):
        cm = tc.tile_pool(*args, **kw)
        p = cm.__enter__()
        pool_cms[id(p)] = cm
        return p

    def close_pool(p):
        pool_cms.pop(id(p)).__exit__(None, None, None)

    const = open_pool(name="const", bufs=1, side="left")
    ident = const.tile([P, P], BF16, name="ident", tag="ident")
    make_identity(nc, ident[:])
    ones_col = const.tile([P, 1], BF16, name="ones_col", tag="ones_col")
    nc.vector.memset(ones_col[:], 1.0)
    ones_row = const.tile([1, P], BF16, name="ones_row", tag="ones_row")
    nc.vector.memset(ones_row[:], 1.0)
    eps_sb = const.tile([P, 1], F32, name="eps", tag="eps")
    nc.vector.memset(eps_sb[:], 1e-5)
    # eps/SCALE^2: sqrt(var/SCALE^2 + eps/SCALE^2) = sqrt(var+eps)/SCALE,
    # so LN1's rstd comes out pre-multiplied by the attention scale.
    eps_s2 = const.tile([P, 1], F32, name="eps_s2", tag="eps_s2")
    nc.vector.memset(eps_s2[:], 1e-5 / (SCALE * SCALE))

    def bias_chunks(pool, name, n):
        t = pool.tile([P, n], F32, name=f"bc_{name}", tag=f"bc_{name}")
        nc.sync.dma_start(out=t[:], in_=d[name].rearrange("(n p) -> p n", p=P))
        return t

    def bcast_row(pool, name):
        src = d[name]
        t = pool.tile([P, D], F32, name=f"br_{name}", tag=f"br_{name}")
        bc = bass.AP(tensor=src.tensor, offset=src.offset,
                     ap=[[0, P]] + list(src.ap))
        nc.sync.dma_start(out=t[:], in_=bc)
        return t

    def ln_slim(pool, x_ap, out_ap, eps_ap, var_scale=1.0, g_b=None,
                be_b=None):
        """LayerNorm along the free axis (D) of a token-major [128, D]
        f32 tile into out_ap. var_scale folds a constant into rstd."""
        x3 = x_ap.rearrange("p (n f) -> p n f", f=512)
        stats = pool.tile([P, 2, 6], F32, name="ln_stats", tag="ln_stats",
                          bufs=4)
        for sg in range(2):
            nc.vector.bn_stats(out=stats[:, sg, :], in_=x3[:, sg, :])
        mv = pool.tile([P, 2], F32, name="ln_mv", tag="ln_mv", bufs=4)
        nc.vector.bn_aggr(out=mv[:], in_=stats[:])
        std = pool.tile([P, 1], F32, name="ln_std", tag="ln_std", bufs=4)
        nc.scalar.activation(out=std[:], in_=mv[:, 1:2], func=AF.Sqrt,
                             bias=eps_ap[:], scale=var_scale)
        rstd = pool.tile([P, 1], F32, name="ln_rstd", tag="ln_rstd", bufs=4)
        nc.vector.reciprocal(out=rstd[:], in_=std[:])
        nc.vector.tensor_scalar(out=out_ap, in0=x_ap, scalar1=mv[:, 0:1],
                                scalar2=rstd[:], op0=ALU.subtract,
                                op1=ALU.mult)
        if g_b is not None:
            nc.vector.tensor_mul(out=out_ap, in0=out_ap, in1=g_b)
        if be_b is not None:
            nc.vector.tensor_add(out=out_ap, in0=out_ap, in1=be_b)

    # ======== pools whose tiles live into ph4 (right-side bottom) =======
    xpre = open_pool(name="xpre", bufs=1, side="right")
    # only half of xT is prefetched (SBUF is tight during rc=3);
    # xT[4..7] load at ph4 open and are the last kcs of each score group
    xT = [xpre.tile([P, T], BF16, name=f"xT{i}", tag=f"xT{i}")
          if i < 4 else None for i in range(KC)]
    a1pl = open_pool(name="a1pl", bufs=1, side="right")
    a1T = [a1pl.tile([P, R], BF16, name=f"a1T{i}", tag=f"a1T{i}")
           for i in range(KC)]

    # ===================== input DMAs (spread across queues) ============
    attn = open_pool(name="attn", bufs=1, side="right")  # live through ph2
    qTp = [attn.tile([P, T], BF16, name=f"qTp{i}", tag=f"qTp{i}")
           for i in range(2)]
    kTp = [attn.tile([P, T], BF16, name=f"kTp{i}", tag=f"kTp{i}")
           for i in range(2)]
    v_sb = [attn.tile([P, HL, DK + 1], BF16, name=f"v{i}", tag=f"v{i}")
            for i in range(TB)]
    mask4 = attn.tile([P, 4, R], BF16, name="mask4", tag="mask4")

    ph2w = open_pool(name="ph2w", bufs=1, side="left")   # ph2 working set
    ph1 = open_pool(name="ph1", bufs=1, side="left")     # closed after rc=3
    # yT lands as 4 column slabs so the first k-projection group can
    # start after ~1MB instead of the full 4MB
    yT_all = ph1.tile([P, KC, T], BF16, name="yT", tag="yT")
    wq_sb = ph1.tile([P, KC, 2 * P], BF16, name="wq", tag="wq")
    wk_sb = ph1.tile([P, KC, 2 * P], BF16, name="wk", tag="wk")
    wv_sb = ph1.tile([P, KC, 2 * P], BF16, name="wv", tag="wv")
    yTsrc = d["yT"].rearrange("(c p) t -> p c t", p=P)
    for tcol in range(4):
        sl = slice(tcol * 512, (tcol + 1) * 512)
        nc.sync.dma_start(out=yT_all[:, :, sl], in_=yTsrc[:, :, sl])
    nc.scalar.dma_start(out=wk_sb[:],
                        in_=d["wk"].rearrange("p (c f) -> p c f", c=KC))
    nc.scalar.dma_start(out=wq_sb[:],
                        in_=d["wq"].rearrange("p (c f) -> p c f", c=KC))
    nc.scalar.dma_start(out=wv_sb[:],
                        in_=d["wv"].rearrange("p (c f) -> p c f", c=KC))
    nc.gpsimd.dma_start(out=mask4[:], in_=d["mask4"][:])
    bq_sb = bias_chunks(ph1, "bq_s", 2)
    bk_sb = bias_chunks(ph1, "bk_f", 2)
    bv_b = ph1.tile([P, 2 * P], F32, name="bv_b", tag="bv_b")
    bv_src = d["bv_f"]
    nc.sync.dma_start(out=bv_b[:], in_=bass.AP(
        tensor=bv_src.tensor, offset=bv_src.offset,
        ap=[[0, P]] + list(bv_src.ap)))
    # cross-attention keys prefetched behind yT on the sync queue
    for kc in range(4):
        nc.sync.dma_start(out=xT[kc][:], in_=d["xT"][kc * P:(kc + 1) * P, :])

    # ============ ph1 QKV emission (k full, q tcol 3; rest pending) =====
    # psX: shared 1-bank [P,512] ring for q/v projections, W1 partials and
    # LN1 transposes (keeps psS at bufs=2 within the 8-bank budget)
    psX = open_pool(name="psX", bufs=1, space="PSUM", side="left")
    pending = []
    drained = [0]

    def drain(k):
        for _ in range(min(k, len(pending))):
            pending.pop(0)()
            drained[0] += 1

    def px_tile():
        return psX.tile([P, 512], F32, name="px", tag="px", bufs=2)

    def qk_group(dst, w_sb, b_sb, p, tcol):
        ps = px_tile()
        for kc in range(KC):
            nc.tensor.matmul(ps[:], lhsT=w_sb[:, kc, p * P:(p + 1) * P],
                             rhs=yT_all[:, kc, tcol * 512:(tcol + 1) * 512],
                             start=(kc == 0), stop=(kc == KC - 1))
        nc.vector.tensor_scalar(out=dst[p][:, tcol * 512:(tcol + 1) * 512],
                                in0=ps[:], scalar1=b_sb[:, p:p + 1],
                                scalar2=None, op0=ALU.add)

    # k: all tcols (every score block needs all keys); q: tcol 3 first
    for p in range(2):
        for tcol in range(4):
            qk_group(kTp, wk_sb, bk_sb, p, tcol)
    for p in range(2):
        qk_group(qTp, wq_sb, bq_sb, p, 3)

    def v_work(tb):
        work = []
        box = [None]

        def v_start():
            nc.vector.memset(v_sb[tb][:, :, DK:DK + 1], 1.0)
            box[0] = px_tile()

        def v_mm(kc):
            nc.tensor.matmul(box[0][:, 0:2 * P],
                             lhsT=yT_all[:, kc, tb * P:(tb + 1) * P],
                             rhs=wv_sb[:, kc, :],
                             start=(kc == 0), stop=(kc == KC - 1))

        def v_evict():
            nc.vector.tensor_add(
                out=v_sb[tb][:, :, 0:DK],
                in0=box[0][:, 0:2 * P].rearrange("p (h k) -> p h k", h=HL),
                in1=bv_b[:].rearrange("p (h k) -> p h k", h=HL))

        work.append(v_start)
        work.extend(lambda kc=kc: v_mm(kc) for kc in range(KC))
        work.append(v_evict)
        return work

    def q_work(p, tcol):
        work = []
        box = [None]

        def q_start():
            box[0] = px_tile()

        def q_mm(kc):
            nc.tensor.matmul(box[0][:],
                             lhsT=wq_sb[:, kc, p * P:(p + 1) * P],
                             rhs=yT_all[:, kc, tcol * 512:(tcol + 1) * 512],
                             start=(kc == 0), stop=(kc == KC - 1))

        def q_evict():
            nc.vector.tensor_scalar(
                out=qTp[p][:, tcol * 512:(tcol + 1) * 512],
                in0=box[0][:], scalar1=bq_sb[:, p:p + 1],
                scalar2=None, op0=ALU.add)

        work.append(q_start)
        work.extend(lambda kc=kc: q_mm(kc) for kc in range(KC))
        work.append(q_evict)
        return work

    # v must be fully projected before the first attnV drains; emit v
    # first, then the remaining q columns.
    for tb in range(TB):
        pending.extend(v_work(tb))
    for tcol in (2, 1, 0):
        for p in range(2):
            pending.extend(q_work(p, tcol))
    ph1_work_n = len(pending)

    # ============ ph2: causal attention + W1 + ReduceScatter ============
    cat = open_pool(name="cat", bufs=1, side="right")     # catT, ph2-long
    catT = [cat.tile([P, T], BF16, name=f"catT{i}", tag=f"catT{i}")
            for i in range(2)]
    ph3 = open_pool(name="ph3", bufs=1, side="right")     # W1/LN1 working
    w1_sb = ph3.tile([P, 2, D], BF16, name="w1", tag="w1")
    nc.gpsimd.dma_start(out=w1_sb[:],
                        in_=d["w1loc"].rearrange("p (c n) -> p c n", c=2))
    a1pre_box = {}
    if not f["b1_zero"]:
        f["b1_b"] = bcast_row(ph3, "b1")
    if not f["ln1_unit_g"]:
        f["g1_b"] = bcast_row(ph3, "ln1_g")
    if not f["ln1_zero_b"]:
        f["be1_b"] = bcast_row(ph3, "ln1_b")

    dramp = open_pool(name="dramp", bufs=1, space="DRAM", side="left")
    rs_in = [dramp.tile([4 * P, D], BF16, name=f"rsi{i}", tag=f"rsi{i}")
             for i in range(RB)]
    rs_out = [dramp.tile([P, D], BF16, name=f"rso{i}", tag=f"rso{i}")
              for i in range(RB)]

    psS = open_pool(name="psS", bufs=1, space="PSUM", side="left")
    psV = open_pool(name="psV", bufs=1, space="PSUM", side="left")

    def make_attn_work(rc, p, expP):
        nkb = 4 * rc + 4
        work = []
        pa_t = [None, None]

        def start_head(hh):
            pa_t[hh] = psV.tile([DK + 1, 512], F32, name="pa", tag="pa",
                                bufs=2)

        def mm_head(hh, kb):
            hl = 2 * p + hh
            nc.tensor.matmul(pa_t[hh][:], lhsT=v_sb[kb][:, hl, :],
                             rhs=expP[:, kb, hh, :],
                             start=(kb == 0), stop=(kb == nkb - 1))

        den_row = [None]

        def evict_den(hh):
            if hh == 0:
                # f32: RECIPROCAL with bf16 input hits a ~40x slower path
                den_row[0] = ph2w.tile([1, 2 * 512], F32, name="den_row",
                                       tag="den_row", bufs=2)
            nc.vector.tensor_copy(out=den_row[0][:, hh * 512:(hh + 1) * 512],
                                  in_=pa_t[hh][DK:DK + 1, :])

        recB = [None]

        def recip_bcast():
            rec_row = ph2w.tile([1, 2 * 512], BF16, name="rec_row",
                                tag="rec_row", bufs=2)
            with nc.allow_low_precision(reason="softmax denom bf16 ok"):
                nc.vector.reciprocal(out=rec_row[:], in_=den_row[0][:])
            recB[0] = ph2w.tile([DK, 2 * 512], BF16, name="recB", tag="recB",
                                bufs=1)
            nc.gpsimd.partition_broadcast(recB[0][:], rec_row[:])

        def mul_head(hh):
            nc.vector.tensor_mul(
                out=catT[p][hh * DK:(hh + 1) * DK,
                            rc * 512:(rc + 1) * 512],
                in0=pa_t[hh][0:DK, :],
                in1=recB[0][:, hh * 512:(hh + 1) * 512])

        for hh in range(2):
            work.append(lambda hh=hh: start_head(hh))
            for kb in range(nkb):
                work.append(lambda hh=hh, kb=kb: mm_head(hh, kb))
            work.append(lambda hh=hh: evict_den(hh))
        work.append(recip_bcast)
        work.append(lambda: mul_head(0))
        work.append(lambda: mul_head(1))
        return work

    y_box = {}

    def make_w1_work(rc):
        work = []

        def w1_block(rb, nt, box):
            c0 = rc * 512 + rb * P
            if nt == 0:
                box[0] = ph2w.tile([P, D], BF16, name="a1p", tag="a1p",
                                   bufs=2)
            psw = px_tile()
            for kc2 in range(2):
                nc.tensor.matmul(psw[:],
                                 lhsT=catT[kc2][:, c0:c0 + P],
                                 rhs=w1_sb[:, kc2, nt * 512:(nt + 1) * 512],
                                 start=(kc2 == 0), stop=(kc2 == 1))
            nc.vector.tensor_copy(out=box[0][:, nt * 512:(nt + 1) * 512],
                                  in_=psw[:])
            if nt == 1:
                nc.gpsimd.dma_start(out=rs_in[rc][rb * P:(rb + 1) * P, :],
                                    in_=box[0][:])

        for rb in range(4):
            box = [None]
            for nt in range(2):
                work.append(lambda rb=rb, nt=nt, box=box: w1_block(rb, nt, box))

        def do_rs():
            nc.gpsimd.collective_compute(
                "ReduceScatter", ALU.add, replica_groups=RG,
                ins=[rs_in[rc][:].opt()], outs=[rs_out[rc][:].opt()])
            # y residual rows for this chunk (sync queue, no waits)
            yb = ph3.tile([P, D], F32, name="y_sb", tag="y_sb", bufs=1)
            nc.sync.dma_start(out=yb[:],
                              in_=d["y_rows"][rc * P:(rc + 1) * P, :])
            y_box[rc] = yb
            # RS result lands via sync queue (gpsimd must stay unblocked)
            ap = ph3.tile([P, D], BF16, name="a1pre", tag="a1pre", bufs=2)
            nc.sync.dma_start(out=ap[:], in_=rs_out[rc][:])
            a1pre_box[rc] = ap
        work.append(do_rs)
        return work

    def make_ln1_work(rc):
        work = []
        a1 = [None]
        fold = f["ln1_unit_g"] and f["ln1_zero_b"]

        def residual():
            a1[0] = ph3.tile([P, D], F32, name="a1", tag="a1", bufs=1)
            nc.vector.tensor_add(out=a1[0][:], in0=a1pre_box[rc][:],
                                 in1=y_box[rc][:])
            if not f["b1_zero"]:
                nc.vector.tensor_add(out=a1[0][:], in0=a1[0][:],
                                     in1=f["b1_b"][:])

        a1b = [None]

        def ln():
            a1b[0] = ph3.tile([P, D], BF16, name="a1b", tag="a1b", bufs=1)
            if fold:
                # rstd folds SCALE -> a1T comes out pre-scaled
                ln_slim(ph3, a1[0][:], a1b[0][:], eps_s2,
                        var_scale=1.0 / (SCALE * SCALE))
            else:
                ln_slim(ph3, a1[0][:], a1b[0][:], eps_sb,
                        g_b=None if f["ln1_unit_g"] else f["g1_b"][:],
                        be_b=None if f["ln1_zero_b"] else f["be1_b"][:])

        def tr(kc):
            pt = px_tile()[:].bitcast(BF16)[:, 0:P]   # bf16 view of psum
            nc.tensor.transpose(pt, a1b[0][:, kc * P:(kc + 1) * P],
                                ident[:])
            if fold:
                nc.vector.tensor_copy(out=a1T[kc][:, rc * P:(rc + 1) * P],
                                      in_=pt)
            else:
                nc.vector.tensor_scalar(
                    out=a1T[kc][:, rc * P:(rc + 1) * P], in0=pt,
                    scalar1=float(SCALE), scalar2=None, op0=ALU.mult)

        work.append(residual)
        work.append(ln)
        work.extend(lambda kc=kc: tr(kc) for kc in range(KC))
        return work

    ph1_closed = False
    prev_ln1 = None
    for rc in (3, 2, 1, 0):
        nkb = 4 * rc + 4
        dn = 7 if rc == 3 else 4
        for p in range(2):
            # expP[kb][i] holds exp(scores) for head 2p+i, keys block kb
            expP = ph2w.tile([P, TB, 2, 512], BF16, name="expP", tag="expP",
                             bufs=2)
            for kb in range(nkb):
                ps = psS.tile([P, 1024], F32, name="ps_sc", tag="ps_sc",
                              bufs=2)
                nc.tensor.matmul(ps[:, 0:512],
                                 lhsT=kTp[p][0:DK, kb * P:(kb + 1) * P],
                                 rhs=qTp[p][0:DK, rc * 512:(rc + 1) * 512],
                                 start=True, stop=True,
                                 tile_position=(0, 0))
                nc.tensor.matmul(ps[:, 512:1024],
                                 lhsT=kTp[p][DK:P, kb * P:(kb + 1) * P],
                                 rhs=qTp[p][DK:P, rc * 512:(rc + 1) * 512],
                                 start=True, stop=True,
                                 tile_position=(64, 0))
                nc.scalar.activation(
                    out=expP[:, kb, :, :],
                    in_=ps[:].rearrange("p (h r) -> p h r", h=2),
                    func=AF.Exp)
                if kb >= 4 * rc:       # diagonal block: apply causal mask
                    i = kb - 4 * rc
                    for hh in range(2):
                        nc.vector.tensor_mul(out=expP[:, kb, hh, :],
                                             in0=expP[:, kb, hh, :],
                                             in1=mask4[:, i, :])
                drain(dn)
            pending.extend(make_attn_work(rc, p, expP))
        # previous chunk's LN1 is emitted only now: its first DVE op waits
        # on that chunk's RS, and emitting it too early stalls the whole
        # in-order DVE queue behind the collective
        if prev_ln1 is not None:
            pending.extend(prev_ln1)
        pending.extend(make_w1_work(rc))
        prev_ln1 = make_ln1_work(rc)
        if not ph1_closed:
            # all q/v pending work must be emitted before ph1 frees
            ph1_closed = True
            drain(max(0, ph1_work_n - drained[0]))
            close_pool(ph1)
    pending.extend(prev_ln1)     # rc=0's LN1
    drain(len(pending))
    close_pool(psV)
    close_pool(psS)
    close_pool(ph3)
    close_pool(cat)
    close_pool(ph2w)
    close_pool(psX)
    close_pool(attn)

    # ================= Phase 4: cross-attention =========================
    fw = open_pool(name="fw", bufs=1, side="left")        # Wf1, lives to FFN1
    wf1_all = fw.tile([P, KC, DF], BF16, name="wf1", tag="wf1")
    at2p = open_pool(name="at2p", bufs=1, side="left")    # at2T, into ph5
    at2T = [at2p.tile([P, R], BF16, name=f"at2T{i}", tag=f"at2T{i}")
            for i in range(KC)]
    w2p = open_pool(name="w2p", bufs=1, side="left")      # W2sum, into ph5
    w2 = [w2p.tile([P, D], BF16, name=f"w2_{i}", tag=f"w2_{i}")
          for i in range(KC)]
    ph4 = open_pool(name="ph4", bufs=1, side="left")
    pp4 = open_pool(name="pp4", bufs=4, space="PSUM", side="left")
    pd4 = open_pool(name="pd4", bufs=1, space="PSUM", side="left")
    for kc in range(4, KC):
        xT[kc] = ph4.tile([P, T], BF16, name=f"xT{kc}", tag=f"xT{kc}")
        nc.sync.dma_start(out=xT[kc][:],
                          in_=d["xT"][kc * P:(kc + 1) * P, :])
    # x_tm: single strided DMA (one descriptor on the scalar queue)
    x_tm = ph4.tile([P, TB, D], BF16, name="xtm", tag="xtm")
    nc.scalar.dma_start(out=x_tm[:],
                        in_=d["x_tm"].rearrange("(t p) d -> p t d", p=P))

    p2T = [ph4.tile([P, R], BF16, name=f"p2T{i}", tag=f"p2T{i}")
           for i in range(TB)]
    for tb in range(TB):
        ps = pp4.tile([P, 512], F32, name="ps4", tag="ps4")
        for kc in range(KC):
            nc.tensor.matmul(ps[:], lhsT=xT[kc][:, tb * P:(tb + 1) * P],
                             rhs=a1T[kc][:, :],
                             start=(kc == 0), stop=(kc == KC - 1))
        nc.scalar.activation(out=p2T[tb][:], in_=ps[:], func=AF.Exp)
    # Wf1/W2sum loads issued only now: their 10MB would otherwise contend
    # with the last ReduceScatter's network traffic
    nc.gpsimd.dma_start(out=wf1_all[:],
                        in_=d["Wf1"].rearrange("p (c f) -> p c f", c=KC))
    for kc in range(KC):
        nc.gpsimd.dma_start(out=w2[kc][:],
                            in_=d["W2sum"][kc * P:(kc + 1) * P, :])
    # denominator: 4 col-tiled ones-matmul accumulators run concurrently
    pd = pd4.tile([P, R], F32, name="ps_d2", tag="ps_d2")
    for g in range(4):
        for u in range(4):
            tb = 4 * g + u
            nc.tensor.matmul(pd[32 * g:32 * g + 1, :], lhsT=ones_col[:],
                             rhs=p2T[tb][:], start=(u == 0), stop=(u == 3),
                             tile_position=(0, 32 * g))
    den4 = ph4.tile([1, 4, R], F32, name="den4", tag="den4")
    for g in range(4):
        nc.vector.tensor_copy(out=den4[:, g, :], in_=pd[32 * g:32 * g + 1, :])
    den2a = ph4.tile([1, R], F32, name="den2a", tag="den2a")
    den2b = ph4.tile([1, R], F32, name="den2b", tag="den2b")
    den2 = ph4.tile([1, R], F32, name="den2", tag="den2")
    nc.vector.tensor_add(out=den2a[:], in0=den4[:, 0, :], in1=den4[:, 1, :])
    nc.vector.tensor_add(out=den2b[:], in0=den4[:, 2, :], in1=den4[:, 3, :])
    nc.vector.tensor_add(out=den2[:], in0=den2a[:], in1=den2b[:])
    recip2 = ph4.tile([1, R], BF16, name="recip2", tag="recip2")
    with nc.allow_low_precision(reason="softmax denom bf16 ok"):
        nc.vector.reciprocal(out=recip2[:], in_=den2[:])
    psb2 = pd4.tile([P, R], F32, name="psb2", tag="psb2")
    nc.tensor.matmul(psb2[:], lhsT=ones_row[:], rhs=recip2[:],
                     start=True, stop=True)
    recip2b = ph4.tile([P, R], F32, name="recip2b", tag="recip2b")
    nc.vector.tensor_copy(out=recip2b[:], in_=psb2[:])
    for db in range(KC):
        ps = pp4.tile([P, 512], F32, name="ps4", tag="ps4")
        for tb in range(TB):
            nc.tensor.matmul(ps[:], lhsT=x_tm[:, tb, db * P:(db + 1) * P],
                             rhs=p2T[tb][:],
                             start=(tb == 0), stop=(tb == TB - 1))
        nc.vector.tensor_mul(out=at2T[db][:], in0=ps[:], in1=recip2b[:])
    close_pool(pd4)
    close_pool(pp4)
    close_pool(ph4)
    close_pool(a1pl)
    close_pool(xpre)

    # ========= Phase 5: W2sum + residual + LN2, produce a2T =============
    a2p = open_pool(name="a2p", bufs=1, side="right")     # a2T into ph6
    a2T = [a2p.tile([P, R], BF16, name=f"a2T{i}", tag=f"a2T{i}")
           for i in range(KC)]
    ph5 = open_pool(name="ph5", bufs=1, side="right")
    pp5 = open_pool(name="pp5", bufs=4, space="PSUM", side="left")
    pt5 = open_pool(name="pt5", bufs=2, space="PSUM", side="left")
    if not f["b2_zero"]:
        f["b2_b"] = bcast_row(ph5, "b2")
    if not f["ln2_unit_g"]:
        f["g2_b"] = bcast_row(ph5, "ln2_g")
    if not f["ln2_zero_b"]:
        f["be2_b"] = bcast_row(ph5, "ln2_b")
    for rb in range(RB):
        y5 = ph5.tile([P, D], F32, name="y5", tag="y5", bufs=2)
        nc.sync.dma_start(out=y5[:], in_=d["y_rows"][rb * P:(rb + 1) * P, :])
        a2 = ph5.tile([P, D], F32, name="a2", tag="a2", bufs=2)
        for nt in range(2):
            ps = pp5.tile([P, 512], F32, name="ps_a2", tag="ps_a2")
            for kc in range(KC):
                nc.tensor.matmul(ps[:],
                                 lhsT=at2T[kc][:, rb * P:(rb + 1) * P],
                                 rhs=w2[kc][:, nt * 512:(nt + 1) * 512],
                                 start=(kc == 0), stop=(kc == KC - 1))
            sl = slice(nt * 512, (nt + 1) * 512)
            nc.vector.tensor_add(out=a2[:, sl], in0=ps[:], in1=y5[:, sl])
            if not f["b2_zero"]:
                nc.vector.tensor_add(out=a2[:, sl], in0=a2[:, sl],
                                     in1=f["b2_b"][:, sl])
        a2b = ph5.tile([P, D], BF16, name="a2b", tag="a2b", bufs=2)
        ln_slim(ph5, a2[:], a2b[:], eps_sb,
                g_b=None if f["ln2_unit_g"] else f["g2_b"][:],
                be_b=None if f["ln2_zero_b"] else f["be2_b"][:])
        for kc in range(KC):
            pt = pt5.tile([P, P], BF16, name="pt_a2", tag="pt_a2")
            nc.tensor.transpose(pt[:], a2b[:, kc * P:(kc + 1) * P], ident[:])
            nc.vector.tensor_copy(out=a2T[kc][:, rb * P:(rb + 1) * P],
                                  in_=pt[:])
    close_pool(pt5)
    close_pool(pp5)
    close_pool(ph5)
    close_pool(w2p)
    close_pool(at2p)

    # ========== Phase 6: FFN + residual + LN3 ===========================
    fA = open_pool(name="fA", bufs=1, side="right")
    f1T = [fA.tile([P, R], BF16, name=f"f1T{i}", tag=f"f1T{i}")
           for i in range(FB)]
    bf1_sb = bias_chunks(fA, "bf1", FB)
    pfA = open_pool(name="pfA", bufs=3, space="PSUM", side="left")
    for fb in range(FB):
        ps = pfA.tile([P, 512], F32, name="ps_f1", tag="ps_f1")
        for kc in range(KC):
            nc.tensor.matmul(ps[:], lhsT=wf1_all[:, kc, fb * P:(fb + 1) * P],
                             rhs=a2T[kc][:, :],
                             start=(kc == 0), stop=(kc == KC - 1))
        # relu + bias on ACT (idle during the FFN)
        nc.scalar.activation(out=f1T[fb][:], in_=ps[:], func=AF.Relu,
                             bias=bf1_sb[:, fb:fb + 1], scale=1.0)
    close_pool(pfA)
    close_pool(fw)

    pfB = open_pool(name="pfB", bufs=1, space="PSUM", side="left")
    fB = open_pool(name="fB", bufs=1, side="right")
    ps_rb = [pfB.tile([P, D], F32, name=f"ps_rb{i}", tag=f"ps_rb{i}")
             for i in range(RB)]
    y6 = [fB.tile([P, D], F32, name=f"y6{i}", tag=f"y6{i}")
          for i in range(RB)]
    for rb in range(RB):
        nc.sync.dma_start(out=y6[rb][:],
                          in_=d["y_rows"][rb * P:(rb + 1) * P, :])
    if not f["bf2_zero"]:
        f["bf2_b"] = bcast_row(fB, "bf2")
    if not f["ln3_unit_g"]:
        f["g3_b"] = bcast_row(fB, "ln3_g")
    if not f["ln3_zero_b"]:
        f["be3_b"] = bcast_row(fB, "ln3_b")
    wf2_t = {}
    for fb in range(FB):
        wf2_fb = fB.tile([P, D], BF16, name="wf2s", tag="wf2s", bufs=6)
        nc.sync.dma_start(out=wf2_fb[:], in_=d["Wf2"][fb * P:(fb + 1) * P, :])
        wf2_t[fb] = wf2_fb
        if fb < FB - 4:
            for rb in range(RB):
                for nt in range(2):
                    nc.tensor.matmul(
                        ps_rb[rb][:, nt * 512:(nt + 1) * 512],
                        lhsT=f1T[fb][:, rb * P:(rb + 1) * P],
                        rhs=wf2_fb[:, nt * 512:(nt + 1) * 512],
                        start=(fb == 0), stop=False)

    def tail(rb):
        ff = fB.tile([P, D], F32, name="ff", tag="ff", bufs=2)
        nc.vector.tensor_add(out=ff[:], in0=ps_rb[rb][:], in1=y6[rb][:])
        if not f["bf2_zero"]:
            nc.vector.tensor_add(out=ff[:], in0=ff[:], in1=f["bf2_b"][:])
        o = fB.tile([P, D], F32, name="o", tag="o", bufs=2)
        ln_slim(fB, ff[:], o[:], eps_sb,
                g_b=None if f["ln3_unit_g"] else f["g3_b"][:],
                be_b=None if f["ln3_zero_b"] else f["be3_b"][:])
        nc.sync.dma_start(out=out_d[rb * P:(rb + 1) * P, :], in_=o[:])

    # last 4 fb row-major: each row block finishes early and its LN3+store
    # overlaps the remaining matmuls
    for rb in range(RB):
        for fb in range(FB - 4, FB):
            for nt in range(2):
                nc.tensor.matmul(ps_rb[rb][:, nt * 512:(nt + 1) * 512],
                                 lhsT=f1T[fb][:, rb * P:(rb + 1) * P],
                                 rhs=wf2_t[fb][:, nt * 512:(nt + 1) * 512],
                                 start=False, stop=(fb == FB - 1))
        tail(rb)
    close_pool(fB)
    close_pool(pfB)
    close_pool(fA)
    close_pool(a2p)
    close_pool(dramp)
    close_pool(const)


def _row_idx(j):
    return np.concatenate(
        [np.arange(512 * rc + 128 * j, 512 * rc + 128 * j + 128)
         for rc in range(4)])


def _flags(inputs):
    z = lambda a: bool(np.all(np.asarray(a) == 0.0))
    u = lambda a: bool(np.all(np.asarray(a) == 1.0))
    return {
        "b1_zero": z(inputs["b1"]), "b2_zero": z(inputs["b2"]),
        "bf2_zero": z(inputs["bf2"]),
        "ln1_unit_g": u(inputs["ln1_g"]), "ln1_zero_b": z(inputs["ln1_b"]),
        "ln2_unit_g": u(inputs["ln2_g"]), "ln2_zero_b": z(inputs["ln2_b"]),
        "ln3_unit_g": u(inputs["ln3_g"]), "ln3_zero_b": z(inputs["ln3_b"]),
    }


def _prep_host(inputs):
    f32 = lambda a: np.ascontiguousarray(np.asarray(a, np.float32))
    bf = lambda a: np.ascontiguousarray(
        np.asarray(a, np.float32).astype(ml_dtypes.bfloat16))
    x = f32(inputs["x"])
    y = f32(inputs["y"])
    mask = np.asarray(inputs["y_mask"]).astype(np.float32)
    # diagonal-block masks: mask4[ky, i, r] = mask[r, 128*i + ky]
    m4 = mask[0:512, 0:512].reshape(512, 4, 128).transpose(2, 1, 0)
    Wq = f32(inputs["Wq"])   # [H, D, DK]
    Wk = f32(inputs["Wk"])
    Wv = f32(inputs["Wv"])

    def chunkP(a):
        """[C*P, F] -> [P, C*F] so each partition's data is contiguous."""
        cp, fdim = a.shape
        return np.ascontiguousarray(
            a.reshape(cp // P, P, fdim).transpose(1, 0, 2).reshape(P, -1))

    shared = {
        "mask4": bf(m4),
        "b1": f32(inputs["b1"]),
        "ln1_g": f32(inputs["ln1_g"]), "ln1_b": f32(inputs["ln1_b"]),
        "W2sum": bf(f32(inputs["W2"]).reshape(H, D, D).sum(0)),
        "b2": f32(inputs["b2"]),
        "ln2_g": f32(inputs["ln2_g"]), "ln2_b": f32(inputs["ln2_b"]),
        "Wf1": chunkP(bf(inputs["Wf1"])),
        "bf1": f32(inputs["bf1"]),
        "Wf2": bf(inputs["Wf2"]),
        "bf2": f32(inputs["bf2"]),
        "ln3_g": f32(inputs["ln3_g"]), "ln3_b": f32(inputs["ln3_b"]),
    }
    in_maps = []
    for c in range(NCORES):
        b, j = c // 4, c % 4
        hh = slice(4 * j, 4 * j + 4)
        ridx = _row_idx(j)
        in_maps.append({
            "yT": bf(y[b].T),
            "wq": chunkP(bf(Wq[hh].transpose(1, 0, 2).reshape(D, 256) * SCALE)),
            "wk": chunkP(bf(Wk[hh].transpose(1, 0, 2).reshape(D, 256))),
            "wv": chunkP(bf(Wv[hh].transpose(1, 0, 2).reshape(D, 256))),
            "bq_s": f32(inputs["bq"])[hh].reshape(256) * np.float32(SCALE),
            "bk_f": f32(inputs["bk"])[hh].reshape(256),
            "bv_f": f32(inputs["bv"])[hh].reshape(256),
            "w1loc": chunkP(bf(f32(inputs["W1"])[256 * j:256 * (j + 1), :])),
            "y_rows": np.ascontiguousarray(y[b][ridx]),
            "xT": bf(x[b].T),
            "x_tm": bf(x[b]),
            **shared,
        })
    return in_maps


def kernel(**inputs):
    fl = _flags(inputs)
    key = tuple(sorted(fl.items()))
    if key not in _cached:
        _cached[key] = build_nc(dict(fl))
    nc = _cached[key]
    in_maps = _prep_host(inputs)
    res = run_bass_kernel_spmd(nc, in_maps, core_ids=list(range(NCORES)))
    out = np.zeros((B, S, D), np.float32)
    for c in range(NCORES):
        b, j = c // 4, c % 4
        out[b, _row_idx(j)] = res.results[c]["out"]
    return out


# revision 36
# speedup vs baseline: 1.0706x; 1.0426x over previous
"""Trainium2 Bass kernel for nn_DecoderLayer_33758442946809.

Sharding (8 cores = 2 batches x 4-core groups):
- Self-attention is HEAD-sharded: core (b, j) computes heads 4j..4j+3 for
  all T=2048 rows of batch b; causal skipping is SPMD-uniform (only
  lower-triangle key blocks are scored/exp'd).
- W1 is row-parallel over the head-sharded cat features; partials are
  summed with a chunked ReduceScatter (bf16) over each 4-core group.
  After the RS, core (b, j) owns the strided row set
  {512*rc + 128*j + i : rc<4, i<128}; cross-attention, W2 and the FFN
  are data-parallel over those rows.
- tile(attn2, H) @ W2 == attn2 @ sum_h W2[h] (host precomputes the sum).

v2 schedule (vs the phase-serial v1):
- Row chunks processed big-first [3,2,1,0] so the last ReduceScatter has
  the shortest dependency tail.
- Score PSUM is [128,1024] tiles (1 key block, both heads) with bufs=2,
  so ACT exp streams without stalling on PSUM WAR.
- Softmax normalization: denominator row is reciprocal'd at [1,2,512]
  (cheap) then partition-broadcast; the numerator is multiplied straight
  out of PSUM -- no [65,512] evictions, no 6.5us [64,1024] reciprocals.
- QKV projection matmuls (q tcols 2..0, all of v) drain as pending PE
  work under the first row-chunk's exp stream.
- RS outputs land via the sync queue (the gpsimd queue used to block
  ~90us on the RS-done semaphore).
- LN affine ops are skipped when gamma==1/beta==0 (host-checked program
  variant); the attention 1/sqrt(dk) scale is folded into LN1's rstd.
- Transposes run in bf16 (pre-cast) -- 1 PE cycle/row instead of 2.
- xT / x_tm / Wf1 / W2sum are prefetched a phase early.
- FFN1 relu+bias runs on ACT (idle there) instead of DVE.
- FFN2's last 4 weight blocks run row-major so row block 0 finishes
  ~6us early and the final LN3+store overlaps the remaining matmuls.
"""
import math
import sys

import numpy as np

sys.path.insert(0, "/opt/trn_rl_repo")

import ml_dtypes  # noqa: E402

import concourse.bass as bass  # noqa: E402
import concourse.tile as tile  # noqa: E402
from concourse import bacc, mybir  # noqa: E402
from concourse.bass_utils import run_bass_kernel_spmd  # noqa: E402
from concourse.masks import make_identity  # noqa: E402

B, S, D, H, DF = 2, 2048, 1024, 16, 4096
DK = D // H                      # 64
P = 128
T = S                            # rows/keys per batch
R = 512                          # own rows per core (after RS)
KC = D // P                      # 8 contraction chunks of D
TB = T // P                      # 16 key blocks
RB = R // P                      # 4 row blocks
FB = DF // P                     # 32 ffn blocks
NCORES = 8
HL = 4                           # local heads per core
SCALE = 1.0 / math.sqrt(DK)
RG = [[0, 1, 2, 3], [4, 5, 6, 7]]

F32 = mybir.dt.float32
BF16 = mybir.dt.bfloat16
AF = mybir.ActivationFunctionType
ALU = mybir.AluOpType

_cached = {}


def build_nc(f):
    nc = bacc.Bacc("TRN2", target_bir_lowering=False, debug=False,
                   num_devices=NCORES)

    dram = {}

    def din(name, shape, dt):
        dram[name] = nc.dram_tensor(name, shape, dt, kind="ExternalInput").ap()

    din("yT", [D, T], BF16)          # y[b].T
    din("wq", [P, KC * HL * DK], BF16)   # pre-chunked [p][kc][f] layout
    din("wk", [P, KC * HL * DK], BF16)
    din("wv", [P, KC * HL * DK], BF16)
    din("bq_s", [HL * DK], F32)      # bq * SCALE, local heads
    din("bk_f", [HL * DK], F32)
    din("bv_f", [HL * DK], F32)
    din("mask4", [P, 4, R], BF16)    # diagonal-block masks (key, i, row)
    din("w1loc", [P, 2 * D], BF16)   # W1 rows owned by this core, pre-chunked
    din("b1", [D], F32)
    din("ln1_g", [D], F32)
    din("ln1_b", [D], F32)
    din("y_rows", [R, D], F32)       # this core's (strided) y rows
    din("xT", [D, T], BF16)
    din("x_tm", [T, D], BF16)
    din("W2sum", [D, D], BF16)
    din("b2", [D], F32)
    din("ln2_g", [D], F32)
    din("ln2_b", [D], F32)
    din("Wf1", [P, KC * DF], BF16)   # pre-chunked [p][kc][f] layout
    din("bf1", [DF], F32)
    din("Wf2", [DF, D], BF16)
    din("bf2", [D], F32)
    din("ln3_g", [D], F32)
    din("ln3_b", [D], F32)
    out_d = nc.dram_tensor("out", [R, D], F32, kind="ExternalOutput").ap()

    with tile.TileContext(nc) as tc:
        _build(nc, tc, dram, out_d, f)
    nc.compile()
    return nc


def _build(nc, tc, d, out_d, f):
    pool_cms = {}

    def open_pool(*args, **kw):
        cm = tc.tile_pool(*args, **kw)
        p = cm.__enter__()
        pool_cms[id(p)] = cm
        return p

    def close_pool(p):
        pool_cms.pop(id(p)).__exit__(None, None, None)

    const = open_pool(name="const", bufs=1, side="left")
    ident = const.tile([P, P], BF16, name="ident", tag="ident")
    make_identity(nc, ident[:])
    ones_col = const.tile([P, 1], BF16, name="ones_col", tag="ones_col")
    nc.vector.memset(ones_col[:], 1.0)
    ones_row = const.tile([1, P], BF16, name="ones_row", tag="ones_row")
    nc.vector.memset(ones_row[:], 1.0)
    eps_sb = const.tile([P, 1], F32, name="eps", tag="eps")
    nc.vector.memset(eps_sb[:], 1e-5)
    # eps/SCALE^2: sqrt(var/SCALE^2 + eps/SCALE^2) = sqrt(var+eps)/SCALE,
    # so LN1's rstd comes out pre-multiplied by the attention scale.
    eps_s2 = const.tile([P, 1], F32, name="eps_s2", tag="eps_s2")
    nc.vector.memset(eps_s2[:], 1e-5 / (SCALE * SCALE))

    def bias_chunks(pool, name, n):
        t = pool.tile([P, n], F32, name=f"bc_{name}", tag=f"bc_{name}")
        nc.sync.dma_start(out=t[:], in_=d[name].rearrange("(n p) -> p n", p=P))
        return t

    def bcast_row(pool, name):
        src = d[name]
        t = pool.tile([P, D], F32, name=f"br_{name}", tag=f"br_{name}")
        bc = bass.AP(tensor=src.tensor, offset=src.offset,
                     ap=[[0, P]] + list(src.ap))
        nc.sync.dma_start(out=t[:], in_=bc)
        return t

    def ln_slim(pool, x_ap, out_ap, eps_ap, var_scale=1.0, g_b=None,
                be_b=None):
        """LayerNorm along the free axis (D) of a token-major [128, D]
        f32 tile into out_ap. var_scale folds a constant into rstd."""
        x3 = x_ap.rearrange("p (n f) -> p n f", f=512)
        stats = pool.tile([P, 2, 6], F32, name="ln_stats", tag="ln_stats",
                          bufs=4)
        for sg in range(2):
            nc.vector.bn_stats(out=stats[:, sg, :], in_=x3[:, sg, :])
        mv = pool.tile([P, 2], F32, name="ln_mv", tag="ln_mv", bufs=4)
        nc.vector.bn_aggr(out=mv[:], in_=stats[:])
        std = pool.tile([P, 1], F32, name="ln_std", tag="ln_std", bufs=4)
        nc.scalar.activation(out=std[:], in_=mv[:, 1:2], func=AF.Sqrt,
                             bias=eps_ap[:], scale=var_scale)
        rstd = pool.tile([P, 1], F32, name="ln_rstd", tag="ln_rstd", bufs=4)
        nc.vector.reciprocal(out=rstd[:], in_=std[:])
        nc.vector.tensor_scalar(out=out_ap, in0=x_ap, scalar1=mv[:, 0:1],
                                scalar2=rstd[:], op0=ALU.subtract,
                                op1=ALU.mult)
        if g_b is not None:
            nc.vector.tensor_mul(out=out_ap, in0=out_ap, in1=g_b)
        if be_b is not None:
            nc.vector.tensor_add(out=out_ap, in0=out_ap, in1=be_b)

    # ======== pools whose tiles live into ph4 (right-side bottom) =======
    xpre = open_pool(name="xpre", bufs=1, side="right")
    # only half of xT is prefetched (SBUF is tight during rc=3);
    # xT[4..7] load at ph4 open and are the last kcs of each score group
    xT = [xpre.tile([P, T], BF16, name=f"xT{i}", tag=f"xT{i}")
          if i < 4 else None for i in range(KC)]
    a1pl = open_pool(name="a1pl", bufs=1, side="right")
    a1T = [a1pl.tile([P, R], BF16, name=f"a1T{i}", tag=f"a1T{i}")
           for i in range(KC)]

    # ===================== input DMAs (spread across queues) ============
    attn = open_pool(name="attn", bufs=1, side="right")  # live through ph2
    qTp = [attn.tile([P, T], BF16, name=f"qTp{i}", tag=f"qTp{i}")
           for i in range(2)]
    kTp = [attn.tile([P, T], BF16, name=f"kTp{i}", tag=f"kTp{i}")
           for i in range(2)]
    v_sb = [attn.tile([P, HL, DK + 1], BF16, name=f"v{i}", tag=f"v{i}")
            for i in range(TB)]
    mask4 = attn.tile([P, 4, R], BF16, name="mask4", tag="mask4")

    ph2w = open_pool(name="ph2w", bufs=1, side="left")   # ph2 working set
    ph1 = open_pool(name="ph1", bufs=1, side="left")     # closed after rc=3
    # yT lands as 4 column slabs so the first k-projection group can
    # start after ~1MB instead of the full 4MB
    yT_all = ph1.tile([P, KC, T], BF16, name="yT", tag="yT")
    wq_sb = ph1.tile([P, KC, 2 * P], BF16, name="wq", tag="wq")
    wk_sb = ph1.tile([P, KC, 2 * P], BF16, name="wk", tag="wk")
    wv_sb = ph1.tile([P, KC, 2 * P], BF16, name="wv", tag="wv")
    yTsrc = d["yT"].rearrange("(c p) t -> p c t", p=P)
    for tcol in range(4):
        sl = slice(tcol * 512, (tcol + 1) * 512)
        nc.sync.dma_start(out=yT_all[:, :, sl], in_=yTsrc[:, :, sl])
    nc.scalar.dma_start(out=wk_sb[:],
                        in_=d["wk"].rearrange("p (c f) -> p c f", c=KC))
    nc.scalar.dma_start(out=wq_sb[:],
                        in_=d["wq"].rearrange("p (c f) -> p c f", c=KC))
    nc.scalar.dma_start(out=wv_sb[:],
                        in_=d["wv"].rearrange("p (c f) -> p c f", c=KC))
    nc.gpsimd.dma_start(out=mask4[:], in_=d["mask4"][:])
    bq_sb = bias_chunks(ph1, "bq_s", 2)
    bk_sb = bias_chunks(ph1, "bk_f", 2)
    bv_b = ph1.tile([P, 2 * P], F32, name="bv_b", tag="bv_b")
    bv_src = d["bv_f"]
    nc.sync.dma_start(out=bv_b[:], in_=bass.AP(
        tensor=bv_src.tensor, offset=bv_src.offset,
        ap=[[0, P]] + list(bv_src.ap)))
    # cross-attention keys prefetched behind yT on the sync queue
    for kc in range(4):
        nc.sync.dma_start(out=xT[kc][:], in_=d["xT"][kc * P:(kc + 1) * P, :])

    # ============ ph1 QKV emission (k full, q tcol 3; rest pending) =====
    # psX: shared 1-bank [P,512] ring for q/v projections, W1 partials and
    # LN1 transposes (keeps psS at bufs=2 within the 8-bank budget)
    psX = open_pool(name="psX", bufs=1, space="PSUM", side="left")
    pending = []
    drained = [0]

    def drain(k):
        for _ in range(min(k, len(pending))):
            pending.pop(0)()
            drained[0] += 1

    def px_tile():
        return psX.tile([P, 512], F32, name="px", tag="px", bufs=2)

    def qk_group(dst, w_sb, b_sb, p, tcol):
        ps = px_tile()
        for kc in range(KC):
            nc.tensor.matmul(ps[:], lhsT=w_sb[:, kc, p * P:(p + 1) * P],
                             rhs=yT_all[:, kc, tcol * 512:(tcol + 1) * 512],
                             start=(kc == 0), stop=(kc == KC - 1))
        nc.vector.tensor_scalar(out=dst[p][:, tcol * 512:(tcol + 1) * 512],
                                in0=ps[:], scalar1=b_sb[:, p:p + 1],
                                scalar2=None, op0=ALU.add)

    # k: all tcols (every score block needs all keys); q: tcol 3 first
    for p in range(2):
        for tcol in range(4):
            qk_group(kTp, wk_sb, bk_sb, p, tcol)
    for p in range(2):
        qk_group(qTp, wq_sb, bq_sb, p, 3)

    def v_work(tb):
        work = []
        box = [None]

        def v_start():
            nc.vector.memset(v_sb[tb][:, :, DK:DK + 1], 1.0)
            box[0] = px_tile()

        def v_mm(kc):
            nc.tensor.matmul(box[0][:, 0:2 * P],
                             lhsT=yT_all[:, kc, tb * P:(tb + 1) * P],
                             rhs=wv_sb[:, kc, :],
                             start=(kc == 0), stop=(kc == KC - 1))

        def v_evict():
            nc.vector.tensor_add(
                out=v_sb[tb][:, :, 0:DK],
                in0=box[0][:, 0:2 * P].rearrange("p (h k) -> p h k", h=HL),
                in1=bv_b[:].rearrange("p (h k) -> p h k", h=HL))

        work.append(v_start)
        work.extend(lambda kc=kc: v_mm(kc) for kc in range(KC))
        work.append(v_evict)
        return work

    def q_work(p, tcol):
        work = []
        box = [None]

        def q_start():
            box[0] = px_tile()

        def q_mm(kc):
            nc.tensor.matmul(box[0][:],
                             lhsT=wq_sb[:, kc, p * P:(p + 1) * P],
                             rhs=yT_all[:, kc, tcol * 512:(tcol + 1) * 512],
                             start=(kc == 0), stop=(kc == KC - 1))

        def q_evict():
            nc.vector.tensor_scalar(
                out=qTp[p][:, tcol * 512:(tcol + 1) * 512],
                in0=box[0][:], scalar1=bq_sb[:, p:p + 1],
                scalar2=None, op0=ALU.add)

        work.append(q_start)
        work.extend(lambda kc=kc: q_mm(kc) for kc in range(KC))
        work.append(q_evict)
        return work

    # v must be fully projected before the first attnV drains; emit v
    # first, then the remaining q columns.
    for tb in range(TB):
        pending.extend(v_work(tb))
    for tcol in (2, 1, 0):
        for p in range(2):
            pending.extend(q_work(p, tcol))
    ph1_work_n = len(pending)

    # ============ ph2: causal attention + W1 + ReduceScatter ============
    cat = open_pool(name="cat", bufs=1, side="right")     # catT, ph2-long
    catT = [cat.tile([P, T], BF16, name=f"catT{i}", tag=f"catT{i}")
            for i in range(2)]
    ph3 = open_pool(name="ph3", bufs=1, side="right")     # W1/LN1 working
    w1_sb = ph3.tile([P, 2, D], BF16, name="w1", tag="w1")
    nc.gpsimd.dma_start(out=w1_sb[:],
                        in_=d["w1loc"].rearrange("p (c n) -> p c n", c=2))
    a1pre_box = {}
    if not f["b1_zero"]:
        f["b1_b"] = bcast_row(ph3, "b1")
    if not f["ln1_unit_g"]:
        f["g1_b"] = bcast_row(ph3, "ln1_g")
    if not f["ln1_zero_b"]:
        f["be1_b"] = bcast_row(ph3, "ln1_b")

    dramp = open_pool(name="dramp", bufs=1, space="DRAM", side="left")
    rs_in = [dramp.tile([4 * P, D], BF16, name=f"rsi{i}", tag=f"rsi{i}")
             for i in range(RB)]
    rs_out = [dramp.tile([P, D], BF16, name=f"rso{i}", tag=f"rso{i}")
              for i in range(RB)]

    psS = open_pool(name="psS", bufs=1, space="PSUM", side="left")
    psV = open_pool(name="psV", bufs=1, space="PSUM", side="left")

    def make_attn_work(rc, p, expP):
        nkb = 4 * rc + 4
        work = []
        pa_t = [None, None]

        def start_head(hh):
            pa_t[hh] = psV.tile([DK + 1, 512], F32, name="pa", tag="pa",
                                bufs=2)

        def mm_head(hh, kb):
            hl = 2 * p + hh
            nc.tensor.matmul(pa_t[hh][:], lhsT=v_sb[kb][:, hl, :],
                             rhs=expP[:, kb, hh, :],
                             start=(kb == 0), stop=(kb == nkb - 1))

        den_row = [None]

        def evict_den(hh):
            if hh == 0:
                # f32: RECIPROCAL with bf16 input hits a ~40x slower path
                den_row[0] = ph2w.tile([1, 2 * 512], F32, name="den_row",
                                       tag="den_row", bufs=1)
            nc.vector.tensor_copy(out=den_row[0][:, hh * 512:(hh + 1) * 512],
                                  in_=pa_t[hh][DK:DK + 1, :])

        recB = [None]

        def recip_bcast():
            # plain RECIPROCAL costs ~6.4ns per free element (iterative
            # divide); the approx version is ~5x faster and 18-bit exact
            rec_f = ph2w.tile([1, 2 * 512], F32, name="rec_f",
                              tag="rec_f", bufs=1)
            nc.vector.reciprocal_approx_fast(rec_f[:], den_row[0][:])
            rec_row = ph2w.tile([1, 2 * 512], BF16, name="rec_row",
                                tag="rec_row", bufs=1)
            nc.vector.tensor_copy(out=rec_row[:], in_=rec_f[:])
            recB[0] = ph2w.tile([DK, 2 * 512], BF16, name="recB", tag="recB",
                                bufs=1)
            nc.gpsimd.partition_broadcast(recB[0][:], rec_row[:])

        def mul_head(hh):
            nc.vector.tensor_mul(
                out=catT[p][hh * DK:(hh + 1) * DK,
                            rc * 512:(rc + 1) * 512],
                in0=pa_t[hh][0:DK, :],
                in1=recB[0][:, hh * 512:(hh + 1) * 512])

        for hh in range(2):
            work.append(lambda hh=hh: start_head(hh))
            for kb in range(nkb):
                work.append(lambda hh=hh, kb=kb: mm_head(hh, kb))
            work.append(lambda hh=hh: evict_den(hh))
        work.append(recip_bcast)
        work.append(lambda: mul_head(0))
        work.append(lambda: mul_head(1))
        return work

    y_box = {}

    def make_w1_work(rc):
        work = []

        def w1_block(rb, nt, box):
            c0 = rc * 512 + rb * P
            if nt == 0:
                box[0] = ph2w.tile([P, D], BF16, name="a1p", tag="a1p",
                                   bufs=2)
            psw = px_tile()
            for kc2 in range(2):
                nc.tensor.matmul(psw[:],
                                 lhsT=catT[kc2][:, c0:c0 + P],
                                 rhs=w1_sb[:, kc2, nt * 512:(nt + 1) * 512],
                                 start=(kc2 == 0), stop=(kc2 == 1))
            nc.vector.tensor_copy(out=box[0][:, nt * 512:(nt + 1) * 512],
                                  in_=psw[:])
            if nt == 1:
                nc.gpsimd.dma_start(out=rs_in[rc][rb * P:(rb + 1) * P, :],
                                    in_=box[0][:])

        for rb in range(4):
            box = [None]
            for nt in range(2):
                work.append(lambda rb=rb, nt=nt, box=box: w1_block(rb, nt, box))

        def do_rs():
            nc.gpsimd.collective_compute(
                "ReduceScatter", ALU.add, replica_groups=RG,
                ins=[rs_in[rc][:].opt()], outs=[rs_out[rc][:].opt()])
            # y residual rows for this chunk (sync queue, no waits)
            yb = ph3.tile([P, D], F32, name="y_sb", tag="y_sb", bufs=1)
            nc.sync.dma_start(out=yb[:],
                              in_=d["y_rows"][rc * P:(rc + 1) * P, :])
            y_box[rc] = yb
            # RS result lands via sync queue (gpsimd must stay unblocked)
            ap = ph3.tile([P, D], BF16, name="a1pre", tag="a1pre", bufs=2)
            nc.sync.dma_start(out=ap[:], in_=rs_out[rc][:])
            a1pre_box[rc] = ap
        work.append(do_rs)
        return work

    def make_ln1_work(rc):
        work = []
        a1 = [None]
        fold = f["ln1_unit_g"] and f["ln1_zero_b"]

        def residual():
            a1[0] = ph3.tile([P, D], F32, name="a1", tag="a1", bufs=1)
            nc.vector.tensor_add(out=a1[0][:], in0=a1pre_box[rc][:],
                                 in1=y_box[rc][:])
            if not f["b1_zero"]:
                nc.vector.tensor_add(out=a1[0][:], in0=a1[0][:],
                                     in1=f["b1_b"][:])

        a1b = [None]

        def ln():
            a1b[0] = ph3.tile([P, D], BF16, name="a1b", tag="a1b", bufs=1)
            if fold:
                # rstd folds SCALE -> a1T comes out pre-scaled
                ln_slim(ph3, a1[0][:], a1b[0][:], eps_s2,
                        var_scale=1.0 / (SCALE * SCALE))
            else:
                ln_slim(ph3, a1[0][:], a1b[0][:], eps_sb,
                        g_b=None if f["ln1_unit_g"] else f["g1_b"][:],
                        be_b=None if f["ln1_zero_b"] else f["be1_b"][:])

        def tr(kc):
            pt = px_tile()[:].bitcast(BF16)[:, 0:P]   # bf16 view of psum
            nc.tensor.transpose(pt, a1b[0][:, kc * P:(kc + 1) * P],
                                ident[:])
            if fold:
                nc.vector.tensor_copy(out=a1T[kc][:, rc * P:(rc + 1) * P],
                                      in_=pt)
            else:
                nc.vector.tensor_scalar(
                    out=a1T[kc][:, rc * P:(rc + 1) * P], in0=pt,
                    scalar1=float(SCALE), scalar2=None, op0=ALU.mult)

        work.append(residual)
        work.append(ln)
        work.extend(lambda kc=kc: tr(kc) for kc in range(KC))
        return work

    ph1_closed = False
    ln1_q = []
    for rc in (3, 2, 1, 0):
        nkb = 4 * rc + 4
        dn = 7 if rc == 3 else 4
        for p in range(2):
            # expP[kb][i] holds exp(scores) for head 2p+i, keys block kb
            expP = ph2w.tile([P, TB, 2, 512], BF16, name="expP", tag="expP",
                             bufs=2)
            for kb in range(nkb):
                ps = psS.tile([P, 1024], F32, name="ps_sc", tag="ps_sc",
                              bufs=2)
                nc.tensor.matmul(ps[:, 0:512],
                                 lhsT=kTp[p][0:DK, kb * P:(kb + 1) * P],
                                 rhs=qTp[p][0:DK, rc * 512:(rc + 1) * 512],
                                 start=True, stop=True,
                                 tile_position=(0, 0))
                nc.tensor.matmul(ps[:, 512:1024],
                                 lhsT=kTp[p][DK:P, kb * P:(kb + 1) * P],
                                 rhs=qTp[p][DK:P, rc * 512:(rc + 1) * 512],
                                 start=True, stop=True,
                                 tile_position=(64, 0))
                nc.scalar.activation(
                    out=expP[:, kb, :, :],
                    in_=ps[:].rearrange("p (h r) -> p h r", h=2),
                    func=AF.Exp)
                if kb >= 4 * rc:       # diagonal block: apply causal mask
                    i = kb - 4 * rc
                    for hh in range(2):
                        nc.vector.tensor_mul(out=expP[:, kb, hh, :],
                                             in0=expP[:, kb, hh, :],
                                             in1=mask4[:, i, :])
                drain(dn)
            pending.extend(make_attn_work(rc, p, expP))
        # LN1 for chunk rc is emitted TWO chunks later: its first DVE op
        # waits on that chunk's RS (~30-40us after the trigger), and any
        # earlier emission stalls the whole in-order DVE queue behind the
        # collective
        if len(ln1_q) >= 2:
            pending.extend(ln1_q.pop(0))
        pending.extend(make_w1_work(rc))
        ln1_q.append(make_ln1_work(rc))
        if not ph1_closed:
            # all q/v pending work must be emitted before ph1 frees
            ph1_closed = True
            drain(max(0, ph1_work_n - drained[0]))
            close_pool(ph1)
    for w in ln1_q:              # LN1 for rc=1 and rc=0
        pending.extend(w)
    drain(len(pending))
    close_pool(psV)
    close_pool(psS)
    close_pool(ph3)
    close_pool(cat)
    close_pool(ph2w)
    close_pool(psX)
    close_pool(attn)

    # ================= Phase 4: cross-attention =========================
    fw = open_pool(name="fw", bufs=1, side="left")        # Wf1, lives to FFN1
    wf1_all = fw.tile([P, KC, DF], BF16, name="wf1", tag="wf1")
    at2p = open_pool(name="at2p", bufs=1, side="left")    # at2T, into ph5
    at2T = [at2p.tile([P, R], BF16, name=f"at2T{i}", tag=f"at2T{i}")
            for i in range(KC)]
    w2p = open_pool(name="w2p", bufs=1, side="left")      # W2sum, into ph5
    w2 = [w2p.tile([P, D], BF16, name=f"w2_{i}", tag=f"w2_{i}")
          for i in range(KC)]
    ph4 = open_pool(name="ph4", bufs=1, side="left")
    pp4 = open_pool(name="pp4", bufs=4, space="PSUM", side="left")
    pd4 = open_pool(name="pd4", bufs=1, space="PSUM", side="left")
    for kc in range(4, KC):
        xT[kc] = ph4.tile([P, T], BF16, name=f"xT{kc}", tag=f"xT{kc}")
        nc.sync.dma_start(out=xT[kc][:],
                          in_=d["xT"][kc * P:(kc + 1) * P, :])
    # Wf1/W2sum ride the sync queue BEHIND the rc=0 RS-result load, so
    # their 10MB never contends with the last ReduceScatter's network
    nc.sync.dma_start(out=wf1_all[:],
                      in_=d["Wf1"].rearrange("p (c f) -> p c f", c=KC))
    for kc in range(KC):
        nc.sync.dma_start(out=w2[kc][:],
                          in_=d["W2sum"][kc * P:(kc + 1) * P, :])
    # x_tm: single strided DMA (one descriptor on the scalar queue)
    x_tm = ph4.tile([P, TB, D], BF16, name="xtm", tag="xtm")
    nc.scalar.dma_start(out=x_tm[:],
                        in_=d["x_tm"].rearrange("(t p) d -> p t d", p=P))

    p2T = [ph4.tile([P, R], BF16, name=f"p2T{i}", tag=f"p2T{i}")
           for i in range(TB)]
    for tb in range(TB):
        ps = pp4.tile([P, 512], F32, name="ps4", tag="ps4")
        for kc in range(KC):
            nc.tensor.matmul(ps[:], lhsT=xT[kc][:, tb * P:(tb + 1) * P],
                             rhs=a1T[kc][:, :],
                             start=(kc == 0), stop=(kc == KC - 1))
        nc.scalar.activation(out=p2T[tb][:], in_=ps[:], func=AF.Exp)
    # denominator: 4 col-tiled ones-matmul accumulators run concurrently
    pd = pd4.tile([P, R], F32, name="ps_d2", tag="ps_d2")
    for g in range(4):
        for u in range(4):
            tb = 4 * g + u
            nc.tensor.matmul(pd[32 * g:32 * g + 1, :], lhsT=ones_col[:],
                             rhs=p2T[tb][:], start=(u == 0), stop=(u == 3),
                             tile_position=(0, 32 * g))
    den4 = ph4.tile([1, 4, R], F32, name="den4", tag="den4")
    for g in range(4):
        nc.vector.tensor_copy(out=den4[:, g, :], in_=pd[32 * g:32 * g + 1, :])
    den2a = ph4.tile([1, R], F32, name="den2a", tag="den2a")
    den2b = ph4.tile([1, R], F32, name="den2b", tag="den2b")
    den2 = ph4.tile([1, R], F32, name="den2", tag="den2")
    nc.vector.tensor_add(out=den2a[:], in0=den4[:, 0, :], in1=den4[:, 1, :])
    nc.vector.tensor_add(out=den2b[:], in0=den4[:, 2, :], in1=den4[:, 3, :])
    nc.vector.tensor_add(out=den2[:], in0=den2a[:], in1=den2b[:])
    recip2f = ph4.tile([1, R], F32, name="recip2f", tag="recip2f")
    nc.vector.reciprocal_approx_fast(recip2f[:], den2[:])
    recip2 = ph4.tile([1, R], BF16, name="recip2", tag="recip2")
    nc.vector.tensor_copy(out=recip2[:], in_=recip2f[:])
    psb2 = pd4.tile([P, R], F32, name="psb2", tag="psb2")
    nc.tensor.matmul(psb2[:], lhsT=ones_row[:], rhs=recip2[:],
                     start=True, stop=True)
    recip2b = ph4.tile([P, R], F32, name="recip2b", tag="recip2b")
    nc.vector.tensor_copy(out=recip2b[:], in_=psb2[:])
    for db in range(KC):
        ps = pp4.tile([P, 512], F32, name="ps4", tag="ps4")
        for tb in range(TB):
            nc.tensor.matmul(ps[:], lhsT=x_tm[:, tb, db * P:(db + 1) * P],
                             rhs=p2T[tb][:],
                             start=(tb == 0), stop=(tb == TB - 1))
        nc.vector.tensor_mul(out=at2T[db][:], in0=ps[:], in1=recip2b[:])
    close_pool(pd4)
    close_pool(pp4)
    close_pool(ph4)
    close_pool(a1pl)
    close_pool(xpre)

    # ========= Phase 5: W2sum + residual + LN2, produce a2T =============
    a2p = open_pool(name="a2p", bufs=1, side="right")     # a2T into ph6
    a2T = [a2p.tile([P, R], BF16, name=f"a2T{i}", tag=f"a2T{i}")
           for i in range(KC)]
    ph5 = open_pool(name="ph5", bufs=1, side="right")
    pp5 = open_pool(name="pp5", bufs=4, space="PSUM", side="left")
    pt5 = open_pool(name="pt5", bufs=2, space="PSUM", side="left")
    if not f["b2_zero"]:
        f["b2_b"] = bcast_row(ph5, "b2")
    if not f["ln2_unit_g"]:
        f["g2_b"] = bcast_row(ph5, "ln2_g")
    if not f["ln2_zero_b"]:
        f["be2_b"] = bcast_row(ph5, "ln2_b")
    for rb in range(RB):
        y5 = ph5.tile([P, D], F32, name="y5", tag="y5", bufs=2)
        nc.sync.dma_start(out=y5[:], in_=d["y_rows"][rb * P:(rb + 1) * P, :])
        a2 = ph5.tile([P, D], F32, name="a2", tag="a2", bufs=2)
        for nt in range(2):
            ps = pp5.tile([P, 512], F32, name="ps_a2", tag="ps_a2")
            for kc in range(KC):
                nc.tensor.matmul(ps[:],
                                 lhsT=at2T[kc][:, rb * P:(rb + 1) * P],
                                 rhs=w2[kc][:, nt * 512:(nt + 1) * 512],
                                 start=(kc == 0), stop=(kc == KC - 1))
            sl = slice(nt * 512, (nt + 1) * 512)
            nc.vector.tensor_add(out=a2[:, sl], in0=ps[:], in1=y5[:, sl])
            if not f["b2_zero"]:
                nc.vector.tensor_add(out=a2[:, sl], in0=a2[:, sl],
                                     in1=f["b2_b"][:, sl])
        a2b = ph5.tile([P, D], BF16, name="a2b", tag="a2b", bufs=2)
        ln_slim(ph5, a2[:], a2b[:], eps_sb,
                g_b=None if f["ln2_unit_g"] else f["g2_b"][:],
                be_b=None if f["ln2_zero_b"] else f["be2_b"][:])
        for kc in range(KC):
            pt = pt5.tile([P, P], BF16, name="pt_a2", tag="pt_a2")
            nc.tensor.transpose(pt[:], a2b[:, kc * P:(kc + 1) * P], ident[:])
            nc.vector.tensor_copy(out=a2T[kc][:, rb * P:(rb + 1) * P],
                                  in_=pt[:])
    close_pool(pt5)
    close_pool(pp5)
    close_pool(ph5)
    close_pool(w2p)
    close_pool(at2p)

    # ========== Phase 6: FFN + residual + LN3 ===========================
    fA = open_pool(name="fA", bufs=1, side="right")
    f1T = [fA.tile([P, R], BF16, name=f"f1T{i}", tag=f"f1T{i}")
           for i in range(FB)]
    bf1_sb = bias_chunks(fA, "bf1", FB)
    pfA = open_pool(name="pfA", bufs=3, space="PSUM", side="left")
    for fb in range(FB):
        ps = pfA.tile([P, 512], F32, name="ps_f1", tag="ps_f1")
        for kc in range(KC):
            nc.tensor.matmul(ps[:], lhsT=wf1_all[:, kc, fb * P:(fb + 1) * P],
                             rhs=a2T[kc][:, :],
                             start=(kc == 0), stop=(kc == KC - 1))
        # relu + bias on ACT (idle during the FFN)
        nc.scalar.activation(out=f1T[fb][:], in_=ps[:], func=AF.Relu,
                             bias=bf1_sb[:, fb:fb + 1], scale=1.0)
    close_pool(pfA)
    close_pool(fw)

    pfB = open_pool(name="pfB", bufs=1, space="PSUM", side="left")
    fB = open_pool(name="fB", bufs=1, side="right")
    ps_rb = [pfB.tile([P, D], F32, name=f"ps_rb{i}", tag=f"ps_rb{i}")
             for i in range(RB)]
    y6 = [fB.tile([P, D], F32, name=f"y6{i}", tag=f"y6{i}")
          for i in range(RB)]
    for rb in range(RB):
        nc.sync.dma_start(out=y6[rb][:],
                          in_=d["y_rows"][rb * P:(rb + 1) * P, :])
    if not f["bf2_zero"]:
        f["bf2_b"] = bcast_row(fB, "bf2")
    if not f["ln3_unit_g"]:
        f["g3_b"] = bcast_row(fB, "ln3_g")
    if not f["ln3_zero_b"]:
        f["be3_b"] = bcast_row(fB, "ln3_b")
    wf2_t = {}
    for fb in range(FB):
        wf2_fb = fB.tile([P, D], BF16, name="wf2s", tag="wf2s", bufs=6)
        nc.sync.dma_start(out=wf2_fb[:], in_=d["Wf2"][fb * P:(fb + 1) * P, :])
        wf2_t[fb] = wf2_fb
        if fb < FB - 4:
            for rb in range(RB):
                for nt in range(2):
                    nc.tensor.matmul(
                        ps_rb[rb][:, nt * 512:(nt + 1) * 512],
                        lhsT=f1T[fb][:, rb * P:(rb + 1) * P],
                        rhs=wf2_fb[:, nt * 512:(nt + 1) * 512],
                        start=(fb == 0), stop=False)

    def tail(rb):
        ff = fB.tile([P, D], F32, name="ff", tag="ff", bufs=2)
        nc.vector.tensor_add(out=ff[:], in0=ps_rb[rb][:], in1=y6[rb][:])
        if not f["bf2_zero"]:
            nc.vector.tensor_add(out=ff[:], in0=ff[:], in1=f["bf2_b"][:])
        o = fB.tile([P, D], F32, name="o", tag="o", bufs=2)
        ln_slim(fB, ff[:], o[:], eps_sb,
                g_b=None if f["ln3_unit_g"] else f["g3_b"][:],
                be_b=None if f["ln3_zero_b"] else f["be3_b"][:])
        nc.sync.dma_start(out=out_d[rb * P:(rb + 1) * P, :], in_=o[:])

    # last 4 fb row-major: each row block finishes early and its LN3+store
    # overlaps the remaining matmuls
    for rb in range(RB):
        for fb in range(FB - 4, FB):
            for nt in range(2):
                nc.tensor.matmul(ps_rb[rb][:, nt * 512:(nt + 1) * 512],
                                 lhsT=f1T[fb][:, rb * P:(rb + 1) * P],
                                 rhs=wf2_t[fb][:, nt * 512:(nt + 1) * 512],
                                 start=False, stop=(fb == FB - 1))
        tail(rb)
    close_pool(fB)
    close_pool(pfB)
    close_pool(fA)
    close_pool(a2p)
    close_pool(dramp)
    close_pool(const)


def _row_idx(j):
    return np.concatenate(
        [np.arange(512 * rc + 128 * j, 512 * rc + 128 * j + 128)
         for rc in range(4)])


def _flags(inputs):
    z = lambda a: bool(np.all(np.asarray(a) == 0.0))
    u = lambda a: bool(np.all(np.asarray(a) == 1.0))
    return {
        "b1_zero": z(inputs["b1"]), "b2_zero": z(inputs["b2"]),
        "bf2_zero": z(inputs["bf2"]),
        "ln1_unit_g": u(inputs["ln1_g"]), "ln1_zero_b": z(inputs["ln1_b"]),
        "ln2_unit_g": u(inputs["ln2_g"]), "ln2_zero_b": z(inputs["ln2_b"]),
        "ln3_unit_g": u(inputs["ln3_g"]), "ln3_zero_b": z(inputs["ln3_b"]),
    }


def _prep_host(inputs):
    f32 = lambda a: np.ascontiguousarray(np.asarray(a, np.float32))
    bf = lambda a: np.ascontiguousarray(
        np.asarray(a, np.float32).astype(ml_dtypes.bfloat16))
    x = f32(inputs["x"])
    y = f32(inputs["y"])
    mask = np.asarray(inputs["y_mask"]).astype(np.float32)
    # diagonal-block masks: mask4[ky, i, r] = mask[r, 128*i + ky]
    m4 = mask[0:512, 0:512].reshape(512, 4, 128).transpose(2, 1, 0)
    Wq = f32(inputs["Wq"])   # [H, D, DK]
    Wk = f32(inputs["Wk"])
    Wv = f32(inputs["Wv"])

    def chunkP(a):
        """[C*P, F] -> [P, C*F] so each partition's data is contiguous."""
        cp, fdim = a.shape
        return np.ascontiguousarray(
            a.reshape(cp // P, P, fdim).transpose(1, 0, 2).reshape(P, -1))

    shared = {
        "mask4": bf(m4),
        "b1": f32(inputs["b1"]),
        "ln1_g": f32(inputs["ln1_g"]), "ln1_b": f32(inputs["ln1_b"]),
        "W2sum": bf(f32(inputs["W2"]).reshape(H, D, D).sum(0)),
        "b2": f32(inputs["b2"]),
        "ln2_g": f32(inputs["ln2_g"]), "ln2_b": f32(inputs["ln2_b"]),
        "Wf1": chunkP(bf(inputs["Wf1"])),
        "bf1": f32(inputs["bf1"]),
        "Wf2": bf(inputs["Wf2"]),
        "bf2": f32(inputs["bf2"]),
        "ln3_g": f32(inputs["ln3_g"]), "ln3_b": f32(inputs["ln3_b"]),
    }
    in_maps = []
    for c in range(NCORES):
        b, j = c // 4, c % 4
        hh = slice(4 * j, 4 * j + 4)
        ridx = _row_idx(j)
        in_maps.append({
            "yT": bf(y[b].T),
            "wq": chunkP(bf(Wq[hh].transpose(1, 0, 2).reshape(D, 256) * SCALE)),
            "wk": chunkP(bf(Wk[hh].transpose(1, 0, 2).reshape(D, 256))),
            "wv": chunkP(bf(Wv[hh].transpose(1, 0, 2).reshape(D, 256))),
            "bq_s": f32(inputs["bq"])[hh].reshape(256) * np.float32(SCALE),
            "bk_f": f32(inputs["bk"])[hh].reshape(256),
            "bv_f": f32(inputs["bv"])[hh].reshape(256),
            "w1loc": chunkP(bf(f32(inputs["W1"])[256 * j:256 * (j + 1), :])),
            "y_rows": np.ascontiguousarray(y[b][ridx]),
            "xT": bf(x[b].T),
            "x_tm": bf(x[b]),
            **shared,
        })
    return in_maps


def kernel(**inputs):
    fl = _flags(inputs)
    key = tuple(sorted(fl.items()))
    if key not in _cached:
        _cached[key] = build_nc(dict(fl))
    nc = _cached[key]
    in_maps = _prep_host(inputs)
    res = run_bass_kernel_spmd(nc, in_maps, core_ids=list(range(NCORES)))
    out = np.zeros((B, S, D), np.float32)
    for c in range(NCORES):
        b, j = c // 4, c % 4
        out[b, _row_idx(j)] = res.results[c]["out"]
    return out


# revision 61
# speedup vs baseline: 1.1624x; 1.0857x over previous
"""Trainium2 Bass kernel for nn_DecoderLayer_33758442946809.

Sharding (8 cores = 2 batches x 4-core groups):
- Self-attention is HEAD-sharded: core (b, j) computes heads 4j..4j+3 for
  all T=2048 rows of batch b; causal skipping is SPMD-uniform (only
  lower-triangle key blocks are scored/exp'd).
- W1 is row-parallel over the head-sharded cat features; partials are
  summed with a chunked ReduceScatter (bf16) over each 4-core group.
  After the RS, core (b, j) owns the strided row set
  {512*rc + 128*j + i : rc<4, i<128}; cross-attention, W2 and the FFN
  are data-parallel over those rows.
- tile(attn2, H) @ W2 == attn2 @ sum_h W2[h] (host precomputes the sum).

v2 schedule (vs the phase-serial v1):
- Row chunks processed big-first [3,2,1,0] so the last ReduceScatter has
  the shortest dependency tail.
- Score PSUM is [128,1024] tiles (1 key block, both heads) with bufs=2,
  so ACT exp streams without stalling on PSUM WAR.
- Softmax normalization: denominator row is reciprocal'd at [1,2,512]
  (cheap) then partition-broadcast; the numerator is multiplied straight
  out of PSUM -- no [65,512] evictions, no 6.5us [64,1024] reciprocals.
- QKV projection matmuls (q tcols 2..0, all of v) drain as pending PE
  work under the first row-chunk's exp stream.
- RS outputs land via the sync queue (the gpsimd queue used to block
  ~90us on the RS-done semaphore).
- LN affine ops are skipped when gamma==1/beta==0 (host-checked program
  variant); the attention 1/sqrt(dk) scale is folded into LN1's rstd.
- Transposes run in bf16 (pre-cast) -- 1 PE cycle/row instead of 2.
- xT / x_tm / Wf1 / W2sum are prefetched a phase early.
- FFN1 relu+bias runs on ACT (idle there) instead of DVE.
- FFN2's last 4 weight blocks run row-major so row block 0 finishes
  ~6us early and the final LN3+store overlaps the remaining matmuls.
"""
import math
import sys

import numpy as np

sys.path.insert(0, "/opt/trn_rl_repo")

import ml_dtypes  # noqa: E402

import concourse.bass as bass  # noqa: E402
import concourse.tile as tile  # noqa: E402
from concourse import bacc, mybir  # noqa: E402
from concourse.bass_utils import run_bass_kernel_spmd  # noqa: E402
from concourse.masks import make_identity  # noqa: E402

B, S, D, H, DF = 2, 2048, 1024, 16, 4096
DK = D // H                      # 64
P = 128
T = S                            # rows/keys per batch
R = 512                          # own rows per core (after RS)
KC = D // P                      # 8 contraction chunks of D
TB = T // P                      # 16 key blocks
RB = R // P                      # 4 row blocks
FB = DF // P                     # 32 ffn blocks
NCORES = 8
HL = 4                           # local heads per core
SCALE = 1.0 / math.sqrt(DK)
RG = [[0, 1, 2, 3], [4, 5, 6, 7]]

F32 = mybir.dt.float32
BF16 = mybir.dt.bfloat16
AF = mybir.ActivationFunctionType
ALU = mybir.AluOpType

_cached = {}


def build_nc(f):
    nc = bacc.Bacc("TRN2", target_bir_lowering=False, debug=False,
                   num_devices=NCORES)

    dram = {}

    def din(name, shape, dt):
        dram[name] = nc.dram_tensor(name, shape, dt, kind="ExternalInput").ap()

    din("yT", [P, 4 * KC * 512], BF16)   # y[b].T, [p][tcol][kc][c] chunks
    din("wq", [P, KC * HL * DK], BF16)   # pre-chunked [p][kc][f] layout
    din("wk", [P, KC * HL * DK], BF16)
    din("wv", [P, KC * HL * DK], BF16)
    din("bq_s", [HL * DK], F32)      # bq * SCALE, local heads
    din("bk_f", [HL * DK], F32)
    din("bv_f", [HL * DK], F32)
    din("mask4", [P, 4, R], BF16)    # diagonal-block masks (key, i, row)
    din("w1loc", [P, 2 * D], BF16)   # W1 rows owned by this core, pre-chunked
    din("b1", [D], F32)
    din("ln1_g", [D], F32)
    din("ln1_b", [D], F32)
    din("y_rows", [R, D], F32)       # this core's (strided) y rows
    din("xT", [D, T], BF16)
    din("x_tm", [T, D], BF16)
    din("W2sum", [D, D], BF16)
    din("b2", [D], F32)
    din("ln2_g", [D], F32)
    din("ln2_b", [D], F32)
    din("Wf1", [P, KC * DF], BF16)   # pre-chunked [p][kc][f] layout
    din("bf1", [DF], F32)
    din("Wf2", [DF, D], BF16)
    din("bf2", [D], F32)
    din("ln3_g", [D], F32)
    din("ln3_b", [D], F32)
    out_d = nc.dram_tensor("out", [R, D], F32, kind="ExternalOutput").ap()

    with tile.TileContext(nc) as tc:
        _build(nc, tc, dram, out_d, f)
    nc.compile()
    return nc


def _build(nc, tc, d, out_d, f):
    pool_cms = {}

    def open_pool(*args, **kw):
        cm = tc.tile_pool(*args, **kw)
        p = cm.__enter__()
        pool_cms[id(p)] = cm
        return p

    def close_pool(p):
        pool_cms.pop(id(p)).__exit__(None, None, None)

    const = open_pool(name="const", bufs=1, side="left")
    ident = const.tile([P, P], BF16, name="ident", tag="ident")
    make_identity(nc, ident[:])
    ones_col = const.tile([P, 1], BF16, name="ones_col", tag="ones_col")
    nc.vector.memset(ones_col[:], 1.0)
    ones_row = const.tile([1, P], BF16, name="ones_row", tag="ones_row")
    nc.vector.memset(ones_row[:], 1.0)
    ones64 = const.tile([1, DK], BF16, name="ones64", tag="ones64")
    nc.vector.memset(ones64[:], 1.0)
    eps_sb = const.tile([P, 1], F32, name="eps", tag="eps")
    nc.vector.memset(eps_sb[:], 1e-5)
    # eps/SCALE^2: sqrt(var/SCALE^2 + eps/SCALE^2) = sqrt(var+eps)/SCALE,
    # so LN1's rstd comes out pre-multiplied by the attention scale.
    eps_s2 = const.tile([P, 1], F32, name="eps_s2", tag="eps_s2")
    nc.vector.memset(eps_s2[:], 1e-5 / (SCALE * SCALE))

    def bias_chunks(pool, name, n):
        t = pool.tile([P, n], F32, name=f"bc_{name}", tag=f"bc_{name}")
        nc.sync.dma_start(out=t[:], in_=d[name].rearrange("(n p) -> p n", p=P))
        return t

    def bcast_row(pool, name):
        src = d[name]
        t = pool.tile([P, D], F32, name=f"br_{name}", tag=f"br_{name}")
        bc = bass.AP(tensor=src.tensor, offset=src.offset,
                     ap=[[0, P]] + list(src.ap))
        nc.sync.dma_start(out=t[:], in_=bc)
        return t

    def ln_slim(pool, x_ap, out_ap, eps_ap, var_scale=1.0, g_b=None,
                be_b=None):
        """LayerNorm along the free axis (D) of a token-major [128, D]
        f32 tile into out_ap. var_scale folds a constant into rstd."""
        x3 = x_ap.rearrange("p (n f) -> p n f", f=512)
        stats = pool.tile([P, 2, 6], F32, name="ln_stats", tag="ln_stats",
                          bufs=4)
        for sg in range(2):
            nc.vector.bn_stats(out=stats[:, sg, :], in_=x3[:, sg, :])
        mv = pool.tile([P, 2], F32, name="ln_mv", tag="ln_mv", bufs=4)
        nc.vector.bn_aggr(out=mv[:], in_=stats[:])
        std = pool.tile([P, 1], F32, name="ln_std", tag="ln_std", bufs=4)
        nc.scalar.activation(out=std[:], in_=mv[:, 1:2], func=AF.Sqrt,
                             bias=eps_ap[:], scale=var_scale)
        rstd = pool.tile([P, 1], F32, name="ln_rstd", tag="ln_rstd", bufs=4)
        nc.vector.reciprocal(out=rstd[:], in_=std[:])
        nc.vector.tensor_scalar(out=out_ap, in0=x_ap, scalar1=mv[:, 0:1],
                                scalar2=rstd[:], op0=ALU.subtract,
                                op1=ALU.mult)
        if g_b is not None:
            nc.vector.tensor_mul(out=out_ap, in0=out_ap, in1=g_b)
        if be_b is not None:
            nc.vector.tensor_add(out=out_ap, in0=out_ap, in1=be_b)

    # ======== pools whose tiles live into ph4 (right-side bottom) =======
    xpre = open_pool(name="xpre", bufs=1, side="right")
    # only half of xT is prefetched (SBUF is tight during rc=3);
    # xT[4..7] load at ph4 open and are the last kcs of each score group
    xT = [xpre.tile([P, T], BF16, name=f"xT{i}", tag=f"xT{i}")
          if i < 4 else None for i in range(KC)]
    a1pl = open_pool(name="a1pl", bufs=1, side="right")
    a1T = [a1pl.tile([P, R], BF16, name=f"a1T{i}", tag=f"a1T{i}")
           for i in range(KC)]

    # ===================== input DMAs (spread across queues) ============
    # ph3 opens below attn/cat on the right so those can close at the
    # ph2->ph4 boundary while LN1(rc=0) still runs out of ph3
    ph3 = open_pool(name="ph3", bufs=1, side="right")     # LN1 working
    a1pre_box = {}
    if not f["b1_zero"]:
        f["b1_b"] = bcast_row(ph3, "b1")
    if not f["ln1_unit_g"]:
        f["g1_b"] = bcast_row(ph3, "ln1_g")
    if not f["ln1_zero_b"]:
        f["be1_b"] = bcast_row(ph3, "ln1_b")

    attn = open_pool(name="attn", bufs=1, side="right")  # live through ph2
    qTp = [attn.tile([P, T], BF16, name=f"qTp{i}", tag=f"qTp{i}")
           for i in range(2)]
    kTp = [attn.tile([P, T], BF16, name=f"kTp{i}", tag=f"kTp{i}")
           for i in range(2)]
    v_sb = [attn.tile([P, HL, DK + 1], BF16, name=f"v{i}", tag=f"v{i}")
            for i in range(TB)]
    mask4 = attn.tile([P, 4, R], BF16, name="mask4", tag="mask4")

    ph2w = open_pool(name="ph2w", bufs=1, side="left")   # ph2 working set
    ph1 = open_pool(name="ph1", bufs=1, side="left")     # closed after rc=3
    # yT lands as 4 tcol-major column slabs (host pre-chunked so each
    # partition reads one contiguous 8KB line per slab): the first
    # k-projection group can start after ~1MB instead of the full 4MB
    yT_all = ph1.tile([P, 4, KC, 512], BF16, name="yT", tag="yT")
    wq_sb = ph1.tile([P, KC, 2 * P], BF16, name="wq", tag="wq")
    wk_sb = ph1.tile([P, KC, 2 * P], BF16, name="wk", tag="wk")
    wv_sb = ph1.tile([P, KC, 2 * P], BF16, name="wv", tag="wv")
    yTsrc = d["yT"].rearrange("p (a c r) -> p a c r", a=4, c=KC)
    for tcol in range(4):
        nc.sync.dma_start(out=yT_all[:, tcol, :, :], in_=yTsrc[:, tcol, :, :])

    def yT_cols(kc, lo, n):
        """columns [lo, lo+n) of the logical [P, KC, T] yT (n <= 512)"""
        tcol, c = lo // 512, lo % 512
        return yT_all[:, tcol, kc, c:c + n]
    nc.scalar.dma_start(out=wk_sb[:],
                        in_=d["wk"].rearrange("p (c f) -> p c f", c=KC))
    nc.scalar.dma_start(out=wq_sb[:],
                        in_=d["wq"].rearrange("p (c f) -> p c f", c=KC))
    nc.scalar.dma_start(out=wv_sb[:],
                        in_=d["wv"].rearrange("p (c f) -> p c f", c=KC))
    nc.gpsimd.dma_start(out=mask4[:], in_=d["mask4"][:])
    bq_sb = bias_chunks(ph1, "bq_s", 2)
    bk_sb = bias_chunks(ph1, "bk_f", 2)
    bv_b = ph1.tile([P, 2 * P], F32, name="bv_b", tag="bv_b")
    bv_src = d["bv_f"]
    nc.sync.dma_start(out=bv_b[:], in_=bass.AP(
        tensor=bv_src.tensor, offset=bv_src.offset,
        ap=[[0, P]] + list(bv_src.ap)))
    # cross-attention keys prefetched behind yT on the sync queue
    for kc in range(4):
        nc.sync.dma_start(out=xT[kc][:], in_=d["xT"][kc * P:(kc + 1) * P, :])

    # ============ ph1 QKV emission (k full, q tcol 3; rest pending) =====
    # psX: shared 1-bank [P,512] ring for q/v projections, W1 partials and
    # LN1 transposes (keeps psS at bufs=2 within the 8-bank budget)
    psX = open_pool(name="psX", bufs=1, space="PSUM", side="left")
    pending = []
    drained = [0]

    def drain(k):
        for _ in range(min(k, len(pending))):
            pending.pop(0)()
            drained[0] += 1

    def px_tile():
        return psX.tile([P, 512], F32, name="px", tag="px", bufs=2)

    def qk_group(dst, w_sb, b_sb, p, tcol):
        ps = px_tile()
        for kc in range(KC):
            nc.tensor.matmul(ps[:], lhsT=w_sb[:, kc, p * P:(p + 1) * P],
                             rhs=yT_all[:, tcol, kc, :],
                             start=(kc == 0), stop=(kc == KC - 1))
        nc.vector.tensor_scalar(out=dst[p][:, tcol * 512:(tcol + 1) * 512],
                                in0=ps[:], scalar1=b_sb[:, p:p + 1],
                                scalar2=None, op0=ALU.add)

    # k: all tcols (every score block needs all keys); q: tcol 3 first
    for p in range(2):
        for tcol in range(4):
            qk_group(kTp, wk_sb, bk_sb, p, tcol)
    for p in range(2):
        qk_group(qTp, wq_sb, bq_sb, p, 3)

    def v_work(tb):
        work = []
        box = [None]

        def v_start():
            nc.vector.memset(v_sb[tb][:, :, DK:DK + 1], 1.0)
            box[0] = px_tile()

        def v_mm(kc):
            nc.tensor.matmul(box[0][:, 0:2 * P],
                             lhsT=yT_cols(kc, tb * P, P),
                             rhs=wv_sb[:, kc, :],
                             start=(kc == 0), stop=(kc == KC - 1))

        def v_evict():
            nc.vector.tensor_add(
                out=v_sb[tb][:, :, 0:DK],
                in0=box[0][:, 0:2 * P].rearrange("p (h k) -> p h k", h=HL),
                in1=bv_b[:].rearrange("p (h k) -> p h k", h=HL))

        work.append(v_start)
        work.extend(lambda kc=kc: v_mm(kc) for kc in range(KC))
        work.append(v_evict)
        return work

    def q_work(p, tcol):
        work = []
        box = [None]

        def q_start():
            box[0] = px_tile()

        def q_mm(kc):
            nc.tensor.matmul(box[0][:],
                             lhsT=wq_sb[:, kc, p * P:(p + 1) * P],
                             rhs=yT_all[:, tcol, kc, :],
                             start=(kc == 0), stop=(kc == KC - 1))

        def q_evict():
            nc.vector.tensor_scalar(
                out=qTp[p][:, tcol * 512:(tcol + 1) * 512],
                in0=box[0][:], scalar1=bq_sb[:, p:p + 1],
                scalar2=None, op0=ALU.add)

        work.append(q_start)
        work.extend(lambda kc=kc: q_mm(kc) for kc in range(KC))
        work.append(q_evict)
        return work

    # v must be fully projected before the first attnV drains; emit v
    # first, then the remaining q columns.
    for tb in range(TB):
        pending.extend(v_work(tb))
    for tcol in (2, 1, 0):
        for p in range(2):
            pending.extend(q_work(p, tcol))
    ph1_work_n = len(pending)

    # ============ ph2: causal attention + W1 + ReduceScatter ============
    cat = open_pool(name="cat", bufs=1, side="right")     # catT, ph2-long
    catT = [cat.tile([P, T], BF16, name=f"catT{i}", tag=f"catT{i}")
            for i in range(2)]
    w1_sb = cat.tile([P, 2, D], BF16, name="w1", tag="w1")
    nc.gpsimd.dma_start(out=w1_sb[:],
                        in_=d["w1loc"].rearrange("p (c n) -> p c n", c=2))

    dramp = open_pool(name="dramp", bufs=1, space="DRAM", side="left")
    rs_in = [dramp.tile([4 * P, D], BF16, name=f"rsi{i}", tag=f"rsi{i}")
             for i in range(RB)]
    rs_out = [dramp.tile([P, D], BF16, name=f"rso{i}", tag=f"rso{i}")
              for i in range(RB)]

    psS = open_pool(name="psS", bufs=1, space="PSUM", side="left")
    psV = open_pool(name="psV", bufs=1, space="PSUM", side="left")

    def make_attn_work(rc, p, expP):
        nkb = 4 * rc + 4
        work = []
        pa_t = [None, None]

        def start_head(hh):
            pa_t[hh] = psV.tile([DK + 1, 512], F32, name="pa", tag="pa",
                                bufs=2)

        def mm_head(hh, kb):
            hl = 2 * p + hh
            nc.tensor.matmul(pa_t[hh][:], lhsT=v_sb[kb][:, hl, :],
                             rhs=expP[:, kb, hh, :],
                             start=(kb == 0), stop=(kb == nkb - 1))

        den_row = [None]

        def evict_den(hh):
            if hh == 0:
                # f32: RECIPROCAL with bf16 input hits a ~40x slower path
                den_row[0] = ph2w.tile([1, 2 * 512], F32, name="den_row",
                                       tag="den_row", bufs=1)
            nc.vector.tensor_copy(out=den_row[0][:, hh * 512:(hh + 1) * 512],
                                  in_=pa_t[hh][DK:DK + 1, :])

        recB = [None]

        def recip_bcast():
            # plain RECIPROCAL costs ~6.4ns per free element (iterative
            # divide); the approx version is ~5x faster and 18-bit exact
            rec_f = ph2w.tile([1, 2 * 512], F32, name="rec_f",
                              tag="rec_f", bufs=1)
            nc.vector.reciprocal_approx_fast(rec_f[:], den_row[0][:])
            rec_row = ph2w.tile([1, 2 * 512], BF16, name="rec_row",
                                tag="rec_row", bufs=1)
            nc.vector.tensor_copy(out=rec_row[:], in_=rec_f[:])
            # broadcast across partitions via K=1 PE matmuls (gpsimd's
            # partition_broadcast sits behind collectives in that queue);
            # matmul output must be f32, so one 1-bank tile per head
            recB[0] = ph2w.tile([DK, 2 * 512], BF16, name="recB", tag="recB",
                                bufs=1)
            for hh in range(2):
                pb = px_tile()
                nc.tensor.matmul(pb[0:DK, :], lhsT=ones64[:],
                                 rhs=rec_row[:, hh * 512:(hh + 1) * 512],
                                 start=True, stop=True)
                nc.vector.tensor_copy(
                    out=recB[0][:, hh * 512:(hh + 1) * 512],
                    in_=pb[0:DK, :])

        def mul_head(hh):
            nc.vector.tensor_mul(
                out=catT[p][hh * DK:(hh + 1) * DK,
                            rc * 512:(rc + 1) * 512],
                in0=pa_t[hh][0:DK, :],
                in1=recB[0][:, hh * 512:(hh + 1) * 512])

        for hh in range(2):
            work.append(lambda hh=hh: start_head(hh))
            for kb in range(nkb):
                work.append(lambda hh=hh, kb=kb: mm_head(hh, kb))
            work.append(lambda hh=hh: evict_den(hh))
        work.append(recip_bcast)
        work.append(lambda: mul_head(0))
        work.append(lambda: mul_head(1))
        return work

    y_box = {}

    def make_w1_work(rc):
        work = []

        def w1_block(rb, nt, box):
            c0 = rc * 512 + rb * P
            if nt == 0:
                box[0] = ph2w.tile([P, D], BF16, name="a1p", tag="a1p",
                                   bufs=2)
            psw = px_tile()
            for kc2 in range(2):
                nc.tensor.matmul(psw[:],
                                 lhsT=catT[kc2][:, c0:c0 + P],
                                 rhs=w1_sb[:, kc2, nt * 512:(nt + 1) * 512],
                                 start=(kc2 == 0), stop=(kc2 == 1))
            nc.vector.tensor_copy(out=box[0][:, nt * 512:(nt + 1) * 512],
                                  in_=psw[:])
            if nt == 1:
                nc.sync.dma_start(out=rs_in[rc][rb * P:(rb + 1) * P, :],
                                  in_=box[0][:])

        for rb in range(4):
            box = [None]
            for nt in range(2):
                work.append(lambda rb=rb, nt=nt, box=box: w1_block(rb, nt, box))

        def do_rs():
            # the collective is the ONLY gpsimd-queue op in ph2: anything
            # queued behind it stalls on the mesh handshake when peers skew
            nc.gpsimd.collective_compute(
                "ReduceScatter", ALU.add, replica_groups=RG,
                ins=[rs_in[rc][:].opt()], outs=[rs_out[rc][:].opt()])
        work.append(do_rs)
        return work

    def make_ln1_work(rc):
        work = []
        a1 = [None]
        fold = f["ln1_unit_g"] and f["ln1_zero_b"]

        def load():
            # emitted two chunks after the RS trigger, so these sync-queue
            # DMAs never sit blocked on the RS-done semaphore
            yb = ph3.tile([P, D], F32, name="y_sb", tag="y_sb", bufs=1)
            nc.sync.dma_start(out=yb[:],
                              in_=d["y_rows"][rc * P:(rc + 1) * P, :])
            y_box[rc] = yb
            ap = ph3.tile([P, D], BF16, name="a1pre", tag="a1pre", bufs=1)
            nc.sync.dma_start(out=ap[:], in_=rs_out[rc][:])
            a1pre_box[rc] = ap

        def residual():
            a1[0] = ph3.tile([P, D], F32, name="a1", tag="a1", bufs=1)
            nc.vector.tensor_add(out=a1[0][:], in0=a1pre_box[rc][:],
                                 in1=y_box[rc][:])
            if not f["b1_zero"]:
                nc.vector.tensor_add(out=a1[0][:], in0=a1[0][:],
                                     in1=f["b1_b"][:])

        a1b = [None]

        def ln():
            a1b[0] = ph3.tile([P, D], BF16, name="a1b", tag="a1b", bufs=1)
            if fold:
                # rstd folds SCALE -> a1T comes out pre-scaled
                ln_slim(ph3, a1[0][:], a1b[0][:], eps_s2,
                        var_scale=1.0 / (SCALE * SCALE))
            else:
                ln_slim(ph3, a1[0][:], a1b[0][:], eps_sb,
                        g_b=None if f["ln1_unit_g"] else f["g1_b"][:],
                        be_b=None if f["ln1_zero_b"] else f["be1_b"][:])

        def tr(kc):
            pt = px_tile()[:].bitcast(BF16)[:, 0:P]   # bf16 view of psum
            nc.tensor.transpose(pt, a1b[0][:, kc * P:(kc + 1) * P],
                                ident[:])
            if fold:
                nc.vector.tensor_copy(out=a1T[kc][:, rc * P:(rc + 1) * P],
                                      in_=pt)
            else:
                nc.vector.tensor_scalar(
                    out=a1T[kc][:, rc * P:(rc + 1) * P], in0=pt,
                    scalar1=float(SCALE), scalar2=None, op0=ALU.mult)

        work.append(load)
        work.append(residual)
        work.append(ln)
        work.extend(lambda kc=kc: tr(kc) for kc in range(KC))
        return work

    ph1_closed = False
    ln1_q = []
    for rc in (3, 2, 1, 0):
        nkb = 4 * rc + 4
        dn = 7 if rc == 3 else 4
        for p in range(2):
            # expP[kb][i] holds exp(scores) for head 2p+i, keys block kb
            expP = ph2w.tile([P, TB, 2, 512], BF16, name="expP", tag="expP",
                             bufs=2)
            for kb in range(nkb):
                ps = psS.tile([P, 1024], F32, name="ps_sc", tag="ps_sc",
                              bufs=2)
                nc.tensor.matmul(ps[:, 0:512],
                                 lhsT=kTp[p][0:DK, kb * P:(kb + 1) * P],
                                 rhs=qTp[p][0:DK, rc * 512:(rc + 1) * 512],
                                 start=True, stop=True,
                                 tile_position=(0, 0))
                nc.tensor.matmul(ps[:, 512:1024],
                                 lhsT=kTp[p][DK:P, kb * P:(kb + 1) * P],
                                 rhs=qTp[p][DK:P, rc * 512:(rc + 1) * 512],
                                 start=True, stop=True,
                                 tile_position=(64, 0))
                nc.scalar.activation(
                    out=expP[:, kb, :, :],
                    in_=ps[:].rearrange("p (h r) -> p h r", h=2),
                    func=AF.Exp)
                if kb >= 4 * rc:       # diagonal block: apply causal mask
                    i = kb - 4 * rc
                    for hh in range(2):
                        nc.vector.tensor_mul(out=expP[:, kb, hh, :],
                                             in0=expP[:, kb, hh, :],
                                             in1=mask4[:, i, :])
                drain(dn)
            pending.extend(make_attn_work(rc, p, expP))
        # LN1 for chunk rc is emitted TWO chunks later: its first DVE op
        # waits on that chunk's RS (~30-40us after the trigger), and any
        # earlier emission stalls the whole in-order DVE queue behind the
        # collective
        if len(ln1_q) >= 2:
            pending.extend(ln1_q.pop(0))
        pending.extend(make_w1_work(rc))
        ln1_q.append(make_ln1_work(rc))
        if not ph1_closed:
            # all q/v pending work must be emitted before ph1 frees
            ph1_closed = True
            drain(max(0, ph1_work_n - drained[0]))
            close_pool(ph1)
    pending.extend(ln1_q.pop(0))     # LN1 for rc=1
    drain(len(pending))
    close_pool(psV)
    close_pool(psS)
    close_pool(cat)
    close_pool(attn)
    close_pool(ph2w)

    # ================= Phase 4: cross-attention =========================
    # pools open before LN1(rc=0) is emitted, so the early score groups
    # (and their xT loads) slot in AHEAD of it on the PE/sync queues and
    # run during the last ReduceScatter's flight
    fw = open_pool(name="fw", bufs=1, side="left")        # Wf1, lives to FFN1
    wf1_all = fw.tile([P, KC, DF], BF16, name="wf1", tag="wf1")
    at2p = open_pool(name="at2p", bufs=1, side="left")    # at2T, into ph5
    at2T = [at2p.tile([P, R], BF16, name=f"at2T{i}", tag=f"at2T{i}")
            for i in range(KC)]
    w2p = open_pool(name="w2p", bufs=1, side="left")      # W2sum, into ph5
    w2 = [w2p.tile([P, D], BF16, name=f"w2_{i}", tag=f"w2_{i}")
          for i in range(KC)]
    ph4 = open_pool(name="ph4", bufs=1, side="left")
    pp4 = open_pool(name="pp4", bufs=4, space="PSUM", side="left")
    pd4 = open_pool(name="pd4", bufs=1, space="PSUM", side="left")
    for kc in range(4, KC):
        xT[kc] = ph4.tile([P, T], BF16, name=f"xT{kc}", tag=f"xT{kc}")
        nc.sync.dma_start(out=xT[kc][:],
                          in_=d["xT"][kc * P:(kc + 1) * P, :])
    p2T = [ph4.tile([P, R], BF16, name=f"p2T{i}", tag=f"p2T{i}")
           for i in range(TB)]
    # rows 128:512 of a1T (chunks rc=1..3) are ready long before rc=0's
    # ReduceScatter lands: the first 4 tb score groups accumulate that
    # 3/4 first, hiding PE work under the last RS
    ps_box4 = {}

    def sc4_a(tb):
        ps_box4[tb] = pp4.tile([P, 512], F32, name="ps4", tag="ps4")
        for kc in range(KC):
            nc.tensor.matmul(ps_box4[tb][:, P:512],
                             lhsT=xT[kc][:, tb * P:(tb + 1) * P],
                             rhs=a1T[kc][:, P:512],
                             start=(kc == 0), stop=(kc == KC - 1))

    for tb in range(4):
        sc4_a(tb)
    pending.extend(ln1_q.pop(0))     # LN1 for rc=0 (waits on its RS)
    drain(len(pending))
    close_pool(ph3)
    # x_tm / Wf1 / W2sum ride the sync queue BEHIND the rc=0 RS-result
    # load, so none of this 14MB contends with the last ReduceScatter;
    # x_tm goes first (attn2 needs it ~25us after the boundary)
    xtm = open_pool(name="xtm", bufs=1, side="right")
    x_tm = xtm.tile([P, TB, D], BF16, name="xtm", tag="xtm")
    nc.sync.dma_start(out=x_tm[:],
                      in_=d["x_tm"].rearrange("(t p) d -> p t d", p=P))
    nc.sync.dma_start(out=wf1_all[:],
                      in_=d["Wf1"].rearrange("p (c f) -> p c f", c=KC))
    for kc in range(KC):
        nc.sync.dma_start(out=w2[kc][:],
                          in_=d["W2sum"][kc * P:(kc + 1) * P, :])
    for tb in range(TB):
        if tb < 4:
            ps = ps_box4[tb]
            for kc in range(KC):
                nc.tensor.matmul(ps[:, 0:P],
                                 lhsT=xT[kc][:, tb * P:(tb + 1) * P],
                                 rhs=a1T[kc][:, 0:P],
                                 start=(kc == 0), stop=(kc == KC - 1))
        else:
            ps = pp4.tile([P, 512], F32, name="ps4", tag="ps4")
            for kc in range(KC):
                nc.tensor.matmul(ps[:], lhsT=xT[kc][:, tb * P:(tb + 1) * P],
                                 rhs=a1T[kc][:, :],
                                 start=(kc == 0), stop=(kc == KC - 1))
        nc.scalar.activation(out=p2T[tb][:], in_=ps[:], func=AF.Exp)
    # denominator: 4 col-tiled ones-matmul accumulators run concurrently
    pd = pd4.tile([P, R], F32, name="ps_d2", tag="ps_d2")
    for g in range(4):
        for u in range(4):
            tb = 4 * g + u
            nc.tensor.matmul(pd[32 * g:32 * g + 1, :], lhsT=ones_col[:],
                             rhs=p2T[tb][:], start=(u == 0), stop=(u == 3),
                             tile_position=(0, 32 * g))
    den4 = ph4.tile([1, 4, R], F32, name="den4", tag="den4")
    for g in range(4):
        nc.vector.tensor_copy(out=den4[:, g, :], in_=pd[32 * g:32 * g + 1, :])
    den2a = ph4.tile([1, R], F32, name="den2a", tag="den2a")
    den2b = ph4.tile([1, R], F32, name="den2b", tag="den2b")
    den2 = ph4.tile([1, R], F32, name="den2", tag="den2")
    nc.vector.tensor_add(out=den2a[:], in0=den4[:, 0, :], in1=den4[:, 1, :])
    nc.vector.tensor_add(out=den2b[:], in0=den4[:, 2, :], in1=den4[:, 3, :])
    nc.vector.tensor_add(out=den2[:], in0=den2a[:], in1=den2b[:])
    recip2f = ph4.tile([1, R], F32, name="recip2f", tag="recip2f")
    nc.vector.reciprocal_approx_fast(recip2f[:], den2[:])
    recip2 = ph4.tile([1, R], BF16, name="recip2", tag="recip2")
    nc.vector.tensor_copy(out=recip2[:], in_=recip2f[:])
    psb2 = pd4.tile([P, R], F32, name="psb2", tag="psb2")
    nc.tensor.matmul(psb2[:], lhsT=ones_row[:], rhs=recip2[:],
                     start=True, stop=True)
    recip2b = ph4.tile([P, R], F32, name="recip2b", tag="recip2b")
    nc.vector.tensor_copy(out=recip2b[:], in_=psb2[:])
    for db in range(KC):
        ps = pp4.tile([P, 512], F32, name="ps4", tag="ps4")
        for tb in range(TB):
            nc.tensor.matmul(ps[:], lhsT=x_tm[:, tb, db * P:(db + 1) * P],
                             rhs=p2T[tb][:],
                             start=(tb == 0), stop=(tb == TB - 1))
        nc.vector.tensor_mul(out=at2T[db][:], in0=ps[:], in1=recip2b[:])
    close_pool(pd4)
    close_pool(pp4)
    close_pool(psX)
    close_pool(xtm)
    close_pool(ph4)
    close_pool(a1pl)
    close_pool(xpre)

    # ========= Phase 5: W2sum + residual + LN2, produce a2T =============
    a2p = open_pool(name="a2p", bufs=1, side="right")     # a2T into ph6
    a2T = [a2p.tile([P, R], BF16, name=f"a2T{i}", tag=f"a2T{i}")
           for i in range(KC)]
    ph5 = open_pool(name="ph5", bufs=1, side="right")
    pp5 = open_pool(name="pp5", bufs=4, space="PSUM", side="left")
    pt5 = open_pool(name="pt5", bufs=2, space="PSUM", side="left")
    if not f["b2_zero"]:
        f["b2_b"] = bcast_row(ph5, "b2")
    if not f["ln2_unit_g"]:
        f["g2_b"] = bcast_row(ph5, "ln2_g")
    if not f["ln2_zero_b"]:
        f["be2_b"] = bcast_row(ph5, "ln2_b")
    for rb in range(RB):
        y5 = ph5.tile([P, D], F32, name="y5", tag="y5", bufs=2)
        nc.sync.dma_start(out=y5[:], in_=d["y_rows"][rb * P:(rb + 1) * P, :])
        a2 = ph5.tile([P, D], F32, name="a2", tag="a2", bufs=2)
        for nt in range(2):
            ps = pp5.tile([P, 512], F32, name="ps_a2", tag="ps_a2")
            for kc in range(KC):
                nc.tensor.matmul(ps[:],
                                 lhsT=at2T[kc][:, rb * P:(rb + 1) * P],
                                 rhs=w2[kc][:, nt * 512:(nt + 1) * 512],
                                 start=(kc == 0), stop=(kc == KC - 1))
            sl = slice(nt * 512, (nt + 1) * 512)
            nc.vector.tensor_add(out=a2[:, sl], in0=ps[:], in1=y5[:, sl])
            if not f["b2_zero"]:
                nc.vector.tensor_add(out=a2[:, sl], in0=a2[:, sl],
                                     in1=f["b2_b"][:, sl])
        a2b = ph5.tile([P, D], BF16, name="a2b", tag="a2b", bufs=2)
        ln_slim(ph5, a2[:], a2b[:], eps_sb,
                g_b=None if f["ln2_unit_g"] else f["g2_b"][:],
                be_b=None if f["ln2_zero_b"] else f["be2_b"][:])
        for kc in range(KC):
            pt = pt5.tile([P, P], BF16, name="pt_a2", tag="pt_a2")
            nc.tensor.transpose(pt[:], a2b[:, kc * P:(kc + 1) * P], ident[:])
            nc.vector.tensor_copy(out=a2T[kc][:, rb * P:(rb + 1) * P],
                                  in_=pt[:])
    close_pool(pt5)
    close_pool(pp5)
    close_pool(ph5)
    close_pool(w2p)
    close_pool(at2p)

    # ========== Phase 6: FFN + residual + LN3 ===========================
    fA = open_pool(name="fA", bufs=1, side="right")
    f1T = [fA.tile([P, R], BF16, name=f"f1T{i}", tag=f"f1T{i}")
           for i in range(FB)]
    bf1_sb = bias_chunks(fA, "bf1", FB)
    pfA = open_pool(name="pfA", bufs=3, space="PSUM", side="left")
    for fb in range(FB):
        ps = pfA.tile([P, 512], F32, name="ps_f1", tag="ps_f1")
        for kc in range(KC):
            nc.tensor.matmul(ps[:], lhsT=wf1_all[:, kc, fb * P:(fb + 1) * P],
                             rhs=a2T[kc][:, :],
                             start=(kc == 0), stop=(kc == KC - 1))
        # relu + bias on ACT (idle during the FFN)
        nc.scalar.activation(out=f1T[fb][:], in_=ps[:], func=AF.Relu,
                             bias=bf1_sb[:, fb:fb + 1], scale=1.0)
    close_pool(pfA)
    close_pool(fw)

    pfB = open_pool(name="pfB", bufs=1, space="PSUM", side="left")
    fB = open_pool(name="fB", bufs=1, side="right")
    ps_rb = [pfB.tile([P, D], F32, name=f"ps_rb{i}", tag=f"ps_rb{i}")
             for i in range(RB)]
    y6 = [fB.tile([P, D], F32, name=f"y6{i}", tag=f"y6{i}")
          for i in range(RB)]
    for rb in range(RB):
        nc.sync.dma_start(out=y6[rb][:],
                          in_=d["y_rows"][rb * P:(rb + 1) * P, :])
    if not f["bf2_zero"]:
        f["bf2_b"] = bcast_row(fB, "bf2")
    if not f["ln3_unit_g"]:
        f["g3_b"] = bcast_row(fB, "ln3_g")
    if not f["ln3_zero_b"]:
        f["be3_b"] = bcast_row(fB, "ln3_b")
    wf2_t = {}
    for fb in range(FB):
        wf2_fb = fB.tile([P, D], BF16, name="wf2s", tag="wf2s", bufs=6)
        nc.sync.dma_start(out=wf2_fb[:], in_=d["Wf2"][fb * P:(fb + 1) * P, :])
        wf2_t[fb] = wf2_fb
        if fb < FB - 4:
            for rb in range(RB):
                for nt in range(2):
                    nc.tensor.matmul(
                        ps_rb[rb][:, nt * 512:(nt + 1) * 512],
                        lhsT=f1T[fb][:, rb * P:(rb + 1) * P],
                        rhs=wf2_fb[:, nt * 512:(nt + 1) * 512],
                        start=(fb == 0), stop=False)

    def tail(rb):
        ff = fB.tile([P, D], F32, name="ff", tag="ff", bufs=2)
        nc.vector.tensor_add(out=ff[:], in0=ps_rb[rb][:], in1=y6[rb][:])
        if not f["bf2_zero"]:
            nc.vector.tensor_add(out=ff[:], in0=ff[:], in1=f["bf2_b"][:])
        o = fB.tile([P, D], F32, name="o", tag="o", bufs=2)
        ln_slim(fB, ff[:], o[:], eps_sb,
                g_b=None if f["ln3_unit_g"] else f["g3_b"][:],
                be_b=None if f["ln3_zero_b"] else f["be3_b"][:])
        nc.sync.dma_start(out=out_d[rb * P:(rb + 1) * P, :], in_=o[:])

    # last 4 fb row-major: each row block finishes early and its LN3+store
    # overlaps the remaining matmuls
    for rb in range(RB):
        for fb in range(FB - 4, FB):
            for nt in range(2):
                nc.tensor.matmul(ps_rb[rb][:, nt * 512:(nt + 1) * 512],
                                 lhsT=f1T[fb][:, rb * P:(rb + 1) * P],
                                 rhs=wf2_t[fb][:, nt * 512:(nt + 1) * 512],
                                 start=False, stop=(fb == FB - 1))
        tail(rb)
    close_pool(fB)
    close_pool(pfB)
    close_pool(fA)
    close_pool(a2p)
    close_pool(dramp)
    close_pool(const)


def _row_idx(j):
    return np.concatenate(
        [np.arange(512 * rc + 128 * j, 512 * rc + 128 * j + 128)
         for rc in range(4)])


def _flags(inputs):
    z = lambda a: bool(np.all(np.asarray(a) == 0.0))
    u = lambda a: bool(np.all(np.asarray(a) == 1.0))
    return {
        "b1_zero": z(inputs["b1"]), "b2_zero": z(inputs["b2"]),
        "bf2_zero": z(inputs["bf2"]),
        "ln1_unit_g": u(inputs["ln1_g"]), "ln1_zero_b": z(inputs["ln1_b"]),
        "ln2_unit_g": u(inputs["ln2_g"]), "ln2_zero_b": z(inputs["ln2_b"]),
        "ln3_unit_g": u(inputs["ln3_g"]), "ln3_zero_b": z(inputs["ln3_b"]),
    }


def _prep_host(inputs):
    f32 = lambda a: np.ascontiguousarray(np.asarray(a, np.float32))
    bf = lambda a: np.ascontiguousarray(
        np.asarray(a, np.float32).astype(ml_dtypes.bfloat16))
    x = f32(inputs["x"])
    y = f32(inputs["y"])
    mask = np.asarray(inputs["y_mask"]).astype(np.float32)
    # diagonal-block masks: mask4[ky, i, r] = mask[r, 128*i + ky]
    m4 = mask[0:512, 0:512].reshape(512, 4, 128).transpose(2, 1, 0)
    Wq = f32(inputs["Wq"])   # [H, D, DK]
    Wk = f32(inputs["Wk"])
    Wv = f32(inputs["Wv"])

    def chunkP(a):
        """[C*P, F] -> [P, C*F] so each partition's data is contiguous."""
        cp, fdim = a.shape
        return np.ascontiguousarray(
            a.reshape(cp // P, P, fdim).transpose(1, 0, 2).reshape(P, -1))

    shared = {
        "mask4": bf(m4),
        "b1": f32(inputs["b1"]),
        "ln1_g": f32(inputs["ln1_g"]), "ln1_b": f32(inputs["ln1_b"]),
        "W2sum": bf(f32(inputs["W2"]).reshape(H, D, D).sum(0)),
        "b2": f32(inputs["b2"]),
        "ln2_g": f32(inputs["ln2_g"]), "ln2_b": f32(inputs["ln2_b"]),
        "Wf1": chunkP(bf(inputs["Wf1"])),
        "bf1": f32(inputs["bf1"]),
        "Wf2": bf(inputs["Wf2"]),
        "bf2": f32(inputs["bf2"]),
        "ln3_g": f32(inputs["ln3_g"]), "ln3_b": f32(inputs["ln3_b"]),
    }
    in_maps = []
    for c in range(NCORES):
        b, j = c // 4, c % 4
        hh = slice(4 * j, 4 * j + 4)
        ridx = _row_idx(j)
        in_maps.append({
            "yT": np.ascontiguousarray(
                bf(y[b].T).reshape(KC, P, 4, 512)
                .transpose(1, 2, 0, 3).reshape(P, -1)),
            "wq": chunkP(bf(Wq[hh].transpose(1, 0, 2).reshape(D, 256) * SCALE)),
            "wk": chunkP(bf(Wk[hh].transpose(1, 0, 2).reshape(D, 256))),
            "wv": chunkP(bf(Wv[hh].transpose(1, 0, 2).reshape(D, 256))),
            "bq_s": f32(inputs["bq"])[hh].reshape(256) * np.float32(SCALE),
            "bk_f": f32(inputs["bk"])[hh].reshape(256),
            "bv_f": f32(inputs["bv"])[hh].reshape(256),
            "w1loc": chunkP(bf(f32(inputs["W1"])[256 * j:256 * (j + 1), :])),
            "y_rows": np.ascontiguousarray(y[b][ridx]),
            "xT": bf(x[b].T),
            "x_tm": bf(x[b]),
            **shared,
        })
    return in_maps


def kernel(**inputs):
    fl = _flags(inputs)
    key = tuple(sorted(fl.items()))
    if key not in _cached:
        _cached[key] = build_nc(dict(fl))
    nc = _cached[key]
    in_maps = _prep_host(inputs)
    res = run_bass_kernel_spmd(nc, in_maps, core_ids=list(range(NCORES)))
    out = np.zeros((B, S, D), np.float32)
    for c in range(NCORES):
        b, j = c // 4, c % 4
        out[b, _row_idx(j)] = res.results[c]["out"]
    return out


# revision 69
# speedup vs baseline: 1.1670x; 1.0040x over previous
"""Trainium2 Bass kernel for nn_DecoderLayer_33758442946809.

Sharding (8 cores = 2 batches x 4-core groups):
- Self-attention is HEAD-sharded: core (b, j) computes heads 4j..4j+3 for
  all T=2048 rows of batch b; causal skipping is SPMD-uniform (only
  lower-triangle key blocks are scored/exp'd).
- W1 is row-parallel over the head-sharded cat features; partials are
  summed with a chunked ReduceScatter (bf16) over each 4-core group.
  After the RS, core (b, j) owns the strided row set
  {512*rc + 128*j + i : rc<4, i<128}; cross-attention, W2 and the FFN
  are data-parallel over those rows.
- tile(attn2, H) @ W2 == attn2 @ sum_h W2[h] (host precomputes the sum).

v2 schedule (vs the phase-serial v1):
- Row chunks processed big-first [3,2,1,0] so the last ReduceScatter has
  the shortest dependency tail.
- Score PSUM is [128,1024] tiles (1 key block, both heads) with bufs=2,
  so ACT exp streams without stalling on PSUM WAR.
- Softmax normalization: denominator row is reciprocal'd at [1,2,512]
  (cheap) then partition-broadcast; the numerator is multiplied straight
  out of PSUM -- no [65,512] evictions, no 6.5us [64,1024] reciprocals.
- QKV projection matmuls (q tcols 2..0, all of v) drain as pending PE
  work under the first row-chunk's exp stream.
- RS outputs land via the sync queue (the gpsimd queue used to block
  ~90us on the RS-done semaphore).
- LN affine ops are skipped when gamma==1/beta==0 (host-checked program
  variant); the attention 1/sqrt(dk) scale is folded into LN1's rstd.
- Transposes run in bf16 (pre-cast) -- 1 PE cycle/row instead of 2.
- xT / x_tm / Wf1 / W2sum are prefetched a phase early.
- FFN1 relu+bias runs on ACT (idle there) instead of DVE.
- FFN2's last 4 weight blocks run row-major so row block 0 finishes
  ~6us early and the final LN3+store overlaps the remaining matmuls.
"""
import math
import sys

import numpy as np

sys.path.insert(0, "/opt/trn_rl_repo")

import ml_dtypes  # noqa: E402

import concourse.bass as bass  # noqa: E402
import concourse.tile as tile  # noqa: E402
from concourse import bacc, mybir  # noqa: E402
from concourse.bass_utils import run_bass_kernel_spmd  # noqa: E402
from concourse.masks import make_identity  # noqa: E402

B, S, D, H, DF = 2, 2048, 1024, 16, 4096
DK = D // H                      # 64
P = 128
T = S                            # rows/keys per batch
R = 512                          # own rows per core (after RS)
KC = D // P                      # 8 contraction chunks of D
TB = T // P                      # 16 key blocks
RB = R // P                      # 4 row blocks
FB = DF // P                     # 32 ffn blocks
NCORES = 8
HL = 4                           # local heads per core
SCALE = 1.0 / math.sqrt(DK)
RG = [[0, 1, 2, 3], [4, 5, 6, 7]]

F32 = mybir.dt.float32
BF16 = mybir.dt.bfloat16
AF = mybir.ActivationFunctionType
ALU = mybir.AluOpType

_cached = {}


def build_nc(f):
    nc = bacc.Bacc("TRN2", target_bir_lowering=False, debug=False,
                   num_devices=NCORES)

    dram = {}

    def din(name, shape, dt):
        dram[name] = nc.dram_tensor(name, shape, dt, kind="ExternalInput").ap()

    din("yT", [P, 4 * KC * 512], BF16)   # y[b].T, [p][tcol][kc][c] chunks
    din("wq", [P, KC * HL * DK], BF16)   # pre-chunked [p][kc][f] layout
    din("wk", [P, KC * HL * DK], BF16)
    din("wv", [P, KC * HL * DK], BF16)
    din("bq_s", [HL * DK], F32)      # bq * SCALE, local heads
    din("bk_f", [HL * DK], F32)
    din("bv_f", [HL * DK], F32)
    din("mask4", [P, 4, R], BF16)    # diagonal-block masks (key, i, row)
    din("w1loc", [P, 2 * D], BF16)   # W1 rows owned by this core, pre-chunked
    din("b1", [D], F32)
    din("ln1_g", [D], F32)
    din("ln1_b", [D], F32)
    din("y_rows", [R, D], F32)       # this core's (strided) y rows
    din("xT", [D, T], BF16)
    din("x_tm", [T, D], BF16)
    din("W2sum", [D, D], BF16)
    din("b2", [D], F32)
    din("ln2_g", [D], F32)
    din("ln2_b", [D], F32)
    din("Wf1", [P, KC * DF], BF16)   # pre-chunked [p][kc][f] layout
    din("bf1", [DF], F32)
    din("Wf2", [DF, D], BF16)
    din("bf2", [D], F32)
    din("ln3_g", [D], F32)
    din("ln3_b", [D], F32)
    out_d = nc.dram_tensor("out", [R, D], F32, kind="ExternalOutput").ap()

    with tile.TileContext(nc) as tc:
        _build(nc, tc, dram, out_d, f)
    nc.compile()
    return nc


def _build(nc, tc, d, out_d, f):
    pool_cms = {}

    def open_pool(*args, **kw):
        cm = tc.tile_pool(*args, **kw)
        p = cm.__enter__()
        pool_cms[id(p)] = cm
        return p

    def close_pool(p):
        pool_cms.pop(id(p)).__exit__(None, None, None)

    const = open_pool(name="const", bufs=1, side="left")
    ident = const.tile([P, P], BF16, name="ident", tag="ident")
    make_identity(nc, ident[:])
    ones_col = const.tile([P, 1], BF16, name="ones_col", tag="ones_col")
    nc.vector.memset(ones_col[:], 1.0)
    ones_row = const.tile([1, P], BF16, name="ones_row", tag="ones_row")
    nc.vector.memset(ones_row[:], 1.0)
    ones64 = const.tile([1, DK], BF16, name="ones64", tag="ones64")
    nc.vector.memset(ones64[:], 1.0)
    eps_sb = const.tile([P, 1], F32, name="eps", tag="eps")
    nc.vector.memset(eps_sb[:], 1e-5)
    # eps/SCALE^2: sqrt(var/SCALE^2 + eps/SCALE^2) = sqrt(var+eps)/SCALE,
    # so LN1's rstd comes out pre-multiplied by the attention scale.
    eps_s2 = const.tile([P, 1], F32, name="eps_s2", tag="eps_s2")
    nc.vector.memset(eps_s2[:], 1e-5 / (SCALE * SCALE))

    def bias_chunks(pool, name, n):
        t = pool.tile([P, n], F32, name=f"bc_{name}", tag=f"bc_{name}")
        nc.sync.dma_start(out=t[:], in_=d[name].rearrange("(n p) -> p n", p=P))
        return t

    def bcast_row(pool, name):
        src = d[name]
        t = pool.tile([P, D], F32, name=f"br_{name}", tag=f"br_{name}")
        bc = bass.AP(tensor=src.tensor, offset=src.offset,
                     ap=[[0, P]] + list(src.ap))
        nc.sync.dma_start(out=t[:], in_=bc)
        return t

    def ln_slim(pool, x_ap, out_ap, eps_ap, var_scale=1.0, g_b=None,
                be_b=None):
        """LayerNorm along the free axis (D) of a token-major [128, D]
        f32 tile into out_ap. var_scale folds a constant into rstd."""
        x3 = x_ap.rearrange("p (n f) -> p n f", f=512)
        stats = pool.tile([P, 2, 6], F32, name="ln_stats", tag="ln_stats",
                          bufs=4)
        for sg in range(2):
            nc.vector.bn_stats(out=stats[:, sg, :], in_=x3[:, sg, :])
        mv = pool.tile([P, 2], F32, name="ln_mv", tag="ln_mv", bufs=4)
        nc.vector.bn_aggr(out=mv[:], in_=stats[:])
        std = pool.tile([P, 1], F32, name="ln_std", tag="ln_std", bufs=4)
        nc.scalar.activation(out=std[:], in_=mv[:, 1:2], func=AF.Sqrt,
                             bias=eps_ap[:], scale=var_scale)
        rstd = pool.tile([P, 1], F32, name="ln_rstd", tag="ln_rstd", bufs=4)
        nc.vector.reciprocal(out=rstd[:], in_=std[:])
        nc.vector.tensor_scalar(out=out_ap, in0=x_ap, scalar1=mv[:, 0:1],
                                scalar2=rstd[:], op0=ALU.subtract,
                                op1=ALU.mult)
        if g_b is not None:
            nc.vector.tensor_mul(out=out_ap, in0=out_ap, in1=g_b)
        if be_b is not None:
            nc.vector.tensor_add(out=out_ap, in0=out_ap, in1=be_b)

    # ======== pools whose tiles live into ph4 (right-side bottom) =======
    xpre = open_pool(name="xpre", bufs=1, side="right")
    # only half of xT is prefetched (SBUF is tight during rc=3);
    # xT[4..7] load at ph4 open and are the last kcs of each score group
    xT = [xpre.tile([P, T], BF16, name=f"xT{i}", tag=f"xT{i}")
          if i < 4 else None for i in range(KC)]
    a1pl = open_pool(name="a1pl", bufs=1, side="right")
    a1T = [a1pl.tile([P, R], BF16, name=f"a1T{i}", tag=f"a1T{i}")
           for i in range(KC)]

    # ===================== input DMAs (spread across queues) ============
    # ph3 opens below attn/cat on the right so those can close at the
    # ph2->ph4 boundary while LN1(rc=0) still runs out of ph3
    ph3 = open_pool(name="ph3", bufs=1, side="right")     # LN1 working
    a1pre_box = {}
    if not f["b1_zero"]:
        f["b1_b"] = bcast_row(ph3, "b1")
    if not f["ln1_unit_g"]:
        f["g1_b"] = bcast_row(ph3, "ln1_g")
    if not f["ln1_zero_b"]:
        f["be1_b"] = bcast_row(ph3, "ln1_b")

    attn = open_pool(name="attn", bufs=1, side="right")  # live through ph2
    qTp = [attn.tile([P, T], BF16, name=f"qTp{i}", tag=f"qTp{i}")
           for i in range(2)]
    kTp = [attn.tile([P, T], BF16, name=f"kTp{i}", tag=f"kTp{i}")
           for i in range(2)]
    v_sb = [attn.tile([P, HL, DK + 1], BF16, name=f"v{i}", tag=f"v{i}")
            for i in range(TB)]
    mask4 = attn.tile([P, 4, R], BF16, name="mask4", tag="mask4")

    ph2w = open_pool(name="ph2w", bufs=1, side="left")   # ph2 working set
    ph1 = open_pool(name="ph1", bufs=1, side="left")     # closed after rc=3
    # yT lands as 4 tcol-major column slabs (host pre-chunked so each
    # partition reads one contiguous 8KB line per slab): the first
    # k-projection group can start after ~1MB instead of the full 4MB
    yT_all = ph1.tile([P, 4, KC, 512], BF16, name="yT", tag="yT")
    wq_sb = ph1.tile([P, KC, 2 * P], BF16, name="wq", tag="wq")
    wk_sb = ph1.tile([P, KC, 2 * P], BF16, name="wk", tag="wk")
    wv_sb = ph1.tile([P, KC, 2 * P], BF16, name="wv", tag="wv")
    def yT_cols(kc, lo, n):
        """columns [lo, lo+n) of the logical [P, KC, T] yT (n <= 512)"""
        tcol, c = lo // 512, lo % 512
        return yT_all[:, tcol, kc, c:c + n]
    nc.scalar.dma_start(out=wk_sb[:],
                        in_=d["wk"].rearrange("p (c f) -> p c f", c=KC))
    nc.scalar.dma_start(out=wq_sb[:],
                        in_=d["wq"].rearrange("p (c f) -> p c f", c=KC))
    nc.scalar.dma_start(out=wv_sb[:],
                        in_=d["wv"].rearrange("p (c f) -> p c f", c=KC))
    nc.gpsimd.dma_start(out=mask4[:], in_=d["mask4"][:])
    # half-slab DMAs round-robined over three queues: one queue alone
    # moves only ~150GB/s, which starves the first k-projection groups
    yTsrc = d["yT"].rearrange("p (a c r) -> p a c r", a=4, c=KC)
    qs = [nc.sync, nc.scalar, nc.gpsimd]
    for i in range(8):
        tcol, h = i // 2, i % 2
        qs[i % 3].dma_start(
            out=yT_all[:, tcol, 4 * h:4 * h + 4, :],
            in_=yTsrc[:, tcol, 4 * h:4 * h + 4, :])
    bq_sb = bias_chunks(ph1, "bq_s", 2)
    bk_sb = bias_chunks(ph1, "bk_f", 2)
    bv_b = ph1.tile([P, 2 * P], F32, name="bv_b", tag="bv_b")
    bv_src = d["bv_f"]
    nc.sync.dma_start(out=bv_b[:], in_=bass.AP(
        tensor=bv_src.tensor, offset=bv_src.offset,
        ap=[[0, P]] + list(bv_src.ap)))
    # cross-attention keys prefetched behind yT on the sync queue
    for kc in range(4):
        nc.sync.dma_start(out=xT[kc][:], in_=d["xT"][kc * P:(kc + 1) * P, :])

    # ============ ph1 QKV emission (k full, q tcol 3; rest pending) =====
    # psX: shared 1-bank [P,512] ring for q/v projections, W1 partials and
    # LN1 transposes (keeps psS at bufs=2 within the 8-bank budget)
    psX = open_pool(name="psX", bufs=1, space="PSUM", side="left")
    pending = []
    drained = [0]

    def drain(k):
        for _ in range(min(k, len(pending))):
            pending.pop(0)()
            drained[0] += 1

    def px_tile():
        return psX.tile([P, 512], F32, name="px", tag="px", bufs=2)

    def qk_group(dst, w_sb, b_sb, p, tcol):
        ps = px_tile()
        for kc in range(KC):
            nc.tensor.matmul(ps[:], lhsT=w_sb[:, kc, p * P:(p + 1) * P],
                             rhs=yT_all[:, tcol, kc, :],
                             start=(kc == 0), stop=(kc == KC - 1))
        nc.vector.tensor_scalar(out=dst[p][:, tcol * 512:(tcol + 1) * 512],
                                in0=ps[:], scalar1=b_sb[:, p:p + 1],
                                scalar2=None, op0=ALU.add)

    # k: all tcols (every score block needs all keys); q: tcol 3 first
    for p in range(2):
        for tcol in range(4):
            qk_group(kTp, wk_sb, bk_sb, p, tcol)
    for p in range(2):
        qk_group(qTp, wq_sb, bq_sb, p, 3)

    def v_work(tb):
        work = []
        box = [None]

        def v_start():
            nc.vector.memset(v_sb[tb][:, :, DK:DK + 1], 1.0)
            box[0] = px_tile()

        def v_mm(kc):
            nc.tensor.matmul(box[0][:, 0:2 * P],
                             lhsT=yT_cols(kc, tb * P, P),
                             rhs=wv_sb[:, kc, :],
                             start=(kc == 0), stop=(kc == KC - 1))

        def v_evict():
            nc.vector.tensor_add(
                out=v_sb[tb][:, :, 0:DK],
                in0=box[0][:, 0:2 * P].rearrange("p (h k) -> p h k", h=HL),
                in1=bv_b[:].rearrange("p (h k) -> p h k", h=HL))

        work.append(v_start)
        work.extend(lambda kc=kc: v_mm(kc) for kc in range(KC))
        work.append(v_evict)
        return work

    def q_work(p, tcol):
        work = []
        box = [None]

        def q_start():
            box[0] = px_tile()

        def q_mm(kc):
            nc.tensor.matmul(box[0][:],
                             lhsT=wq_sb[:, kc, p * P:(p + 1) * P],
                             rhs=yT_all[:, tcol, kc, :],
                             start=(kc == 0), stop=(kc == KC - 1))

        def q_evict():
            nc.vector.tensor_scalar(
                out=qTp[p][:, tcol * 512:(tcol + 1) * 512],
                in0=box[0][:], scalar1=bq_sb[:, p:p + 1],
                scalar2=None, op0=ALU.add)

        work.append(q_start)
        work.extend(lambda kc=kc: q_mm(kc) for kc in range(KC))
        work.append(q_evict)
        return work

    # v must be fully projected before the first attnV drains; emit v
    # first, then the remaining q columns.
    for tb in range(TB):
        pending.extend(v_work(tb))
    for tcol in (2, 1, 0):
        for p in range(2):
            pending.extend(q_work(p, tcol))
    ph1_work_n = len(pending)

    # ============ ph2: causal attention + W1 + ReduceScatter ============
    cat = open_pool(name="cat", bufs=1, side="right")     # catT, ph2-long
    catT = [cat.tile([P, T], BF16, name=f"catT{i}", tag=f"catT{i}")
            for i in range(2)]
    w1_sb = cat.tile([P, 2, D], BF16, name="w1", tag="w1")
    nc.gpsimd.dma_start(out=w1_sb[:],
                        in_=d["w1loc"].rearrange("p (c n) -> p c n", c=2))

    dramp = open_pool(name="dramp", bufs=1, space="DRAM", side="left")
    rs_in = [dramp.tile([4 * P, D], BF16, name=f"rsi{i}", tag=f"rsi{i}")
             for i in range(RB)]
    rs_out = [dramp.tile([P, D], BF16, name=f"rso{i}", tag=f"rso{i}")
              for i in range(RB)]

    psS = open_pool(name="psS", bufs=1, space="PSUM", side="left")
    psV = open_pool(name="psV", bufs=1, space="PSUM", side="left")

    def make_attn_work(rc, p, expP):
        nkb = 4 * rc + 4
        work = []
        pa_t = [None, None]

        def start_head(hh):
            pa_t[hh] = psV.tile([DK + 1, 512], F32, name="pa", tag="pa",
                                bufs=2)

        def mm_head(hh, kb):
            hl = 2 * p + hh
            nc.tensor.matmul(pa_t[hh][:], lhsT=v_sb[kb][:, hl, :],
                             rhs=expP[:, kb, hh, :],
                             start=(kb == 0), stop=(kb == nkb - 1))

        den_row = [None]

        def evict_den(hh):
            if hh == 0:
                # f32: RECIPROCAL with bf16 input hits a ~40x slower path
                den_row[0] = ph2w.tile([1, 2 * 512], F32, name="den_row",
                                       tag="den_row", bufs=1)
            nc.vector.tensor_copy(out=den_row[0][:, hh * 512:(hh + 1) * 512],
                                  in_=pa_t[hh][DK:DK + 1, :])

        recB = [None]

        def recip_bcast():
            # plain RECIPROCAL costs ~6.4ns per free element (iterative
            # divide); the approx version is ~5x faster and 18-bit exact
            rec_f = ph2w.tile([1, 2 * 512], F32, name="rec_f",
                              tag="rec_f", bufs=1)
            nc.vector.reciprocal_approx_fast(rec_f[:], den_row[0][:])
            rec_row = ph2w.tile([1, 2 * 512], BF16, name="rec_row",
                                tag="rec_row", bufs=1)
            nc.vector.tensor_copy(out=rec_row[:], in_=rec_f[:])
            # broadcast across partitions via K=1 PE matmuls (gpsimd's
            # partition_broadcast sits behind collectives in that queue);
            # matmul output must be f32, so one 1-bank tile per head
            recB[0] = ph2w.tile([DK, 2 * 512], BF16, name="recB", tag="recB",
                                bufs=1)
            for hh in range(2):
                pb = px_tile()
                nc.tensor.matmul(pb[0:DK, :], lhsT=ones64[:],
                                 rhs=rec_row[:, hh * 512:(hh + 1) * 512],
                                 start=True, stop=True)
                nc.vector.tensor_copy(
                    out=recB[0][:, hh * 512:(hh + 1) * 512],
                    in_=pb[0:DK, :])

        def mul_head(hh):
            nc.vector.tensor_mul(
                out=catT[p][hh * DK:(hh + 1) * DK,
                            rc * 512:(rc + 1) * 512],
                in0=pa_t[hh][0:DK, :],
                in1=recB[0][:, hh * 512:(hh + 1) * 512])

        for hh in range(2):
            work.append(lambda hh=hh: start_head(hh))
            for kb in range(nkb):
                work.append(lambda hh=hh, kb=kb: mm_head(hh, kb))
            work.append(lambda hh=hh: evict_den(hh))
        work.append(recip_bcast)
        work.append(lambda: mul_head(0))
        work.append(lambda: mul_head(1))
        return work

    y_box = {}

    def make_w1_work(rc):
        work = []

        def w1_block(rb, nt, box):
            c0 = rc * 512 + rb * P
            if nt == 0:
                box[0] = ph2w.tile([P, D], BF16, name="a1p", tag="a1p",
                                   bufs=2)
            psw = px_tile()
            for kc2 in range(2):
                nc.tensor.matmul(psw[:],
                                 lhsT=catT[kc2][:, c0:c0 + P],
                                 rhs=w1_sb[:, kc2, nt * 512:(nt + 1) * 512],
                                 start=(kc2 == 0), stop=(kc2 == 1))
            nc.vector.tensor_copy(out=box[0][:, nt * 512:(nt + 1) * 512],
                                  in_=psw[:])
            if nt == 1:
                nc.sync.dma_start(out=rs_in[rc][rb * P:(rb + 1) * P, :],
                                  in_=box[0][:])

        for rb in range(4):
            box = [None]
            for nt in range(2):
                work.append(lambda rb=rb, nt=nt, box=box: w1_block(rb, nt, box))

        def do_rs():
            # the collective is the ONLY gpsimd-queue op in ph2: anything
            # queued behind it stalls on the mesh handshake when peers skew
            nc.gpsimd.collective_compute(
                "ReduceScatter", ALU.add, replica_groups=RG,
                ins=[rs_in[rc][:].opt()], outs=[rs_out[rc][:].opt()])
        work.append(do_rs)
        return work

    def make_ln1_work(rc):
        work = []
        a1 = [None]
        fold = f["ln1_unit_g"] and f["ln1_zero_b"]

        def load():
            # emitted two chunks after the RS trigger, so these sync-queue
            # DMAs never sit blocked on the RS-done semaphore
            yb = ph3.tile([P, D], F32, name="y_sb", tag="y_sb", bufs=1)
            nc.sync.dma_start(out=yb[:],
                              in_=d["y_rows"][rc * P:(rc + 1) * P, :])
            y_box[rc] = yb
            ap = ph3.tile([P, D], BF16, name="a1pre", tag="a1pre", bufs=1)
            nc.sync.dma_start(out=ap[:], in_=rs_out[rc][:])
            a1pre_box[rc] = ap

        def residual():
            a1[0] = ph3.tile([P, D], F32, name="a1", tag="a1", bufs=1)
            nc.vector.tensor_add(out=a1[0][:], in0=a1pre_box[rc][:],
                                 in1=y_box[rc][:])
            if not f["b1_zero"]:
                nc.vector.tensor_add(out=a1[0][:], in0=a1[0][:],
                                     in1=f["b1_b"][:])

        a1b = [None]

        def ln():
            a1b[0] = ph3.tile([P, D], BF16, name="a1b", tag="a1b", bufs=1)
            if fold:
                # rstd folds SCALE -> a1T comes out pre-scaled
                ln_slim(ph3, a1[0][:], a1b[0][:], eps_s2,
                        var_scale=1.0 / (SCALE * SCALE))
            else:
                ln_slim(ph3, a1[0][:], a1b[0][:], eps_sb,
                        g_b=None if f["ln1_unit_g"] else f["g1_b"][:],
                        be_b=None if f["ln1_zero_b"] else f["be1_b"][:])

        def tr(kc):
            pt = px_tile()[:].bitcast(BF16)[:, 0:P]   # bf16 view of psum
            nc.tensor.transpose(pt, a1b[0][:, kc * P:(kc + 1) * P],
                                ident[:])
            if fold:
                nc.vector.tensor_copy(out=a1T[kc][:, rc * P:(rc + 1) * P],
                                      in_=pt)
            else:
                nc.vector.tensor_scalar(
                    out=a1T[kc][:, rc * P:(rc + 1) * P], in0=pt,
                    scalar1=float(SCALE), scalar2=None, op0=ALU.mult)

        work.append(load)
        work.append(residual)
        work.append(ln)
        work.extend(lambda kc=kc: tr(kc) for kc in range(KC))
        return work

    ph1_closed = False
    ln1_q = []
    for rc in (3, 2, 1, 0):
        nkb = 4 * rc + 4
        dn = 7 if rc == 3 else 4
        for p in range(2):
            # expP[kb][i] holds exp(scores) for head 2p+i, keys block kb
            expP = ph2w.tile([P, TB, 2, 512], BF16, name="expP", tag="expP",
                             bufs=2)
            for kb in range(nkb):
                ps = psS.tile([P, 1024], F32, name="ps_sc", tag="ps_sc",
                              bufs=2)
                nc.tensor.matmul(ps[:, 0:512],
                                 lhsT=kTp[p][0:DK, kb * P:(kb + 1) * P],
                                 rhs=qTp[p][0:DK, rc * 512:(rc + 1) * 512],
                                 start=True, stop=True,
                                 tile_position=(0, 0))
                nc.tensor.matmul(ps[:, 512:1024],
                                 lhsT=kTp[p][DK:P, kb * P:(kb + 1) * P],
                                 rhs=qTp[p][DK:P, rc * 512:(rc + 1) * 512],
                                 start=True, stop=True,
                                 tile_position=(64, 0))
                nc.scalar.activation(
                    out=expP[:, kb, :, :],
                    in_=ps[:].rearrange("p (h r) -> p h r", h=2),
                    func=AF.Exp)
                if kb >= 4 * rc:       # diagonal block: apply causal mask
                    i = kb - 4 * rc
                    for hh in range(2):
                        nc.vector.tensor_mul(out=expP[:, kb, hh, :],
                                             in0=expP[:, kb, hh, :],
                                             in1=mask4[:, i, :])
                drain(dn)
            pending.extend(make_attn_work(rc, p, expP))
        # LN1 for chunk rc is emitted TWO chunks later: its first DVE op
        # waits on that chunk's RS (~30-40us after the trigger), and any
        # earlier emission stalls the whole in-order DVE queue behind the
        # collective
        if len(ln1_q) >= 2:
            pending.extend(ln1_q.pop(0))
        pending.extend(make_w1_work(rc))
        ln1_q.append(make_ln1_work(rc))
        if not ph1_closed:
            # all q/v pending work must be emitted before ph1 frees
            ph1_closed = True
            drain(max(0, ph1_work_n - drained[0]))
            close_pool(ph1)
    pending.extend(ln1_q.pop(0))     # LN1 for rc=1
    drain(len(pending))
    close_pool(psV)
    close_pool(psS)
    close_pool(cat)
    close_pool(attn)
    close_pool(ph2w)

    # ================= Phase 4: cross-attention =========================
    # pools open before LN1(rc=0) is emitted, so the early score groups
    # (and their xT loads) slot in AHEAD of it on the PE/sync queues and
    # run during the last ReduceScatter's flight
    fw = open_pool(name="fw", bufs=1, side="left")        # Wf1, lives to FFN1
    wf1_all = fw.tile([P, KC, DF], BF16, name="wf1", tag="wf1")
    at2p = open_pool(name="at2p", bufs=1, side="left")    # at2T, into ph5
    at2T = [at2p.tile([P, R], BF16, name=f"at2T{i}", tag=f"at2T{i}")
            for i in range(KC)]
    w2p = open_pool(name="w2p", bufs=1, side="left")      # W2sum, into ph5
    w2 = [w2p.tile([P, D], BF16, name=f"w2_{i}", tag=f"w2_{i}")
          for i in range(KC)]
    ph4 = open_pool(name="ph4", bufs=1, side="left")
    pp4 = open_pool(name="pp4", bufs=4, space="PSUM", side="left")
    pd4 = open_pool(name="pd4", bufs=1, space="PSUM", side="left")
    for kc in range(4, KC):
        xT[kc] = ph4.tile([P, T], BF16, name=f"xT{kc}", tag=f"xT{kc}")
        nc.sync.dma_start(out=xT[kc][:],
                          in_=d["xT"][kc * P:(kc + 1) * P, :])
    p2T = [ph4.tile([P, R], BF16, name=f"p2T{i}", tag=f"p2T{i}")
           for i in range(TB)]
    # rows 128:512 of a1T (chunks rc=1..3) are ready long before rc=0's
    # ReduceScatter lands: the first 4 tb score groups accumulate that
    # 3/4 first, hiding PE work under the last RS
    ps_box4 = {}

    def sc4_a(tb):
        ps_box4[tb] = pp4.tile([P, 512], F32, name="ps4", tag="ps4")
        for kc in range(KC):
            nc.tensor.matmul(ps_box4[tb][:, P:512],
                             lhsT=xT[kc][:, tb * P:(tb + 1) * P],
                             rhs=a1T[kc][:, P:512],
                             start=(kc == 0), stop=(kc == KC - 1))

    for tb in range(4):
        sc4_a(tb)
    pending.extend(ln1_q.pop(0))     # LN1 for rc=0 (waits on its RS)
    drain(len(pending))
    close_pool(ph3)
    # x_tm / Wf1 / W2sum ride the sync queue BEHIND the rc=0 RS-result
    # load, so none of this 14MB contends with the last ReduceScatter;
    # x_tm goes first (attn2 needs it ~25us after the boundary)
    xtm = open_pool(name="xtm", bufs=1, side="right")
    x_tm = xtm.tile([P, TB, D], BF16, name="xtm", tag="xtm")
    nc.sync.dma_start(out=x_tm[:],
                      in_=d["x_tm"].rearrange("(t p) d -> p t d", p=P))
    nc.sync.dma_start(out=wf1_all[:],
                      in_=d["Wf1"].rearrange("p (c f) -> p c f", c=KC))
    for kc in range(KC):
        nc.sync.dma_start(out=w2[kc][:],
                          in_=d["W2sum"][kc * P:(kc + 1) * P, :])
    for tb in range(TB):
        if tb < 4:
            ps = ps_box4[tb]
            for kc in range(KC):
                nc.tensor.matmul(ps[:, 0:P],
                                 lhsT=xT[kc][:, tb * P:(tb + 1) * P],
                                 rhs=a1T[kc][:, 0:P],
                                 start=(kc == 0), stop=(kc == KC - 1))
        else:
            ps = pp4.tile([P, 512], F32, name="ps4", tag="ps4")
            for kc in range(KC):
                nc.tensor.matmul(ps[:], lhsT=xT[kc][:, tb * P:(tb + 1) * P],
                                 rhs=a1T[kc][:, :],
                                 start=(kc == 0), stop=(kc == KC - 1))
        nc.scalar.activation(out=p2T[tb][:], in_=ps[:], func=AF.Exp)

    at2_ps = {}

    def attn2_mms(db):
        ps = pp4.tile([P, 512], F32, name="ps4", tag="ps4")
        for tb in range(TB):
            nc.tensor.matmul(ps[:], lhsT=x_tm[:, tb, db * P:(db + 1) * P],
                             rhs=p2T[tb][:],
                             start=(tb == 0), stop=(tb == TB - 1))
        at2_ps[db] = ps

    # two attn2 groups keep the PE busy under the denominator chain
    attn2_mms(0)
    attn2_mms(1)
    # denominator: 4 col-tiled ones-matmul accumulators run concurrently
    pd = pd4.tile([P, R], F32, name="ps_d2", tag="ps_d2")
    for g in range(4):
        for u in range(4):
            tb = 4 * g + u
            nc.tensor.matmul(pd[32 * g:32 * g + 1, :], lhsT=ones_col[:],
                             rhs=p2T[tb][:], start=(u == 0), stop=(u == 3),
                             tile_position=(0, 32 * g))
    den4 = ph4.tile([1, 4, R], F32, name="den4", tag="den4")
    for g in range(4):
        nc.vector.tensor_copy(out=den4[:, g, :], in_=pd[32 * g:32 * g + 1, :])
    den2a = ph4.tile([1, R], F32, name="den2a", tag="den2a")
    den2b = ph4.tile([1, R], F32, name="den2b", tag="den2b")
    den2 = ph4.tile([1, R], F32, name="den2", tag="den2")
    nc.vector.tensor_add(out=den2a[:], in0=den4[:, 0, :], in1=den4[:, 1, :])
    nc.vector.tensor_add(out=den2b[:], in0=den4[:, 2, :], in1=den4[:, 3, :])
    nc.vector.tensor_add(out=den2[:], in0=den2a[:], in1=den2b[:])
    recip2f = ph4.tile([1, R], F32, name="recip2f", tag="recip2f")
    nc.vector.reciprocal_approx_fast(recip2f[:], den2[:])
    recip2 = ph4.tile([1, R], BF16, name="recip2", tag="recip2")
    nc.vector.tensor_copy(out=recip2[:], in_=recip2f[:])
    psb2 = pd4.tile([P, R], F32, name="psb2", tag="psb2")
    nc.tensor.matmul(psb2[:], lhsT=ones_row[:], rhs=recip2[:],
                     start=True, stop=True)
    recip2b = ph4.tile([P, R], F32, name="recip2b", tag="recip2b")
    nc.vector.tensor_copy(out=recip2b[:], in_=psb2[:])
    for db in range(KC):
        if db >= 2:
            attn2_mms(db)
        nc.vector.tensor_mul(out=at2T[db][:], in0=at2_ps[db][:],
                             in1=recip2b[:])
    close_pool(pd4)
    close_pool(pp4)
    close_pool(psX)
    close_pool(xtm)
    close_pool(ph4)
    close_pool(a1pl)
    close_pool(xpre)

    # ========= Phase 5: W2sum + residual + LN2, produce a2T =============
    a2p = open_pool(name="a2p", bufs=1, side="right")     # a2T into ph6
    a2T = [a2p.tile([P, R], BF16, name=f"a2T{i}", tag=f"a2T{i}")
           for i in range(KC)]
    ph5 = open_pool(name="ph5", bufs=1, side="right")
    pp5 = open_pool(name="pp5", bufs=4, space="PSUM", side="left")
    pt5 = open_pool(name="pt5", bufs=2, space="PSUM", side="left")
    if not f["b2_zero"]:
        f["b2_b"] = bcast_row(ph5, "b2")
    if not f["ln2_unit_g"]:
        f["g2_b"] = bcast_row(ph5, "ln2_g")
    if not f["ln2_zero_b"]:
        f["be2_b"] = bcast_row(ph5, "ln2_b")
    for rb in range(RB):
        y5 = ph5.tile([P, D], F32, name="y5", tag="y5", bufs=2)
        nc.sync.dma_start(out=y5[:], in_=d["y_rows"][rb * P:(rb + 1) * P, :])
        a2 = ph5.tile([P, D], F32, name="a2", tag="a2", bufs=2)
        for nt in range(2):
            ps = pp5.tile([P, 512], F32, name="ps_a2", tag="ps_a2")
            for kc in range(KC):
                nc.tensor.matmul(ps[:],
                                 lhsT=at2T[kc][:, rb * P:(rb + 1) * P],
                                 rhs=w2[kc][:, nt * 512:(nt + 1) * 512],
                                 start=(kc == 0), stop=(kc == KC - 1))
            sl = slice(nt * 512, (nt + 1) * 512)
            nc.vector.tensor_add(out=a2[:, sl], in0=ps[:], in1=y5[:, sl])
            if not f["b2_zero"]:
                nc.vector.tensor_add(out=a2[:, sl], in0=a2[:, sl],
                                     in1=f["b2_b"][:, sl])
        a2b = ph5.tile([P, D], BF16, name="a2b", tag="a2b", bufs=2)
        ln_slim(ph5, a2[:], a2b[:], eps_sb,
                g_b=None if f["ln2_unit_g"] else f["g2_b"][:],
                be_b=None if f["ln2_zero_b"] else f["be2_b"][:])
        for kc in range(KC):
            pt = pt5.tile([P, P], BF16, name="pt_a2", tag="pt_a2")
            nc.tensor.transpose(pt[:], a2b[:, kc * P:(kc + 1) * P], ident[:])
            nc.vector.tensor_copy(out=a2T[kc][:, rb * P:(rb + 1) * P],
                                  in_=pt[:])
    close_pool(pt5)
    close_pool(pp5)
    close_pool(ph5)
    close_pool(w2p)
    close_pool(at2p)

    # ========== Phase 6: FFN + residual + LN3 ===========================
    fA = open_pool(name="fA", bufs=1, side="right")
    f1T = [fA.tile([P, R], BF16, name=f"f1T{i}", tag=f"f1T{i}")
           for i in range(FB)]
    bf1_sb = bias_chunks(fA, "bf1", FB)
    pfA = open_pool(name="pfA", bufs=3, space="PSUM", side="left")
    for fb in range(FB):
        ps = pfA.tile([P, 512], F32, name="ps_f1", tag="ps_f1")
        for kc in range(KC):
            nc.tensor.matmul(ps[:], lhsT=wf1_all[:, kc, fb * P:(fb + 1) * P],
                             rhs=a2T[kc][:, :],
                             start=(kc == 0), stop=(kc == KC - 1))
        # relu + bias on ACT (idle during the FFN)
        nc.scalar.activation(out=f1T[fb][:], in_=ps[:], func=AF.Relu,
                             bias=bf1_sb[:, fb:fb + 1], scale=1.0)
    close_pool(pfA)
    close_pool(fw)

    pfB = open_pool(name="pfB", bufs=1, space="PSUM", side="left")
    fB = open_pool(name="fB", bufs=1, side="right")
    ps_rb = [pfB.tile([P, D], F32, name=f"ps_rb{i}", tag=f"ps_rb{i}")
             for i in range(RB)]
    y6 = [fB.tile([P, D], F32, name=f"y6{i}", tag=f"y6{i}")
          for i in range(RB)]
    for rb in range(RB):
        nc.sync.dma_start(out=y6[rb][:],
                          in_=d["y_rows"][rb * P:(rb + 1) * P, :])
    if not f["bf2_zero"]:
        f["bf2_b"] = bcast_row(fB, "bf2")
    if not f["ln3_unit_g"]:
        f["g3_b"] = bcast_row(fB, "ln3_g")
    if not f["ln3_zero_b"]:
        f["be3_b"] = bcast_row(fB, "ln3_b")
    wf2_t = {}
    for fb in range(FB):
        wf2_fb = fB.tile([P, D], BF16, name="wf2s", tag="wf2s", bufs=11)
        nc.sync.dma_start(out=wf2_fb[:], in_=d["Wf2"][fb * P:(fb + 1) * P, :])
        wf2_t[fb] = wf2_fb
        if fb < FB - 8:
            for rb in range(RB):
                for nt in range(2):
                    nc.tensor.matmul(
                        ps_rb[rb][:, nt * 512:(nt + 1) * 512],
                        lhsT=f1T[fb][:, rb * P:(rb + 1) * P],
                        rhs=wf2_fb[:, nt * 512:(nt + 1) * 512],
                        start=(fb == 0), stop=False)

    def tail(rb):
        ff = fB.tile([P, D], F32, name="ff", tag="ff", bufs=2)
        nc.vector.tensor_add(out=ff[:], in0=ps_rb[rb][:], in1=y6[rb][:])
        if not f["bf2_zero"]:
            nc.vector.tensor_add(out=ff[:], in0=ff[:], in1=f["bf2_b"][:])
        o = fB.tile([P, D], F32, name="o", tag="o", bufs=2)
        ln_slim(fB, ff[:], o[:], eps_sb,
                g_b=None if f["ln3_unit_g"] else f["g3_b"][:],
                be_b=None if f["ln3_zero_b"] else f["be3_b"][:])
        nc.sync.dma_start(out=out_d[rb * P:(rb + 1) * P, :], in_=o[:])

    # last 8 fb row-major: each row block finishes ~10us early and its
    # LN3+store overlaps the remaining matmuls
    for rb in range(RB):
        for fb in range(FB - 8, FB):
            for nt in range(2):
                nc.tensor.matmul(ps_rb[rb][:, nt * 512:(nt + 1) * 512],
                                 lhsT=f1T[fb][:, rb * P:(rb + 1) * P],
                                 rhs=wf2_t[fb][:, nt * 512:(nt + 1) * 512],
                                 start=False, stop=(fb == FB - 1))
        tail(rb)
    close_pool(fB)
    close_pool(pfB)
    close_pool(fA)
    close_pool(a2p)
    close_pool(dramp)
    close_pool(const)


def _row_idx(j):
    return np.concatenate(
        [np.arange(512 * rc + 128 * j, 512 * rc + 128 * j + 128)
         for rc in range(4)])


def _flags(inputs):
    z = lambda a: bool(np.all(np.asarray(a) == 0.0))
    u = lambda a: bool(np.all(np.asarray(a) == 1.0))
    return {
        "b1_zero": z(inputs["b1"]), "b2_zero": z(inputs["b2"]),
        "bf2_zero": z(inputs["bf2"]),
        "ln1_unit_g": u(inputs["ln1_g"]), "ln1_zero_b": z(inputs["ln1_b"]),
        "ln2_unit_g": u(inputs["ln2_g"]), "ln2_zero_b": z(inputs["ln2_b"]),
        "ln3_unit_g": u(inputs["ln3_g"]), "ln3_zero_b": z(inputs["ln3_b"]),
    }


def _prep_host(inputs):
    f32 = lambda a: np.ascontiguousarray(np.asarray(a, np.float32))
    bf = lambda a: np.ascontiguousarray(
        np.asarray(a, np.float32).astype(ml_dtypes.bfloat16))
    x = f32(inputs["x"])
    y = f32(inputs["y"])
    mask = np.asarray(inputs["y_mask"]).astype(np.float32)
    # diagonal-block masks: mask4[ky, i, r] = mask[r, 128*i + ky]
    m4 = mask[0:512, 0:512].reshape(512, 4, 128).transpose(2, 1, 0)
    Wq = f32(inputs["Wq"])   # [H, D, DK]
    Wk = f32(inputs["Wk"])
    Wv = f32(inputs["Wv"])

    def chunkP(a):
        """[C*P, F] -> [P, C*F] so each partition's data is contiguous."""
        cp, fdim = a.shape
        return np.ascontiguousarray(
            a.reshape(cp // P, P, fdim).transpose(1, 0, 2).reshape(P, -1))

    shared = {
        "mask4": bf(m4),
        "b1": f32(inputs["b1"]),
        "ln1_g": f32(inputs["ln1_g"]), "ln1_b": f32(inputs["ln1_b"]),
        "W2sum": bf(f32(inputs["W2"]).reshape(H, D, D).sum(0)),
        "b2": f32(inputs["b2"]),
        "ln2_g": f32(inputs["ln2_g"]), "ln2_b": f32(inputs["ln2_b"]),
        "Wf1": chunkP(bf(inputs["Wf1"])),
        "bf1": f32(inputs["bf1"]),
        "Wf2": bf(inputs["Wf2"]),
        "bf2": f32(inputs["bf2"]),
        "ln3_g": f32(inputs["ln3_g"]), "ln3_b": f32(inputs["ln3_b"]),
    }
    in_maps = []
    for c in range(NCORES):
        b, j = c // 4, c % 4
        hh = slice(4 * j, 4 * j + 4)
        ridx = _row_idx(j)
        in_maps.append({
            "yT": np.ascontiguousarray(
                bf(y[b].T).reshape(KC, P, 4, 512)
                .transpose(1, 2, 0, 3).reshape(P, -1)),
            "wq": chunkP(bf(Wq[hh].transpose(1, 0, 2).reshape(D, 256) * SCALE)),
            "wk": chunkP(bf(Wk[hh].transpose(1, 0, 2).reshape(D, 256))),
            "wv": chunkP(bf(Wv[hh].transpose(1, 0, 2).reshape(D, 256))),
            "bq_s": f32(inputs["bq"])[hh].reshape(256) * np.float32(SCALE),
            "bk_f": f32(inputs["bk"])[hh].reshape(256),
            "bv_f": f32(inputs["bv"])[hh].reshape(256),
            "w1loc": chunkP(bf(f32(inputs["W1"])[256 * j:256 * (j + 1), :])),
            "y_rows": np.ascontiguousarray(y[b][ridx]),
            "xT": bf(x[b].T),
            "x_tm": bf(x[b]),
            **shared,
        })
    return in_maps


def kernel(**inputs):
    fl = _flags(inputs)
    key = tuple(sorted(fl.items()))
    if key not in _cached:
        _cached[key] = build_nc(dict(fl))
    nc = _cached[key]
    in_maps = _prep_host(inputs)
    res = run_bass_kernel_spmd(nc, in_maps, core_ids=list(range(NCORES)))
    out = np.zeros((B, S, D), np.float32)
    for c in range(NCORES):
        b, j = c // 4, c % 4
        out[b, _row_idx(j)] = res.results[c]["out"]
    return out


# revision 70
# speedup vs baseline: 1.2004x; 1.0286x over previous
"""Trainium2 Bass kernel for nn_DecoderLayer_33758442946809.

Sharding (8 cores = 2 batches x 4-core groups):
- Self-attention is HEAD-sharded: core (b, j) computes heads 4j..4j+3 for
  all T=2048 rows of batch b; causal skipping is SPMD-uniform (only
  lower-triangle key blocks are scored/exp'd).
- W1 is row-parallel over the head-sharded cat features; partials are
  summed with a chunked ReduceScatter (bf16) over each 4-core group.
  After the RS, core (b, j) owns the strided row set
  {512*rc + 128*j + i : rc<4, i<128}; cross-attention, W2 and the FFN
  are data-parallel over those rows.
- tile(attn2, H) @ W2 == attn2 @ sum_h W2[h] (host precomputes the sum).

v2 schedule (vs the phase-serial v1):
- Row chunks processed big-first [3,2,1,0] so the last ReduceScatter has
  the shortest dependency tail.
- Score PSUM is [128,1024] tiles (1 key block, both heads) with bufs=2,
  so ACT exp streams without stalling on PSUM WAR.
- Softmax normalization: denominator row is reciprocal'd at [1,2,512]
  (cheap) then partition-broadcast; the numerator is multiplied straight
  out of PSUM -- no [65,512] evictions, no 6.5us [64,1024] reciprocals.
- QKV projection matmuls (q tcols 2..0, all of v) drain as pending PE
  work under the first row-chunk's exp stream.
- RS outputs land via the sync queue (the gpsimd queue used to block
  ~90us on the RS-done semaphore).
- LN affine ops are skipped when gamma==1/beta==0 (host-checked program
  variant); the attention 1/sqrt(dk) scale is folded into LN1's rstd.
- Transposes run in bf16 (pre-cast) -- 1 PE cycle/row instead of 2.
- xT / x_tm / Wf1 / W2sum are prefetched a phase early.
- FFN1 relu+bias runs on ACT (idle there) instead of DVE.
- FFN2's last 4 weight blocks run row-major so row block 0 finishes
  ~6us early and the final LN3+store overlaps the remaining matmuls.
"""
import math
import sys

import numpy as np

sys.path.insert(0, "/opt/trn_rl_repo")

import ml_dtypes  # noqa: E402

import concourse.bass as bass  # noqa: E402
import concourse.tile as tile  # noqa: E402
from concourse import bacc, mybir  # noqa: E402
from concourse.bass_utils import run_bass_kernel_spmd  # noqa: E402
from concourse.masks import make_identity  # noqa: E402

B, S, D, H, DF = 2, 2048, 1024, 16, 4096
DK = D // H                      # 64
P = 128
T = S                            # rows/keys per batch
R = 512                          # own rows per core (after RS)
KC = D // P                      # 8 contraction chunks of D
TB = T // P                      # 16 key blocks
RB = R // P                      # 4 row blocks
FB = DF // P                     # 32 ffn blocks
NCORES = 8
HL = 4                           # local heads per core
SCALE = 1.0 / math.sqrt(DK)
RG = [[0, 1, 2, 3], [4, 5, 6, 7]]

F32 = mybir.dt.float32
BF16 = mybir.dt.bfloat16
AF = mybir.ActivationFunctionType
ALU = mybir.AluOpType

_cached = {}


def build_nc(f):
    nc = bacc.Bacc("TRN2", target_bir_lowering=False, debug=False,
                   num_devices=NCORES)

    dram = {}

    def din(name, shape, dt):
        dram[name] = nc.dram_tensor(name, shape, dt, kind="ExternalInput").ap()

    din("yT", [P, 4 * KC * 512], BF16)   # y[b].T, [p][tcol][kc][c] chunks
    din("wq", [P, KC * HL * DK], BF16)   # pre-chunked [p][kc][f] layout
    din("wk", [P, KC * HL * DK], BF16)
    din("wv", [P, KC * HL * DK], BF16)
    din("bq_s", [HL * DK], F32)      # bq * SCALE, local heads
    din("bk_f", [HL * DK], F32)
    din("bv_f", [HL * DK], F32)
    din("mask4", [P, 4, R], BF16)    # diagonal-block masks (key, i, row)
    din("w1loc", [P, 2 * D], BF16)   # W1 rows owned by this core, pre-chunked
    din("b1", [D], F32)
    din("ln1_g", [D], F32)
    din("ln1_b", [D], F32)
    din("y_rows", [R, D], F32)       # this core's (strided) y rows
    din("xT", [D, T], BF16)
    din("x_tm", [T, D], BF16)
    din("W2sum", [D, D], BF16)
    din("b2", [D], F32)
    din("ln2_g", [D], F32)
    din("ln2_b", [D], F32)
    din("Wf1", [P, KC * DF], BF16)   # pre-chunked [p][kc][f] layout
    din("bf1", [DF], F32)
    din("Wf2", [DF, D], BF16)
    din("bf2", [D], F32)
    din("ln3_g", [D], F32)
    din("ln3_b", [D], F32)
    out_d = nc.dram_tensor("out", [R, D], F32, kind="ExternalOutput").ap()

    with tile.TileContext(nc) as tc:
        _build(nc, tc, dram, out_d, f)
    nc.compile()
    return nc


def _build(nc, tc, d, out_d, f):
    pool_cms = {}

    def open_pool(*args, **kw):
        cm = tc.tile_pool(*args, **kw)
        p = cm.__enter__()
        pool_cms[id(p)] = cm
        return p

    def close_pool(p):
        pool_cms.pop(id(p)).__exit__(None, None, None)

    const = open_pool(name="const", bufs=1, side="left")
    ident = const.tile([P, P], BF16, name="ident", tag="ident")
    make_identity(nc, ident[:])
    ones_col = const.tile([P, 1], BF16, name="ones_col", tag="ones_col")
    nc.vector.memset(ones_col[:], 1.0)
    ones_row = const.tile([1, P], BF16, name="ones_row", tag="ones_row")
    nc.vector.memset(ones_row[:], 1.0)
    ones64 = const.tile([1, DK], BF16, name="ones64", tag="ones64")
    nc.vector.memset(ones64[:], 1.0)
    eps_sb = const.tile([P, 1], F32, name="eps", tag="eps")
    nc.vector.memset(eps_sb[:], 1e-5)
    # eps/SCALE^2: sqrt(var/SCALE^2 + eps/SCALE^2) = sqrt(var+eps)/SCALE,
    # so LN1's rstd comes out pre-multiplied by the attention scale.
    eps_s2 = const.tile([P, 1], F32, name="eps_s2", tag="eps_s2")
    nc.vector.memset(eps_s2[:], 1e-5 / (SCALE * SCALE))

    def bias_chunks(pool, name, n):
        t = pool.tile([P, n], F32, name=f"bc_{name}", tag=f"bc_{name}")
        nc.sync.dma_start(out=t[:], in_=d[name].rearrange("(n p) -> p n", p=P))
        return t

    def bcast_row(pool, name):
        src = d[name]
        t = pool.tile([P, D], F32, name=f"br_{name}", tag=f"br_{name}")
        bc = bass.AP(tensor=src.tensor, offset=src.offset,
                     ap=[[0, P]] + list(src.ap))
        nc.sync.dma_start(out=t[:], in_=bc)
        return t

    def ln_slim(pool, x_ap, out_ap, eps_ap, var_scale=1.0, g_b=None,
                be_b=None):
        """LayerNorm along the free axis (D) of a token-major [128, D]
        f32 tile into out_ap. var_scale folds a constant into rstd."""
        x3 = x_ap.rearrange("p (n f) -> p n f", f=512)
        stats = pool.tile([P, 2, 6], F32, name="ln_stats", tag="ln_stats",
                          bufs=4)
        for sg in range(2):
            nc.vector.bn_stats(out=stats[:, sg, :], in_=x3[:, sg, :])
        mv = pool.tile([P, 2], F32, name="ln_mv", tag="ln_mv", bufs=4)
        nc.vector.bn_aggr(out=mv[:], in_=stats[:])
        std = pool.tile([P, 1], F32, name="ln_std", tag="ln_std", bufs=4)
        nc.scalar.activation(out=std[:], in_=mv[:, 1:2], func=AF.Sqrt,
                             bias=eps_ap[:], scale=var_scale)
        rstd = pool.tile([P, 1], F32, name="ln_rstd", tag="ln_rstd", bufs=4)
        nc.vector.reciprocal(out=rstd[:], in_=std[:])
        nc.vector.tensor_scalar(out=out_ap, in0=x_ap, scalar1=mv[:, 0:1],
                                scalar2=rstd[:], op0=ALU.subtract,
                                op1=ALU.mult)
        if g_b is not None:
            nc.vector.tensor_mul(out=out_ap, in0=out_ap, in1=g_b)
        if be_b is not None:
            nc.vector.tensor_add(out=out_ap, in0=out_ap, in1=be_b)

    # ======== pools whose tiles live into ph4 (right-side bottom) =======
    xpre = open_pool(name="xpre", bufs=1, side="right")
    # only half of xT is prefetched (SBUF is tight during rc=3);
    # xT[4..7] load at ph4 open and are the last kcs of each score group
    xT = [xpre.tile([P, T], BF16, name=f"xT{i}", tag=f"xT{i}")
          if i < 4 else None for i in range(KC)]
    a1pl = open_pool(name="a1pl", bufs=1, side="right")
    a1T = [a1pl.tile([P, R], BF16, name=f"a1T{i}", tag=f"a1T{i}")
           for i in range(KC)]

    # ===================== input DMAs (spread across queues) ============
    # ph3 opens below attn/cat on the right so those can close at the
    # ph2->ph4 boundary while LN1(rc=0) still runs out of ph3
    ph3 = open_pool(name="ph3", bufs=1, side="right")     # LN1 working
    a1pre_box = {}
    if not f["b1_zero"]:
        f["b1_b"] = bcast_row(ph3, "b1")
    if not f["ln1_unit_g"]:
        f["g1_b"] = bcast_row(ph3, "ln1_g")
    if not f["ln1_zero_b"]:
        f["be1_b"] = bcast_row(ph3, "ln1_b")

    attn = open_pool(name="attn", bufs=1, side="right")  # live through ph2
    qTp = [attn.tile([P, T], BF16, name=f"qTp{i}", tag=f"qTp{i}")
           for i in range(2)]
    kTp = [attn.tile([P, T], BF16, name=f"kTp{i}", tag=f"kTp{i}")
           for i in range(2)]
    v_sb = [attn.tile([P, HL, DK + 1], BF16, name=f"v{i}", tag=f"v{i}")
            for i in range(TB)]
    mask4 = attn.tile([P, 4, R], BF16, name="mask4", tag="mask4")

    ph2w = open_pool(name="ph2w", bufs=1, side="left")   # ph2 working set
    ph1 = open_pool(name="ph1", bufs=1, side="left")     # closed after rc=3
    # yT lands as 4 tcol-major column slabs (host pre-chunked so each
    # partition reads one contiguous 8KB line per slab): the first
    # k-projection group can start after ~1MB instead of the full 4MB
    yT_all = ph1.tile([P, 4, KC, 512], BF16, name="yT", tag="yT")
    wq_sb = ph1.tile([P, KC, 2 * P], BF16, name="wq", tag="wq")
    wk_sb = ph1.tile([P, KC, 2 * P], BF16, name="wk", tag="wk")
    wv_sb = ph1.tile([P, KC, 2 * P], BF16, name="wv", tag="wv")
    def yT_cols(kc, lo, n):
        """columns [lo, lo+n) of the logical [P, KC, T] yT (n <= 512)"""
        tcol, c = lo // 512, lo % 512
        return yT_all[:, tcol, kc, c:c + n]
    nc.scalar.dma_start(out=wk_sb[:],
                        in_=d["wk"].rearrange("p (c f) -> p c f", c=KC))
    nc.scalar.dma_start(out=wq_sb[:],
                        in_=d["wq"].rearrange("p (c f) -> p c f", c=KC))
    nc.scalar.dma_start(out=wv_sb[:],
                        in_=d["wv"].rearrange("p (c f) -> p c f", c=KC))
    nc.gpsimd.dma_start(out=mask4[:], in_=d["mask4"][:])
    # half-slab DMAs round-robined over three queues: one queue alone
    # moves only ~150GB/s, which starves the first k-projection groups
    yTsrc = d["yT"].rearrange("p (a c r) -> p a c r", a=4, c=KC)
    qs = [nc.sync, nc.scalar, nc.gpsimd]
    for i in range(8):
        tcol, h = i // 2, i % 2
        qs[i % 3].dma_start(
            out=yT_all[:, tcol, 4 * h:4 * h + 4, :],
            in_=yTsrc[:, tcol, 4 * h:4 * h + 4, :])
    bq_sb = bias_chunks(ph1, "bq_s", 2)
    bk_sb = bias_chunks(ph1, "bk_f", 2)
    bv_b = ph1.tile([P, 2 * P], F32, name="bv_b", tag="bv_b")
    bv_src = d["bv_f"]
    nc.sync.dma_start(out=bv_b[:], in_=bass.AP(
        tensor=bv_src.tensor, offset=bv_src.offset,
        ap=[[0, P]] + list(bv_src.ap)))
    # cross-attention keys prefetched behind yT on the sync queue
    for kc in range(4):
        nc.sync.dma_start(out=xT[kc][:], in_=d["xT"][kc * P:(kc + 1) * P, :])

    # ============ ph1 QKV emission (k full, q tcol 3; rest pending) =====
    # psX: shared 1-bank [P,512] ring for q/v projections, W1 partials and
    # LN1 transposes (keeps psS at bufs=2 within the 8-bank budget)
    psX = open_pool(name="psX", bufs=1, space="PSUM", side="left")
    pending = []
    drained = [0]

    def drain(k):
        for _ in range(min(k, len(pending))):
            pending.pop(0)()
            drained[0] += 1

    def px_tile():
        return psX.tile([P, 512], F32, name="px", tag="px", bufs=2)

    def qk_group(dst, w_sb, b_sb, p, tcol):
        ps = px_tile()
        for kc in range(KC):
            nc.tensor.matmul(ps[:], lhsT=w_sb[:, kc, p * P:(p + 1) * P],
                             rhs=yT_all[:, tcol, kc, :],
                             start=(kc == 0), stop=(kc == KC - 1))
        nc.vector.tensor_scalar(out=dst[p][:, tcol * 512:(tcol + 1) * 512],
                                in0=ps[:], scalar1=b_sb[:, p:p + 1],
                                scalar2=None, op0=ALU.add)

    # k: all tcols (every score block needs all keys); q: tcol 3 first
    for p in range(2):
        for tcol in range(4):
            qk_group(kTp, wk_sb, bk_sb, p, tcol)
    for p in range(2):
        qk_group(qTp, wq_sb, bq_sb, p, 3)

    def v_work(tb):
        work = []
        box = [None]

        def v_start():
            nc.vector.memset(v_sb[tb][:, :, DK:DK + 1], 1.0)
            box[0] = px_tile()

        def v_mm(kc):
            nc.tensor.matmul(box[0][:, 0:2 * P],
                             lhsT=yT_cols(kc, tb * P, P),
                             rhs=wv_sb[:, kc, :],
                             start=(kc == 0), stop=(kc == KC - 1))

        def v_evict():
            nc.vector.tensor_add(
                out=v_sb[tb][:, :, 0:DK],
                in0=box[0][:, 0:2 * P].rearrange("p (h k) -> p h k", h=HL),
                in1=bv_b[:].rearrange("p (h k) -> p h k", h=HL))

        work.append(v_start)
        work.extend(lambda kc=kc: v_mm(kc) for kc in range(KC))
        work.append(v_evict)
        return work

    def q_work(p, tcol):
        work = []
        box = [None]

        def q_start():
            box[0] = px_tile()

        def q_mm(kc):
            nc.tensor.matmul(box[0][:],
                             lhsT=wq_sb[:, kc, p * P:(p + 1) * P],
                             rhs=yT_all[:, tcol, kc, :],
                             start=(kc == 0), stop=(kc == KC - 1))

        def q_evict():
            nc.vector.tensor_scalar(
                out=qTp[p][:, tcol * 512:(tcol + 1) * 512],
                in0=box[0][:], scalar1=bq_sb[:, p:p + 1],
                scalar2=None, op0=ALU.add)

        work.append(q_start)
        work.extend(lambda kc=kc: q_mm(kc) for kc in range(KC))
        work.append(q_evict)
        return work

    # v must be fully projected before the first attnV drains; emit v
    # first, then the remaining q columns.
    for tb in range(TB):
        pending.extend(v_work(tb))
    for tcol in (2, 1, 0):
        for p in range(2):
            pending.extend(q_work(p, tcol))
    ph1_work_n = len(pending)

    # ============ ph2: causal attention + W1 + ReduceScatter ============
    cat = open_pool(name="cat", bufs=1, side="right")     # catT, ph2-long
    catT = [cat.tile([P, T], BF16, name=f"catT{i}", tag=f"catT{i}")
            for i in range(2)]
    w1_sb = cat.tile([P, 2, D], BF16, name="w1", tag="w1")
    nc.gpsimd.dma_start(out=w1_sb[:],
                        in_=d["w1loc"].rearrange("p (c n) -> p c n", c=2))

    dramp = open_pool(name="dramp", bufs=1, space="DRAM", side="left")
    rs_in = [dramp.tile([4 * P, D], BF16, name=f"rsi{i}", tag=f"rsi{i}")
             for i in range(RB)]
    rs_out = [dramp.tile([P, D], BF16, name=f"rso{i}", tag=f"rso{i}")
              for i in range(RB)]

    psS = open_pool(name="psS", bufs=1, space="PSUM", side="left")
    psV = open_pool(name="psV", bufs=1, space="PSUM", side="left")

    def make_attn_work(rc, p, expP):
        nkb = 4 * rc + 4
        work = []
        pa_t = [None, None]

        def start_head(hh):
            pa_t[hh] = psV.tile([DK + 1, 512], F32, name="pa", tag="pa",
                                bufs=2)

        def mm_head(hh, kb):
            hl = 2 * p + hh
            nc.tensor.matmul(pa_t[hh][:], lhsT=v_sb[kb][:, hl, :],
                             rhs=expP[:, kb, hh, :],
                             start=(kb == 0), stop=(kb == nkb - 1))

        den_row = [None]

        def evict_den(hh):
            if hh == 0:
                # f32: RECIPROCAL with bf16 input hits a ~40x slower path
                den_row[0] = ph2w.tile([1, 2 * 512], F32, name="den_row",
                                       tag="den_row", bufs=1)
            nc.vector.tensor_copy(out=den_row[0][:, hh * 512:(hh + 1) * 512],
                                  in_=pa_t[hh][DK:DK + 1, :])

        recB = [None]

        def recip_bcast():
            # plain RECIPROCAL costs ~6.4ns per free element (iterative
            # divide); the approx version is ~5x faster and 18-bit exact
            rec_f = ph2w.tile([1, 2 * 512], F32, name="rec_f",
                              tag="rec_f", bufs=1)
            nc.vector.reciprocal_approx_fast(rec_f[:], den_row[0][:])
            rec_row = ph2w.tile([1, 2 * 512], BF16, name="rec_row",
                                tag="rec_row", bufs=1)
            nc.vector.tensor_copy(out=rec_row[:], in_=rec_f[:])
            # broadcast across partitions via K=1 PE matmuls (gpsimd's
            # partition_broadcast sits behind collectives in that queue);
            # matmul output must be f32, so one 1-bank tile per head
            recB[0] = ph2w.tile([DK, 2 * 512], BF16, name="recB", tag="recB",
                                bufs=1)
            for hh in range(2):
                pb = px_tile()
                nc.tensor.matmul(pb[0:DK, :], lhsT=ones64[:],
                                 rhs=rec_row[:, hh * 512:(hh + 1) * 512],
                                 start=True, stop=True)
                nc.vector.tensor_copy(
                    out=recB[0][:, hh * 512:(hh + 1) * 512],
                    in_=pb[0:DK, :])

        def mul_head(hh):
            nc.vector.tensor_mul(
                out=catT[p][hh * DK:(hh + 1) * DK,
                            rc * 512:(rc + 1) * 512],
                in0=pa_t[hh][0:DK, :],
                in1=recB[0][:, hh * 512:(hh + 1) * 512])

        for hh in range(2):
            work.append(lambda hh=hh: start_head(hh))
            for kb in range(nkb):
                work.append(lambda hh=hh, kb=kb: mm_head(hh, kb))
            work.append(lambda hh=hh: evict_den(hh))
        work.append(recip_bcast)
        work.append(lambda: mul_head(0))
        work.append(lambda: mul_head(1))
        return work

    y_box = {}

    def make_w1_work(rc):
        work = []

        def w1_block(rb, nt, box):
            c0 = rc * 512 + rb * P
            if nt == 0:
                box[0] = ph2w.tile([P, D], BF16, name="a1p", tag="a1p",
                                   bufs=2)
            psw = px_tile()
            for kc2 in range(2):
                nc.tensor.matmul(psw[:],
                                 lhsT=catT[kc2][:, c0:c0 + P],
                                 rhs=w1_sb[:, kc2, nt * 512:(nt + 1) * 512],
                                 start=(kc2 == 0), stop=(kc2 == 1))
            nc.vector.tensor_copy(out=box[0][:, nt * 512:(nt + 1) * 512],
                                  in_=psw[:])
            if nt == 1:
                nc.sync.dma_start(out=rs_in[rc][rb * P:(rb + 1) * P, :],
                                  in_=box[0][:])

        for rb in range(4):
            box = [None]
            for nt in range(2):
                work.append(lambda rb=rb, nt=nt, box=box: w1_block(rb, nt, box))

        def do_rs():
            # the collective is the ONLY gpsimd-queue op in ph2: anything
            # queued behind it stalls on the mesh handshake when peers skew
            nc.gpsimd.collective_compute(
                "ReduceScatter", ALU.add, replica_groups=RG,
                ins=[rs_in[rc][:].opt()], outs=[rs_out[rc][:].opt()])
        work.append(do_rs)
        return work

    def make_ln1_work(rc):
        work = []
        a1 = [None]
        fold = f["ln1_unit_g"] and f["ln1_zero_b"]

        def load():
            # emitted two chunks after the RS trigger, so these sync-queue
            # DMAs never sit blocked on the RS-done semaphore
            yb = ph3.tile([P, D], F32, name="y_sb", tag="y_sb", bufs=1)
            nc.sync.dma_start(out=yb[:],
                              in_=d["y_rows"][rc * P:(rc + 1) * P, :])
            y_box[rc] = yb
            ap = ph3.tile([P, D], BF16, name="a1pre", tag="a1pre", bufs=1)
            nc.sync.dma_start(out=ap[:], in_=rs_out[rc][:])
            a1pre_box[rc] = ap

        def residual():
            a1[0] = ph3.tile([P, D], F32, name="a1", tag="a1", bufs=1)
            nc.vector.tensor_add(out=a1[0][:], in0=a1pre_box[rc][:],
                                 in1=y_box[rc][:])
            if not f["b1_zero"]:
                nc.vector.tensor_add(out=a1[0][:], in0=a1[0][:],
                                     in1=f["b1_b"][:])

        a1b = [None]

        def ln():
            a1b[0] = ph3.tile([P, D], BF16, name="a1b", tag="a1b", bufs=1)
            if fold:
                # rstd folds SCALE -> a1T comes out pre-scaled
                ln_slim(ph3, a1[0][:], a1b[0][:], eps_s2,
                        var_scale=1.0 / (SCALE * SCALE))
            else:
                ln_slim(ph3, a1[0][:], a1b[0][:], eps_sb,
                        g_b=None if f["ln1_unit_g"] else f["g1_b"][:],
                        be_b=None if f["ln1_zero_b"] else f["be1_b"][:])

        def tr(kc):
            pt = px_tile()[:].bitcast(BF16)[:, 0:P]   # bf16 view of psum
            nc.tensor.transpose(pt, a1b[0][:, kc * P:(kc + 1) * P],
                                ident[:])
            if fold:
                nc.vector.tensor_copy(out=a1T[kc][:, rc * P:(rc + 1) * P],
                                      in_=pt)
            else:
                nc.vector.tensor_scalar(
                    out=a1T[kc][:, rc * P:(rc + 1) * P], in0=pt,
                    scalar1=float(SCALE), scalar2=None, op0=ALU.mult)

        work.append(load)
        work.append(residual)
        work.append(ln)
        work.extend(lambda kc=kc: tr(kc) for kc in range(KC))
        return work

    ph1_closed = False
    ln1_q = []
    for rc in (3, 2, 1, 0):
        nkb = 4 * rc + 4
        dn = 7 if rc == 3 else 4
        for p in range(2):
            # expP[kb][i] holds exp(scores) for head 2p+i, keys block kb
            expP = ph2w.tile([P, TB, 2, 512], BF16, name="expP", tag="expP",
                             bufs=2)
            for kb in range(nkb):
                ps = psS.tile([P, 1024], F32, name="ps_sc", tag="ps_sc",
                              bufs=2)
                nc.tensor.matmul(ps[:, 0:512],
                                 lhsT=kTp[p][0:DK, kb * P:(kb + 1) * P],
                                 rhs=qTp[p][0:DK, rc * 512:(rc + 1) * 512],
                                 start=True, stop=True,
                                 tile_position=(0, 0))
                nc.tensor.matmul(ps[:, 512:1024],
                                 lhsT=kTp[p][DK:P, kb * P:(kb + 1) * P],
                                 rhs=qTp[p][DK:P, rc * 512:(rc + 1) * 512],
                                 start=True, stop=True,
                                 tile_position=(64, 0))
                nc.scalar.activation(
                    out=expP[:, kb, :, :],
                    in_=ps[:].rearrange("p (h r) -> p h r", h=2),
                    func=AF.Exp)
                if kb >= 4 * rc:       # diagonal block: apply causal mask
                    i = kb - 4 * rc
                    for hh in range(2):
                        nc.vector.tensor_mul(out=expP[:, kb, hh, :],
                                             in0=expP[:, kb, hh, :],
                                             in1=mask4[:, i, :])
                drain(dn)
            pending.extend(make_attn_work(rc, p, expP))
        # LN1 for chunk rc is emitted TWO chunks later: its first DVE op
        # waits on that chunk's RS (~30-40us after the trigger), and any
        # earlier emission stalls the whole in-order DVE queue behind the
        # collective
        if len(ln1_q) >= 2:
            pending.extend(ln1_q.pop(0))
        pending.extend(make_w1_work(rc))
        ln1_q.append(make_ln1_work(rc))
        if not ph1_closed:
            # all q/v pending work must be emitted before ph1 frees
            ph1_closed = True
            drain(max(0, ph1_work_n - drained[0]))
            close_pool(ph1)
    pending.extend(ln1_q.pop(0))     # LN1 for rc=1
    drain(len(pending))
    close_pool(psV)
    close_pool(psS)
    close_pool(cat)
    close_pool(attn)
    close_pool(ph2w)

    # ================= Phase 4: cross-attention =========================
    # pools open before LN1(rc=0) is emitted, so the early score groups
    # (and their xT loads) slot in AHEAD of it on the PE/sync queues and
    # run during the last ReduceScatter's flight
    fw = open_pool(name="fw", bufs=1, side="left")        # Wf1, lives to FFN1
    wf1_all = fw.tile([P, KC, DF], BF16, name="wf1", tag="wf1")
    at2p = open_pool(name="at2p", bufs=1, side="left")    # at2T, into ph5
    at2T = [at2p.tile([P, R], BF16, name=f"at2T{i}", tag=f"at2T{i}")
            for i in range(KC)]
    w2p = open_pool(name="w2p", bufs=1, side="left")      # W2sum, into ph5
    w2 = [w2p.tile([P, D], BF16, name=f"w2_{i}", tag=f"w2_{i}")
          for i in range(KC)]
    ph4 = open_pool(name="ph4", bufs=1, side="left")
    pp4 = open_pool(name="pp4", bufs=6, space="PSUM", side="left")
    for kc in range(4, KC):
        xT[kc] = ph4.tile([P, T], BF16, name=f"xT{kc}", tag=f"xT{kc}")
        nc.sync.dma_start(out=xT[kc][:],
                          in_=d["xT"][kc * P:(kc + 1) * P, :])
    p2T = [ph4.tile([P, R], BF16, name=f"p2T{i}", tag=f"p2T{i}")
           for i in range(TB)]
    # rows 128:512 of a1T (chunks rc=1..3) are ready long before rc=0's
    # ReduceScatter lands: the first 4 tb score groups accumulate that
    # 3/4 first, hiding PE work under the last RS
    ps_box4 = {}

    def sc4_a(tb):
        ps_box4[tb] = pp4.tile([P, 512], F32, name="ps4", tag="ps4")
        for kc in range(KC):
            nc.tensor.matmul(ps_box4[tb][:, P:512],
                             lhsT=xT[kc][:, tb * P:(tb + 1) * P],
                             rhs=a1T[kc][:, P:512],
                             start=(kc == 0), stop=(kc == KC - 1))

    for tb in range(6):
        sc4_a(tb)
    pending.extend(ln1_q.pop(0))     # LN1 for rc=0 (waits on its RS)
    drain(len(pending))
    close_pool(ph3)
    # x_tm / Wf1 / W2sum ride the sync queue BEHIND the rc=0 RS-result
    # load, so none of this 14MB contends with the last ReduceScatter;
    # x_tm goes first (attn2 needs it ~25us after the boundary)
    xtm = open_pool(name="xtm", bufs=1, side="right")
    x_tm = xtm.tile([P, TB, D], BF16, name="xtm", tag="xtm")
    nc.sync.dma_start(out=x_tm[:],
                      in_=d["x_tm"].rearrange("(t p) d -> p t d", p=P))
    nc.sync.dma_start(out=wf1_all[:],
                      in_=d["Wf1"].rearrange("p (c f) -> p c f", c=KC))
    for kc in range(KC):
        nc.sync.dma_start(out=w2[kc][:],
                          in_=d["W2sum"][kc * P:(kc + 1) * P, :])
    for tb in range(TB):
        if tb < 6:
            ps = ps_box4[tb]
            for kc in range(KC):
                nc.tensor.matmul(ps[:, 0:P],
                                 lhsT=xT[kc][:, tb * P:(tb + 1) * P],
                                 rhs=a1T[kc][:, 0:P],
                                 start=(kc == 0), stop=(kc == KC - 1))
        else:
            ps = pp4.tile([P, 512], F32, name="ps4", tag="ps4")
            for kc in range(KC):
                nc.tensor.matmul(ps[:], lhsT=xT[kc][:, tb * P:(tb + 1) * P],
                                 rhs=a1T[kc][:, :],
                                 start=(kc == 0), stop=(kc == KC - 1))
        nc.scalar.activation(out=p2T[tb][:], in_=ps[:], func=AF.Exp)

    at2_ps = {}

    def attn2_mms(db):
        ps = pp4.tile([P, 512], F32, name="ps4", tag="ps4")
        for tb in range(TB):
            nc.tensor.matmul(ps[:], lhsT=x_tm[:, tb, db * P:(db + 1) * P],
                             rhs=p2T[tb][:],
                             start=(tb == 0), stop=(tb == TB - 1))
        at2_ps[db] = ps

    # two attn2 groups keep the PE busy under the denominator chain
    attn2_mms(0)
    attn2_mms(1)
    # denominator: 4 col-tiled ones-matmul accumulators run concurrently
    # (psum from the px ring -- frees two banks for pp4's deeper pipeline)
    pd = px_tile()
    for g in range(4):
        for u in range(4):
            tb = 4 * g + u
            nc.tensor.matmul(pd[32 * g:32 * g + 1, :], lhsT=ones_col[:],
                             rhs=p2T[tb][:], start=(u == 0), stop=(u == 3),
                             tile_position=(0, 32 * g))
    den4 = ph4.tile([1, 4, R], F32, name="den4", tag="den4")
    for g in range(4):
        nc.vector.tensor_copy(out=den4[:, g, :], in_=pd[32 * g:32 * g + 1, :])
    den2a = ph4.tile([1, R], F32, name="den2a", tag="den2a")
    den2b = ph4.tile([1, R], F32, name="den2b", tag="den2b")
    den2 = ph4.tile([1, R], F32, name="den2", tag="den2")
    nc.vector.tensor_add(out=den2a[:], in0=den4[:, 0, :], in1=den4[:, 1, :])
    nc.vector.tensor_add(out=den2b[:], in0=den4[:, 2, :], in1=den4[:, 3, :])
    nc.vector.tensor_add(out=den2[:], in0=den2a[:], in1=den2b[:])
    recip2f = ph4.tile([1, R], F32, name="recip2f", tag="recip2f")
    nc.vector.reciprocal_approx_fast(recip2f[:], den2[:])
    recip2 = ph4.tile([1, R], BF16, name="recip2", tag="recip2")
    nc.vector.tensor_copy(out=recip2[:], in_=recip2f[:])
    psb2 = px_tile()
    nc.tensor.matmul(psb2[:], lhsT=ones_row[:], rhs=recip2[:],
                     start=True, stop=True)
    recip2b = ph4.tile([P, R], F32, name="recip2b", tag="recip2b")
    nc.vector.tensor_copy(out=recip2b[:], in_=psb2[:])
    for db in range(KC):
        if db >= 2:
            attn2_mms(db)
        nc.vector.tensor_mul(out=at2T[db][:], in0=at2_ps[db][:],
                             in1=recip2b[:])
    close_pool(pp4)
    close_pool(psX)
    close_pool(xtm)
    close_pool(ph4)
    close_pool(a1pl)
    close_pool(xpre)

    # ========= Phase 5: W2sum + residual + LN2, produce a2T =============
    a2p = open_pool(name="a2p", bufs=1, side="right")     # a2T into ph6
    a2T = [a2p.tile([P, R], BF16, name=f"a2T{i}", tag=f"a2T{i}")
           for i in range(KC)]
    ph5 = open_pool(name="ph5", bufs=1, side="right")
    pp5 = open_pool(name="pp5", bufs=4, space="PSUM", side="left")
    pt5 = open_pool(name="pt5", bufs=2, space="PSUM", side="left")
    if not f["b2_zero"]:
        f["b2_b"] = bcast_row(ph5, "b2")
    if not f["ln2_unit_g"]:
        f["g2_b"] = bcast_row(ph5, "ln2_g")
    if not f["ln2_zero_b"]:
        f["be2_b"] = bcast_row(ph5, "ln2_b")
    for rb in range(RB):
        y5 = ph5.tile([P, D], F32, name="y5", tag="y5", bufs=2)
        nc.sync.dma_start(out=y5[:], in_=d["y_rows"][rb * P:(rb + 1) * P, :])
        a2 = ph5.tile([P, D], F32, name="a2", tag="a2", bufs=2)
        for nt in range(2):
            ps = pp5.tile([P, 512], F32, name="ps_a2", tag="ps_a2")
            for kc in range(KC):
                nc.tensor.matmul(ps[:],
                                 lhsT=at2T[kc][:, rb * P:(rb + 1) * P],
                                 rhs=w2[kc][:, nt * 512:(nt + 1) * 512],
                                 start=(kc == 0), stop=(kc == KC - 1))
            sl = slice(nt * 512, (nt + 1) * 512)
            nc.vector.tensor_add(out=a2[:, sl], in0=ps[:], in1=y5[:, sl])
            if not f["b2_zero"]:
                nc.vector.tensor_add(out=a2[:, sl], in0=a2[:, sl],
                                     in1=f["b2_b"][:, sl])
        a2b = ph5.tile([P, D], BF16, name="a2b", tag="a2b", bufs=2)
        ln_slim(ph5, a2[:], a2b[:], eps_sb,
                g_b=None if f["ln2_unit_g"] else f["g2_b"][:],
                be_b=None if f["ln2_zero_b"] else f["be2_b"][:])
        for kc in range(KC):
            pt = pt5.tile([P, P], BF16, name="pt_a2", tag="pt_a2")
            nc.tensor.transpose(pt[:], a2b[:, kc * P:(kc + 1) * P], ident[:])
            nc.vector.tensor_copy(out=a2T[kc][:, rb * P:(rb + 1) * P],
                                  in_=pt[:])
    close_pool(pt5)
    close_pool(pp5)
    close_pool(ph5)
    close_pool(w2p)
    close_pool(at2p)

    # ========== Phase 6: FFN + residual + LN3 ===========================
    fA = open_pool(name="fA", bufs=1, side="right")
    f1T = [fA.tile([P, R], BF16, name=f"f1T{i}", tag=f"f1T{i}")
           for i in range(FB)]
    bf1_sb = bias_chunks(fA, "bf1", FB)
    pfA = open_pool(name="pfA", bufs=3, space="PSUM", side="left")
    for fb in range(FB):
        ps = pfA.tile([P, 512], F32, name="ps_f1", tag="ps_f1")
        for kc in range(KC):
            nc.tensor.matmul(ps[:], lhsT=wf1_all[:, kc, fb * P:(fb + 1) * P],
                             rhs=a2T[kc][:, :],
                             start=(kc == 0), stop=(kc == KC - 1))
        # relu + bias on ACT (idle during the FFN)
        nc.scalar.activation(out=f1T[fb][:], in_=ps[:], func=AF.Relu,
                             bias=bf1_sb[:, fb:fb + 1], scale=1.0)
    close_pool(pfA)
    close_pool(fw)

    pfB = open_pool(name="pfB", bufs=1, space="PSUM", side="left")
    fB = open_pool(name="fB", bufs=1, side="right")
    ps_rb = [pfB.tile([P, D], F32, name=f"ps_rb{i}", tag=f"ps_rb{i}")
             for i in range(RB)]
    y6 = [fB.tile([P, D], F32, name=f"y6{i}", tag=f"y6{i}")
          for i in range(RB)]
    for rb in range(RB):
        nc.sync.dma_start(out=y6[rb][:],
                          in_=d["y_rows"][rb * P:(rb + 1) * P, :])
    if not f["bf2_zero"]:
        f["bf2_b"] = bcast_row(fB, "bf2")
    if not f["ln3_unit_g"]:
        f["g3_b"] = bcast_row(fB, "ln3_g")
    if not f["ln3_zero_b"]:
        f["be3_b"] = bcast_row(fB, "ln3_b")
    wf2_t = {}
    for fb in range(FB):
        wf2_fb = fB.tile([P, D], BF16, name="wf2s", tag="wf2s", bufs=11)
        nc.sync.dma_start(out=wf2_fb[:], in_=d["Wf2"][fb * P:(fb + 1) * P, :])
        wf2_t[fb] = wf2_fb
        if fb < FB - 8:
            for rb in range(RB):
                for nt in range(2):
                    nc.tensor.matmul(
                        ps_rb[rb][:, nt * 512:(nt + 1) * 512],
                        lhsT=f1T[fb][:, rb * P:(rb + 1) * P],
                        rhs=wf2_fb[:, nt * 512:(nt + 1) * 512],
                        start=(fb == 0), stop=False)

    def tail(rb):
        ff = fB.tile([P, D], F32, name="ff", tag="ff", bufs=2)
        nc.vector.tensor_add(out=ff[:], in0=ps_rb[rb][:], in1=y6[rb][:])
        if not f["bf2_zero"]:
            nc.vector.tensor_add(out=ff[:], in0=ff[:], in1=f["bf2_b"][:])
        o = fB.tile([P, D], F32, name="o", tag="o", bufs=2)
        ln_slim(fB, ff[:], o[:], eps_sb,
                g_b=None if f["ln3_unit_g"] else f["g3_b"][:],
                be_b=None if f["ln3_zero_b"] else f["be3_b"][:])
        nc.sync.dma_start(out=out_d[rb * P:(rb + 1) * P, :], in_=o[:])

    # last 8 fb row-major: each row block finishes ~10us early and its
    # LN3+store overlaps the remaining matmuls
    for rb in range(RB):
        for fb in range(FB - 8, FB):
            for nt in range(2):
                nc.tensor.matmul(ps_rb[rb][:, nt * 512:(nt + 1) * 512],
                                 lhsT=f1T[fb][:, rb * P:(rb + 1) * P],
                                 rhs=wf2_t[fb][:, nt * 512:(nt + 1) * 512],
                                 start=False, stop=(fb == FB - 1))
        tail(rb)
    close_pool(fB)
    close_pool(pfB)
    close_pool(fA)
    close_pool(a2p)
    close_pool(dramp)
    close_pool(const)


def _row_idx(j):
    return np.concatenate(
        [np.arange(512 * rc + 128 * j, 512 * rc + 128 * j + 128)
         for rc in range(4)])


def _flags(inputs):
    z = lambda a: bool(np.all(np.asarray(a) == 0.0))
    u = lambda a: bool(np.all(np.asarray(a) == 1.0))
    return {
        "b1_zero": z(inputs["b1"]), "b2_zero": z(inputs["b2"]),
        "bf2_zero": z(inputs["bf2"]),
        "ln1_unit_g": u(inputs["ln1_g"]), "ln1_zero_b": z(inputs["ln1_b"]),
        "ln2_unit_g": u(inputs["ln2_g"]), "ln2_zero_b": z(inputs["ln2_b"]),
        "ln3_unit_g": u(inputs["ln3_g"]), "ln3_zero_b": z(inputs["ln3_b"]),
    }


def _prep_host(inputs):
    f32 = lambda a: np.ascontiguousarray(np.asarray(a, np.float32))
    bf = lambda a: np.ascontiguousarray(
        np.asarray(a, np.float32).astype(ml_dtypes.bfloat16))
    x = f32(inputs["x"])
    y = f32(inputs["y"])
    mask = np.asarray(inputs["y_mask"]).astype(np.float32)
    # diagonal-block masks: mask4[ky, i, r] = mask[r, 128*i + ky]
    m4 = mask[0:512, 0:512].reshape(512, 4, 128).transpose(2, 1, 0)
    Wq = f32(inputs["Wq"])   # [H, D, DK]
    Wk = f32(inputs["Wk"])
    Wv = f32(inputs["Wv"])

    def chunkP(a):
        """[C*P, F] -> [P, C*F] so each partition's data is contiguous."""
        cp, fdim = a.shape
        return np.ascontiguousarray(
            a.reshape(cp // P, P, fdim).transpose(1, 0, 2).reshape(P, -1))

    shared = {
        "mask4": bf(m4),
        "b1": f32(inputs["b1"]),
        "ln1_g": f32(inputs["ln1_g"]), "ln1_b": f32(inputs["ln1_b"]),
        "W2sum": bf(f32(inputs["W2"]).reshape(H, D, D).sum(0)),
        "b2": f32(inputs["b2"]),
        "ln2_g": f32(inputs["ln2_g"]), "ln2_b": f32(inputs["ln2_b"]),
        "Wf1": chunkP(bf(inputs["Wf1"])),
        "bf1": f32(inputs["bf1"]),
        "Wf2": bf(inputs["Wf2"]),
        "bf2": f32(inputs["bf2"]),
        "ln3_g": f32(inputs["ln3_g"]), "ln3_b": f32(inputs["ln3_b"]),
    }
    in_maps = []
    for c in range(NCORES):
        b, j = c // 4, c % 4
        hh = slice(4 * j, 4 * j + 4)
        ridx = _row_idx(j)
        in_maps.append({
            "yT": np.ascontiguousarray(
                bf(y[b].T).reshape(KC, P, 4, 512)
                .transpose(1, 2, 0, 3).reshape(P, -1)),
            "wq": chunkP(bf(Wq[hh].transpose(1, 0, 2).reshape(D, 256) * SCALE)),
            "wk": chunkP(bf(Wk[hh].transpose(1, 0, 2).reshape(D, 256))),
            "wv": chunkP(bf(Wv[hh].transpose(1, 0, 2).reshape(D, 256))),
            "bq_s": f32(inputs["bq"])[hh].reshape(256) * np.float32(SCALE),
            "bk_f": f32(inputs["bk"])[hh].reshape(256),
            "bv_f": f32(inputs["bv"])[hh].reshape(256),
            "w1loc": chunkP(bf(f32(inputs["W1"])[256 * j:256 * (j + 1), :])),
            "y_rows": np.ascontiguousarray(y[b][ridx]),
            "xT": bf(x[b].T),
            "x_tm": bf(x[b]),
            **shared,
        })
    return in_maps


def kernel(**inputs):
    fl = _flags(inputs)
    key = tuple(sorted(fl.items()))
    if key not in _cached:
        _cached[key] = build_nc(dict(fl))
    nc = _cached[key]
    in_maps = _prep_host(inputs)
    res = run_bass_kernel_spmd(nc, in_maps, core_ids=list(range(NCORES)))
    out = np.zeros((B, S, D), np.float32)
    for c in range(NCORES):
        b, j = c // 4, c % 4
        out[b, _row_idx(j)] = res.results[c]["out"]
    return out
